# revision 1
# baseline (speedup 1.0000x reference)
"""GQA kernel for Trainium2, 8 NeuronCores.

Sharding: core c = b*4 + g  handles batch b, kv-head g (4 query heads).
Each core computes:
  Q_g^T = Wq_g @ x_q^T        [4 heads][128, S]   (scale 1/sqrt(D) folded in)
  K_g^T = Wk_g @ x_k^T        [128, S]
  V_g   = (x_v @ Wv_g.T)      [S, 128]  (via V^T then PE transpose)
  S^T   = K tile @ Q^T        [k,q] orientation -> +mask (diag) -> exp
  o^T  += V[kt] matmul P~^T   (PSUM accum), l += ones^T P~^T
  o_norm^T = o^T * recip(bcast l)
  partial = o_norm @ Wo_g.T   [S, E]
Host sums the 4 partials per batch.

Matmuls run in bf16 (fp32 PSUM accumulation): 4-byte dtypes serialize
LDWEIGHTS with the matmul (~191ns per 128x128 load, no FWL/prefetch),
which was ~37% of the kernel span in fp32r. l is broadcast across
partitions with a K=1 matmul so the reciprocal runs at full DVE lane
width ([128,512] not [1,512]).
"""

import sys

import numpy as np

for _p in ("/opt/trn_rl_repo",):
    if _p not in sys.path:
        sys.path.insert(0, _p)

import ml_dtypes

import concourse.bass as bass
import concourse.mybir as mybir
from concourse import bacc
from concourse.bass_utils import run_bass_kernel_spmd
from concourse.masks import make_identity
from concourse.tile import TileContext

B, S, E = 2, 2048, 2048
H, HKV = 16, 4
D = E // H  # 128
G = H // HKV  # 4 query heads per kv head
GD = G * D  # 512
NCORES = B * HKV  # 8
SC = 512  # s/q chunk width (free dim of matmuls)
NSC = S // SC  # 4
NET = E // 128  # 16 e-tiles (contraction)
NKT = S // 128  # 16 k-tiles
SCALE = 1.0 / float(np.sqrt(D))

F32 = mybir.dt.float32
BF16 = mybir.dt.bfloat16
F32R = mybir.dt.float32r
AF = mybir.ActivationFunctionType
NPBF = np.dtype(ml_dtypes.bfloat16)


def build_nc():
    nc = bacc.Bacc()
    xq = nc.declare_dram_parameter("xq", [E, S], BF16, isOutput=False)  # query[b].T
    xk = nc.declare_dram_parameter("xk", [E, S], BF16, isOutput=False)  # key[b].T
    xv = nc.declare_dram_parameter("xv", [E, S], BF16, isOutput=False)  # value[b].T
    wq = nc.declare_dram_parameter("wq", [E, GD], BF16, isOutput=False)
    wk = nc.declare_dram_parameter("wk", [E, D], BF16, isOutput=False)
    wv = nc.declare_dram_parameter("wv", [E, D], BF16, isOutput=False)
    wo = nc.declare_dram_parameter("wo", [GD, E], BF16, isOutput=False)
    msk = nc.declare_dram_parameter("msk", [4 * 128, SC], F32, isOutput=False)
    out = nc.declare_dram_parameter("out", [S, E], F32, isOutput=True)

    with TileContext(nc) as tc:
        with (
            tc.tile_pool(name="singles", bufs=1) as singles,
            tc.tile_pool(name="xt", bufs=24) as xtp,
            tc.tile_pool(name="pexp", bufs=4) as pexp,
            tc.tile_pool(name="small", bufs=2) as small,
            tc.tile_pool(name="ob", bufs=3) as obp,
            tc.tile_pool(name="acc", bufs=4, space="PSUM") as acc,
            tc.tile_pool(name="ops", bufs=2, space="PSUM") as ops,
            tc.tile_pool(name="lps", bufs=1, space="PSUM") as lps,
            tc.tile_pool(name="trp", bufs=1, space="PSUM") as trp,
            tc.tile_pool(name="drp", bufs=2, space="DRAM") as drp,
        ):
            # ---- constants / weights resident in SBUF ----
            wq_sb = singles.tile([128, NET, GD], BF16)  # 16KB/p
            wk_sb = singles.tile([128, NET, D], BF16)  # 4KB/p
            wv_sb = singles.tile([128, NET, D], BF16)  # 4KB/p
            wo_sb = singles.tile([128, G, E], BF16)  # 16KB/p
            mask_sb = singles.tile([128, 4, SC], F32)  # 8KB/p
            ident_f = singles.tile([128, 128], F32)
            ident = singles.tile([128, 128], BF16)
            ones_f = singles.tile([128, 1], F32)
            ones = singles.tile([128, 1], BF16)
            qT = singles.tile([128, G, S], BF16)  # 16KB/p
            kT = singles.tile([128, S], BF16)  # 4KB/p
            v_sb = singles.tile([128, NKT, D], BF16)  # 4KB/p
            onrm = singles.tile([128, G, S], BF16)  # 16KB/p
            o_unn = singles.tile([128, G, S], F32)  # 32KB/p

            make_identity(nc, ident_f)
            nc.scalar.activation(out=ident[:], in_=ident_f[:], func=AF.Copy)
            nc.vector.memset(ones_f, 1.0)
            nc.scalar.activation(out=ones[:], in_=ones_f[:], func=AF.Copy)
            for t in range(NET):
                nc.sync.dma_start(
                    out=wq_sb[:, t, :], in_=wq[t * 128 : (t + 1) * 128, :]
                )
                nc.sync.dma_start(out=wk_sb[:, t, :], in_=wk[t * 128 : (t + 1) * 128, :])
                nc.sync.dma_start(out=wv_sb[:, t, :], in_=wv[t * 128 : (t + 1) * 128, :])
            for h in range(G):
                nc.sync.dma_start(
                    out=wo_sb[:, h, :], in_=wo[h * 128 : (h + 1) * 128, :]
                )
            for j in range(4):
                nc.sync.dma_start(
                    out=mask_sb[:, j, :], in_=msk[j * 128 : (j + 1) * 128, :]
                )

            # ---- phase 1: projections ----
            for sc in range(NSC):
                ssl = slice(sc * SC, (sc + 1) * SC)
                # Q^T: 4 heads
                xts = []
                for t in range(NET):
                    xt = xtp.tile([128, SC], BF16, tag="xt")
                    nc.sync.dma_start(out=xt, in_=xq[t * 128 : (t + 1) * 128, ssl])
                    xts.append(xt)
                for h in range(G):
                    ps = acc.tile([128, SC], F32, tag="acc")
                    for t in range(NET):
                        nc.tensor.matmul(
                            ps[:],
                            lhsT=wq_sb[:, t, h * D : (h + 1) * D],
                            rhs=xts[t][:],
                            start=(t == 0),
                            stop=(t == NET - 1),
                        )
                    # fold softmax scale into Q
                    nc.scalar.activation(
                        out=qT[:, h, ssl], in_=ps[:], func=AF.Copy, scale=SCALE
                    )
                # K^T
                xts = []
                for t in range(NET):
                    xt = xtp.tile([128, SC], BF16, tag="xt")
                    nc.sync.dma_start(out=xt, in_=xk[t * 128 : (t + 1) * 128, ssl])
                    xts.append(xt)
                ps = acc.tile([128, SC], F32, tag="acc")
                for t in range(NET):
                    nc.tensor.matmul(
                        ps[:],
                        lhsT=wk_sb[:, t, :],
                        rhs=xts[t][:],
                        start=(t == 0),
                        stop=(t == NET - 1),
                    )
                nc.vector.tensor_copy(out=kT[:, ssl], in_=ps[:])
                # V^T then transpose to V [s, d]
                xts = []
                for t in range(NET):
                    xt = xtp.tile([128, SC], BF16, tag="xt")
                    nc.sync.dma_start(out=xt, in_=xv[t * 128 : (t + 1) * 128, ssl])
                    xts.append(xt)
                ps = acc.tile([128, SC], F32, tag="acc")
                for t in range(NET):
                    nc.tensor.matmul(
                        ps[:],
                        lhsT=wv_sb[:, t, :],
                        rhs=xts[t][:],
                        start=(t == 0),
                        stop=(t == NET - 1),
                    )
                vt_tmp = small.tile([128, SC], BF16, tag="vt")
                nc.scalar.activation(out=vt_tmp[:], in_=ps[:], func=AF.Copy)
                for i in range(SC // 128):
                    tp = trp.tile([128, 128], BF16, tag="tr")
                    nc.tensor.transpose(
                        tp[:], vt_tmp[:, i * 128 : (i + 1) * 128], ident[:]
                    )
                    nc.vector.tensor_copy(out=v_sb[:, sc * 4 + i, :], in_=tp[:])

            # ---- phase 2+3: attention, outproj interleaved per q-chunk ----
            for qc in range(NSC):
                for h in range(G):
                    qsl = slice(qc * SC, (qc + 1) * SC)
                    nkt = (qc + 1) * (SC // 128)  # causal: k tiles 0..nkt-1
                    o_ps = ops.tile([128, SC], F32, tag="o")
                    l_ps = lps.tile([1, SC], F32, tag="l")
                    for kt in range(nkt):
                        s_ps = acc.tile([128, SC], F32, tag="acc")
                        nc.tensor.matmul(
                            s_ps[:],
                            lhsT=kT[:, kt * 128 : (kt + 1) * 128],
                            rhs=qT[:, h, qsl],
                            start=True,
                            stop=True,
                        )
                        if kt >= nkt - 4:
                            j = kt - 4 * qc
                            nc.vector.tensor_add(s_ps[:], s_ps[:], mask_sb[:, j, :])
                        p_sb = pexp.tile([128, SC], BF16, tag="p")
                        nc.scalar.activation(out=p_sb[:], in_=s_ps[:], func=AF.Exp)
                        nc.tensor.matmul(
                            o_ps[:],
                            lhsT=v_sb[:, kt, :],
                            rhs=p_sb[:],
                            start=(kt == 0),
                            stop=(kt == nkt - 1),
                        )
                        nc.tensor.matmul(
                            l_ps[:],
                            lhsT=ones[:],
                            rhs=p_sb[:],
                            start=(kt == 0),
                            stop=(kt == nkt - 1),
                        )
                    # l broadcast across partitions via K=1 matmul, then
                    # reciprocal at full lane width and normalize.
                    nc.scalar.activation(
                        out=o_unn[:, h, qsl], in_=o_ps[:], func=AF.Copy
                    )
                    l_sb = small.tile([1, SC], F32, tag="lsb")
                    nc.scalar.activation(out=l_sb[:], in_=l_ps[:], func=AF.Copy)
                    l_dr = drp.tile([1, SC], F32, tag="ldr")
                    nc.sync.dma_start(out=l_dr[:], in_=l_sb[:])
                    lb = small.tile([128, SC], F32, tag="lb")
                    l_bc = bass.AP(
                        tensor=l_dr[:].tensor,
                        offset=l_dr[:].offset,
                        ap=[[0, 128]] + list(l_dr[:].ap[1:]),
                    )
                    nc.sync.dma_start(out=lb[:], in_=l_bc)
                    rb = small.tile([128, SC], F32, tag="rb")
                    nc.vector.reciprocal(out=rb[:], in_=lb[:])
                    nc.vector.tensor_mul(
                        onrm[:, h, qsl], o_unn[:, h, qsl], rb[:]
                    )

                # output projection for this q-chunk's 4 s-tiles
                for sti in range(SC // 128):
                    st = qc * (SC // 128) + sti
                    stl = slice(st * 128, (st + 1) * 128)
                    for ec in range(E // SC):
                        esl = slice(ec * SC, (ec + 1) * SC)
                        ps = acc.tile([128, SC], F32, tag="acc")
                        for h in range(G):
                            nc.tensor.matmul(
                                ps[:],
                                lhsT=onrm[:, h, stl],
                                rhs=wo_sb[:, h, esl],
                                start=(h == 0),
                                stop=(h == G - 1),
                            )
                        ob = obp.tile([128, SC], F32, tag="ob")
                        nc.scalar.activation(out=ob[:], in_=ps[:], func=AF.Copy)
                        nc.sync.dma_start(out=out[stl, esl], in_=ob[:])
    nc.compile()
    return nc


_NC_CACHE = None


def _get_nc():
    global _NC_CACHE
    if _NC_CACHE is None:
        _NC_CACHE = build_nc()
    return _NC_CACHE


def _prep_in_maps(query, key, value, attn_mask, Wq, Wk, Wv, Wo):
    query = np.asarray(query, dtype=np.float32)
    key = np.asarray(key, dtype=np.float32)
    value = np.asarray(value, dtype=np.float32)
    Wq = np.asarray(Wq, dtype=np.float32)
    Wk = np.asarray(Wk, dtype=np.float32)
    Wv = np.asarray(Wv, dtype=np.float32)
    Wo = np.asarray(Wo, dtype=np.float32)
    am = np.asarray(attn_mask)

    xqT = [np.ascontiguousarray(query[b].T).astype(NPBF) for b in range(B)]
    xkT = [np.ascontiguousarray(key[b].T).astype(NPBF) for b in range(B)]
    xvT = [np.ascontiguousarray(value[b].T).astype(NPBF) for b in range(B)]

    # 4 diagonal mask tiles [128, SC]: tile j covers k in [j*128,(j+1)*128)
    # relative to the q-chunk start; additive -1e9 on masked entries.
    m0 = np.asarray(am[0, 0, :SC, :SC], dtype=np.float32)  # [q, k] for chunk 0
    msk_tiles = np.zeros((4 * 128, SC), dtype=np.float32)
    for j in range(4):
        msk_tiles[j * 128 : (j + 1) * 128, :] = (
            m0[:, j * 128 : (j + 1) * 128].T - 1.0
        ) * 1e9
    in_maps = []
    for b in range(B):
        for g in range(HKV):
            in_maps.append(
                {
                    "xq": xqT[b],
                    "xk": xkT[b],
                    "xv": xvT[b],
                    "wq": np.ascontiguousarray(
                        Wq[g * GD : (g + 1) * GD, :].T
                    ).astype(NPBF),
                    "wk": np.ascontiguousarray(
                        Wk[g * D : (g + 1) * D, :].T
                    ).astype(NPBF),
                    "wv": np.ascontiguousarray(
                        Wv[g * D : (g + 1) * D, :].T
                    ).astype(NPBF),
                    "wo": np.ascontiguousarray(
                        Wo[:, g * GD : (g + 1) * GD].T
                    ).astype(NPBF),
                    "msk": msk_tiles,
                }
            )
    return in_maps


def _run(inputs, trace=False, **kw):
    nc = _get_nc()
    in_maps = _prep_in_maps(**inputs)
    res = run_bass_kernel_spmd(
        nc, in_maps, list(range(NCORES)), trace=trace, **kw
    )
    outs = [np.asarray(r["out"]) for r in res.results]
    full = np.empty((B, S, E), dtype=np.float32)
    for b in range(B):
        acc = outs[b * HKV].astype(np.float32)
        for g in range(1, HKV):
            acc = acc + outs[b * HKV + g]
        full[b] = acc
    return full, res


def kernel(**inputs):
    full, _ = _run(inputs, trace=False)
    return full



# revision 2
# speedup vs baseline: 1.2532x; 1.2532x over previous
"""GQA kernel for Trainium2, 8 NeuronCores.

Sharding: core c = b*4 + g handles batch b, kv-head g (4 query heads).
Host sums the 4 partial out-projections per batch.

Design notes (v2):
- All matmuls bf16 (fp32 PSUM). PE cost is free-dim rows x clock, and the
  clock p-state ramps to 2.4GHz only after ~3us of *continuous* PE busy,
  so the emission order is built to never let the PE idle:
  projections, attention and out-projection chunks are interleaved so x
  DMAs stream under compute, and the out-projection of chunk qc is
  interleaved into the first head's kt-loop of chunk qc+1.
- Diagonal score tiles are column-trimmed: tile j of a q-chunk only
  touches q columns >= 128j (the rest is fully masked), saving ~15us PE
  and ~10us ACT. The first flushed PV/l matmul covers all 512 columns so
  PSUM start=True initializes the full accumulator.
- Softmax normalization without the DRAM round trip: l row-sums
  accumulate via ones-matmuls per kt; then l -> bf16 copy (ACT), K=1
  matmul broadcast across partitions (PE, 213ns), reciprocal_approx_fast
  (DVE, ~0.7us vs 3.3us for reciprocal), multiply into onrm.
- x inputs are host-bf16, tile-blocked [sc, t, 128, 512] so every DMA is
  one contiguous 128KB block (the old column-sliced DMAs fragmented into
  1KB descriptors, ~60ns each on the queues).
- Output is written bf16 tile-blocked and reassembled/cast on host.
"""

import sys

import numpy as np

for _p in ("/opt/trn_rl_repo",):
    if _p not in sys.path:
        sys.path.insert(0, _p)

import ml_dtypes

import concourse.bass as bass  # noqa: F401  (AP tricks if needed)
import concourse.mybir as mybir
from concourse import bacc
from concourse.bass_utils import run_bass_kernel_spmd
from concourse.masks import make_identity
from concourse.tile import TileContext

B, S, E = 2, 2048, 2048
H, HKV = 16, 4
D = E // H  # 128
G = H // HKV  # 4 query heads per kv head
GD = G * D  # 512
NCORES = B * HKV  # 8
SC = 512  # s/q chunk width (free dim of matmuls)
NSC = S // SC  # 4
NET = E // 128  # 16 e-tiles (contraction)
NKT = S // 128  # 16 k-tiles
NST = S // 128  # 16 s-tiles for output
NEC = E // SC  # 4 e-chunks for output
SCALE = 1.0 / float(np.sqrt(D))

F32 = mybir.dt.float32
BF16 = mybir.dt.bfloat16
AF = mybir.ActivationFunctionType
NPBF = np.dtype(ml_dtypes.bfloat16)


def build_nc():
    nc = bacc.Bacc()
    # x inputs: host-transposed [E, S] then tile-blocked (sc, t) so each
    # [128, SC] tile is one contiguous 128KB DMA.
    xq = nc.declare_dram_parameter("xq", [NSC * NET * 128, SC], BF16, isOutput=False)
    xk = nc.declare_dram_parameter("xk", [NSC * NET * 128, SC], BF16, isOutput=False)
    xv = nc.declare_dram_parameter("xv", [NSC * NET * 128, SC], BF16, isOutput=False)
    # weights: full-width rows are already contiguous per 128-row tile
    wq = nc.declare_dram_parameter("wq", [E, GD], BF16, isOutput=False)
    wk = nc.declare_dram_parameter("wk", [E, D], BF16, isOutput=False)
    wv = nc.declare_dram_parameter("wv", [E, D], BF16, isOutput=False)
    wo = nc.declare_dram_parameter("wo", [GD, E], BF16, isOutput=False)
    msk = nc.declare_dram_parameter("msk", [4 * 128, SC], F32, isOutput=False)
    # output tile-blocked (st, ec)
    out = nc.declare_dram_parameter("out", [NST * NEC * 128, SC], BF16, isOutput=True)

    with TileContext(nc) as tc:
        with (
            tc.tile_pool(name="singles", bufs=1) as singles,
            tc.tile_pool(name="xt", bufs=32) as xtp,
            tc.tile_pool(name="pexp", bufs=4) as pexp,
            tc.tile_pool(name="small", bufs=2) as small,
            tc.tile_pool(name="ob", bufs=3) as obp,
            tc.tile_pool(name="acc", bufs=3, space="PSUM") as acc,
            tc.tile_pool(name="ops", bufs=2, space="PSUM") as ops,
            tc.tile_pool(name="lps", bufs=2, space="PSUM") as lps,
            tc.tile_pool(name="misc", bufs=1, space="PSUM") as misc,
        ):
            # ---- constants / weights resident in SBUF ----
            wq_sb = singles.tile([128, NET, GD], BF16)  # 16KB/p
            wk_sb = singles.tile([128, NET, D], BF16)  # 4KB/p
            wv_sb = singles.tile([128, NET, D], BF16)  # 4KB/p
            wo_sb = singles.tile([128, G, E], BF16)  # 16KB/p
            mask_sb = singles.tile([128, 4, SC], F32)  # 8KB/p
            ident_f = singles.tile([128, 128], F32)
            ident = singles.tile([128, 128], BF16)
            onesc_f = singles.tile([128, 1], F32)
            onesc = singles.tile([128, 1], BF16)  # l-matmul lhsT
            onesr_f = singles.tile([1, 128], F32)
            onesr = singles.tile([1, 128], BF16)  # bcast-matmul lhsT
            qT = singles.tile([128, G, S], BF16)  # 16KB/p
            kT = singles.tile([128, S], BF16)  # 4KB/p
            v_sb = singles.tile([128, NKT, D], BF16)  # 4KB/p
            onrm = singles.tile([128, G, S], BF16)  # 16KB/p

            make_identity(nc, ident_f)
            nc.scalar.activation(out=ident[:], in_=ident_f[:], func=AF.Copy)
            nc.vector.memset(onesc_f, 1.0)
            nc.scalar.activation(out=onesc[:], in_=onesc_f[:], func=AF.Copy)
            nc.vector.memset(onesr_f, 1.0)
            nc.scalar.activation(out=onesr[:], in_=onesr_f[:], func=AF.Copy)

            # K/V weights first: K-proj of chunk 0 can start after ~2 tiles
            for t in range(NET):
                nc.sync.dma_start(out=wk_sb[:, t, :], in_=wk[t * 128 : (t + 1) * 128, :])
            for t in range(NET):
                nc.sync.dma_start(out=wv_sb[:, t, :], in_=wv[t * 128 : (t + 1) * 128, :])

            state = {}  # (qc, h) -> (o_ps, l_ps)

            def proj_block(sc):
                ssl = slice(sc * SC, (sc + 1) * SC)
                base = sc * NET * 128

                def x_tiles(src):
                    xts = []
                    for t in range(NET):
                        xt = xtp.tile([128, SC], BF16, tag="xt", name="xt")
                        r0 = base + t * 128
                        nc.sync.dma_start(out=xt, in_=src[r0 : r0 + 128, :])
                        xts.append(xt)
                    return xts

                # K projection
                xts = x_tiles(xk)
                ps = acc.tile([128, SC], F32, tag="acc", name="kps")
                for t in range(NET):
                    nc.tensor.matmul(
                        ps[:], lhsT=wk_sb[:, t, :], rhs=xts[t][:],
                        start=(t == 0), stop=(t == NET - 1),
                    )
                nc.vector.tensor_copy(out=kT[:, ssl], in_=ps[:])
                # V projection, then transpose to [s, d] tiles
                xts = x_tiles(xv)
                ps = acc.tile([128, SC], F32, tag="acc", name="vps")
                for t in range(NET):
                    nc.tensor.matmul(
                        ps[:], lhsT=wv_sb[:, t, :], rhs=xts[t][:],
                        start=(t == 0), stop=(t == NET - 1),
                    )
                vt = small.tile([128, SC], BF16, tag="vt", name="vt")
                nc.scalar.activation(out=vt[:], in_=ps[:], func=AF.Copy)
                for i in range(SC // 128):
                    tp = misc.tile([128, 128], BF16, tag="mx", name="tp")
                    nc.tensor.transpose(tp[:], vt[:, i * 128 : (i + 1) * 128], ident[:])
                    nc.vector.tensor_copy(out=v_sb[:, sc * 4 + i, :], in_=tp[:])
                # Q projection (4 heads); wq streamed during K/V of chunk 0
                if sc == 0:
                    for t in range(NET):
                        nc.sync.dma_start(
                            out=wq_sb[:, t, :], in_=wq[t * 128 : (t + 1) * 128, :]
                        )
                xts = x_tiles(xq)
                for h in range(G):
                    ps = acc.tile([128, SC], F32, tag="acc", name="qps")
                    for t in range(NET):
                        nc.tensor.matmul(
                            ps[:], lhsT=wq_sb[:, t, h * D : (h + 1) * D], rhs=xts[t][:],
                            start=(t == 0), stop=(t == NET - 1),
                        )
                    # fold softmax scale into Q
                    nc.scalar.activation(
                        out=qT[:, h, ssl], in_=ps[:], func=AF.Copy, scale=SCALE
                    )

            def attn_head(qc, h, extras=None, start_iter=0):
                nkt = 4 * (qc + 1)
                # diagonal tiles first: the j=0 tile covers all 512 columns,
                # so the first PV/l flush (start=True) initializes the full
                # accumulator; later tiles only touch their live columns.
                order = [(nkt - 4 + j, j) for j in range(4)] + [
                    (kt, None) for kt in range(nkt - 4)
                ]
                n = len(order)
                o_ps = ops.tile([128, SC], F32, tag="o", name="o_ps")
                l_ps = lps.tile([1, SC], F32, tag="l", name="l_ps")
                state[(qc, h)] = (o_ps, l_ps)
                pend = []
                nfl = [0]

                def flush_one():
                    kt, cl, p = pend.pop(0)
                    first = nfl[0] == 0
                    last = nfl[0] == n - 1
                    nc.tensor.matmul(
                        o_ps[:, cl], lhsT=v_sb[:, kt, :], rhs=p[:, cl],
                        start=first, stop=last, skip_group_check=True,
                    )
                    nc.tensor.matmul(
                        l_ps[:, cl], lhsT=onesc[:, :], rhs=p[:, cl],
                        start=first, stop=last, skip_group_check=True,
                    )
                    nfl[0] += 1

                for i, (kt, j) in enumerate(order):
                    cl = slice(128 * j, SC) if j is not None else slice(0, SC)
                    qsl = slice(qc * SC + cl.start, (qc + 1) * SC)
                    s_ps = acc.tile([128, SC], F32, tag="acc", name="s_ps")
                    nc.tensor.matmul(
                        s_ps[:, cl], lhsT=kT[:, kt * 128 : (kt + 1) * 128],
                        rhs=qT[:, h, qsl], start=True, stop=True,
                    )
                    if j is not None:
                        nc.vector.tensor_add(s_ps[:, cl], s_ps[:, cl], mask_sb[:, j, cl])
                    p = pexp.tile([128, SC], BF16, tag="p", name="p")
                    nc.scalar.activation(out=p[:, cl], in_=s_ps[:, cl], func=AF.Exp)
                    pend.append((kt, cl, p))
                    if extras is not None and i >= start_iter:
                        for _ in range(2):
                            ex = next(extras, None)
                            if ex is not None:
                                ex()
                    if i >= 2:
                        flush_one()
                while pend:
                    flush_one()

            def norm(qc, h):
                o_ps, l_ps = state.pop((qc, h))
                qsl = slice(qc * SC, (qc + 1) * SC)
                l_bf = small.tile([1, SC], BF16, tag="lbf", name="l_bf")
                nc.scalar.activation(out=l_bf[:], in_=l_ps[:], func=AF.Copy)
                bc = misc.tile([128, SC], F32, tag="mx", name="bc")
                nc.tensor.matmul(bc[:], lhsT=onesr[:, :], rhs=l_bf[:], start=True, stop=True)
                o_unn = small.tile([128, SC], F32, tag="ou", name="o_unn")
                nc.scalar.activation(out=o_unn[:], in_=o_ps[:], func=AF.Copy)
                rinv = small.tile([128, SC], F32, tag="ri", name="rinv")
                nc.vector.reciprocal_approx_fast(out=rinv[:], in_=bc[:])
                nc.vector.tensor_mul(onrm[:, h, qsl], o_unn[:], rinv[:])

            def op_gen(qc):
                # out-projection of chunk qc: 16 chains of 4 matmuls
                for sti in range(4):
                    st = qc * 4 + sti
                    stl = slice(st * 128, (st + 1) * 128)
                    for ec in range(NEC):
                        esl = slice(ec * SC, (ec + 1) * SC)

                        def chain(stl=stl, esl=esl, st=st, ec=ec):
                            ps = acc.tile([128, SC], F32, tag="acc", name="ops_ps")
                            for hh in range(G):
                                nc.tensor.matmul(
                                    ps[:], lhsT=onrm[:, hh, stl], rhs=wo_sb[:, hh, esl],
                                    start=(hh == 0), stop=(hh == G - 1),
                                )
                            ob = obp.tile([128, SC], BF16, tag="ob", name="ob")
                            nc.vector.tensor_copy(out=ob[:], in_=ps[:])
                            r0 = (st * NEC + ec) * 128
                            nc.sync.dma_start(out=out[r0 : r0 + 128, :], in_=ob[:])

                        yield chain

            def attn_block(qc, extras=None, start_iter=0):
                for h in range(G):
                    attn_head(qc, h, extras if h == 0 else None, start_iter)
                    if h == 0 and extras is not None:
                        for ex in extras:  # drain leftovers (shouldn't happen)
                            ex()
                    if h >= 1:
                        norm(qc, h - 1)

            # ---- emission schedule ----
            proj_block(0)
            for jj in range(4):
                nc.sync.dma_start(
                    out=mask_sb[:, jj, :], in_=msk[jj * 128 : (jj + 1) * 128, :]
                )
            for h4 in range(G):
                nc.sync.dma_start(
                    out=wo_sb[:, h4, :], in_=wo[h4 * 128 : (h4 + 1) * 128, :]
                )
            proj_block(1)
            attn_block(0)
            proj_block(2)
            norm(0, 3)
            attn_block(1, extras=op_gen(0), start_iter=0)
            proj_block(3)
            norm(1, 3)
            attn_block(2, extras=op_gen(1), start_iter=0)
            norm(2, 3)
            attn_block(3, extras=op_gen(2), start_iter=8)
            norm(3, 3)
            for ch in op_gen(3):
                ch()
    nc.compile()
    return nc


_NC_CACHE = None


def _get_nc():
    global _NC_CACHE
    if _NC_CACHE is None:
        _NC_CACHE = build_nc()
    return _NC_CACHE


def _block_x(xT_bf):
    """[E, S] bf16 -> [(sc,t) blocked] so each [128,SC] tile is contiguous."""
    return np.ascontiguousarray(
        xT_bf.reshape(NET, 128, NSC, SC).transpose(2, 0, 1, 3).reshape(-1, SC)
    )


def _prep_in_maps(query, key, value, attn_mask, Wq, Wk, Wv, Wo):
    query = np.asarray(query, dtype=np.float32)
    key = np.asarray(key, dtype=np.float32)
    value = np.asarray(value, dtype=np.float32)
    Wq = np.asarray(Wq, dtype=np.float32)
    Wk = np.asarray(Wk, dtype=np.float32)
    Wv = np.asarray(Wv, dtype=np.float32)
    Wo = np.asarray(Wo, dtype=np.float32)
    am = np.asarray(attn_mask)

    xqT = [_block_x(np.ascontiguousarray(query[b].T).astype(NPBF)) for b in range(B)]
    xkT = [_block_x(np.ascontiguousarray(key[b].T).astype(NPBF)) for b in range(B)]
    xvT = [_block_x(np.ascontiguousarray(value[b].T).astype(NPBF)) for b in range(B)]

    # 4 diagonal mask tiles [128, SC]: tile j covers k in [j*128,(j+1)*128)
    # relative to the q-chunk start; additive -1e9 on masked entries.
    m0 = np.asarray(am[0, 0, :SC, :SC], dtype=np.float32)  # [q, k] for chunk 0
    msk_tiles = np.zeros((4 * 128, SC), dtype=np.float32)
    for j in range(4):
        msk_tiles[j * 128 : (j + 1) * 128, :] = (
            m0[:, j * 128 : (j + 1) * 128].T - 1.0
        ) * 1e9
    in_maps = []
    for b in range(B):
        for g in range(HKV):
            in_maps.append(
                {
                    "xq": xqT[b],
                    "xk": xkT[b],
                    "xv": xvT[b],
                    "wq": np.ascontiguousarray(
                        Wq[g * GD : (g + 1) * GD, :].T
                    ).astype(NPBF),
                    "wk": np.ascontiguousarray(
                        Wk[g * D : (g + 1) * D, :].T
                    ).astype(NPBF),
                    "wv": np.ascontiguousarray(
                        Wv[g * D : (g + 1) * D, :].T
                    ).astype(NPBF),
                    "wo": np.ascontiguousarray(
                        Wo[:, g * GD : (g + 1) * GD].T
                    ).astype(NPBF),
                    "msk": msk_tiles,
                }
            )
    return in_maps


def _unblock_out(o):
    """[(st,ec) blocked, SC] bf16 -> [S, E] f32."""
    return (
        o.astype(np.float32)
        .reshape(NST, NEC, 128, SC)
        .transpose(0, 2, 1, 3)
        .reshape(S, E)
    )


def _run(inputs, trace=False, **kw):
    nc = _get_nc()
    in_maps = _prep_in_maps(**inputs)
    res = run_bass_kernel_spmd(nc, in_maps, list(range(NCORES)), trace=trace, **kw)
    outs = [np.asarray(r["out"]) for r in res.results]
    full = np.empty((B, S, E), dtype=np.float32)
    for b in range(B):
        acc = _unblock_out(outs[b * HKV])
        for g in range(1, HKV):
            acc = acc + _unblock_out(outs[b * HKV + g])
        full[b] = acc
    return full, res


def kernel(**inputs):
    full, _ = _run(inputs, trace=False)
    return full


# revision 4
# speedup vs baseline: 1.5082x; 1.2035x over previous
"""GQA kernel for Trainium2, 8 NeuronCores.

Sharding: core c = b*4 + g handles batch b, kv-head g (4 query heads).
Host sums the 4 partial out-projections per batch.

Design notes (v3):
- All matmuls bf16 (fp32 PSUM). PE cost is free-dim rows x clock, and the
  clock p-state ramps to 2.4GHz only after ~3us of *continuous* PE busy,
  so the emission order never lets the PE idle: projection, attention and
  out-projection chunks are interleaved, and the out-projection of chunk
  qc is folded into the first head's kt-loop of chunk qc+1.
- DMA descriptors are generated per SBUF partition line, so all DRAM
  layouts are partition-major: x is host-blocked [sc][p][t][f] (16KB
  contiguous per partition per chunk), weights [p][t][..], and the output
  is written per s-tile with 4KB lines. Chunk loads are split along t so
  8 queues stream one chunk in parallel and the first matmul of a chain
  only waits for its own t-range.
- Attention kt order: two non-diagonal tiles first (their exp has no
  DVE mask-add on the critical path, hiding the QK->exp->PV latency at
  each head-loop start), then the 4 diagonal tiles (column-trimmed: tile
  j only touches q columns >= 128j, the rest is fully masked), then the
  remaining tiles. The first flushed PV/l matmul covers all 512 columns
  so PSUM start=True initializes the full accumulator.
- Softmax normalization without a DRAM round trip: l row-sums accumulate
  via ones-matmuls per kt; then l -> bf16 copy (ACT), K=1 matmul
  broadcast across partitions (PE), reciprocal_approx_fast (DVE, ~0.7us
  vs 3.3us for reciprocal), multiply into onrm.
"""

import sys

import numpy as np

for _p in ("/opt/trn_rl_repo",):
    if _p not in sys.path:
        sys.path.insert(0, _p)

import ml_dtypes

import concourse.mybir as mybir
from concourse import bacc
from concourse.bass_utils import run_bass_kernel_spmd
from concourse.masks import make_identity
from concourse.tile import TileContext

B, S, E = 2, 2048, 2048
H, HKV = 16, 4
D = E // H  # 128
G = H // HKV  # 4 query heads per kv head
GD = G * D  # 512
NCORES = B * HKV  # 8
SC = 512  # s/q chunk width (free dim of matmuls)
NSC = S // SC  # 4
NET = E // 128  # 16 e-tiles (contraction)
NKT = S // 128  # 16 k-tiles
NEC = E // SC  # 4 e-chunks for output
SCALE = 1.0 / float(np.sqrt(D))

F32 = mybir.dt.float32
BF16 = mybir.dt.bfloat16
AF = mybir.ActivationFunctionType
NPBF = np.dtype(ml_dtypes.bfloat16)


def build_nc():
    nc = bacc.Bacc()
    # x inputs: [sc][p][t][f] partition-major blocks -> [NSC*128, NET*SC]
    xq = nc.declare_dram_parameter("xq", [NSC * 128, NET * SC], BF16, isOutput=False)
    xk = nc.declare_dram_parameter("xk", [NSC * 128, NET * SC], BF16, isOutput=False)
    xv = nc.declare_dram_parameter("xv", [NSC * 128, NET * SC], BF16, isOutput=False)
    # weights partition-major: [p][t][..]
    wq = nc.declare_dram_parameter("wq", [128, NET * GD], BF16, isOutput=False)
    wk = nc.declare_dram_parameter("wk", [128, NET * D], BF16, isOutput=False)
    wv = nc.declare_dram_parameter("wv", [128, NET * D], BF16, isOutput=False)
    wo = nc.declare_dram_parameter("wo", [128, G * E], BF16, isOutput=False)
    msk = nc.declare_dram_parameter("msk", [128, 4 * SC], F32, isOutput=False)
    # natural [S, E]: written per s-tile as [128, E] full-width rows
    out = nc.declare_dram_parameter("out", [S, E], BF16, isOutput=True)

    with TileContext(nc) as tc:
        with (
            tc.tile_pool(name="singles", bufs=1) as singles,
            tc.tile_pool(name="xc", bufs=1) as xcp,
            tc.tile_pool(name="pexp", bufs=4) as pexp,
            tc.tile_pool(name="small", bufs=2) as small,
            tc.tile_pool(name="ob", bufs=2) as obp,
            tc.tile_pool(name="acc", bufs=3, space="PSUM") as acc,
            tc.tile_pool(name="ops", bufs=2, space="PSUM") as ops,
            tc.tile_pool(name="lps", bufs=2, space="PSUM") as lps,
            tc.tile_pool(name="misc", bufs=1, space="PSUM") as misc,
        ):
            # ---- constants / weights resident in SBUF ----
            wq_sb = singles.tile([128, NET, GD], BF16)  # 16KB/p
            wk_sb = singles.tile([128, NET, D], BF16)  # 4KB/p
            wv_sb = singles.tile([128, NET, D], BF16)  # 4KB/p
            wo_sb = singles.tile([128, G, E], BF16)  # 16KB/p
            mask_sb = singles.tile([128, 4, SC], F32)  # 8KB/p
            ident_f = singles.tile([128, 128], F32)
            ident = singles.tile([128, 128], BF16)
            onesc_f = singles.tile([128, 1], F32)
            onesc = singles.tile([128, 1], BF16)  # l-matmul lhsT
            onesr_f = singles.tile([1, 128], F32)
            onesr = singles.tile([1, 128], BF16)  # bcast-matmul lhsT
            qT = singles.tile([128, G, S], BF16)  # 16KB/p
            kT = singles.tile([128, S], BF16)  # 4KB/p
            v_sb = singles.tile([128, NKT, D], BF16)  # 4KB/p
            onrm = singles.tile([128, G, S], BF16)  # 16KB/p

            make_identity(nc, ident_f)
            nc.scalar.activation(out=ident[:], in_=ident_f[:], func=AF.Copy)
            nc.vector.memset(onesc_f, 1.0)
            nc.scalar.activation(out=onesc[:], in_=onesc_f[:], func=AF.Copy)
            nc.vector.memset(onesr_f, 1.0)
            nc.scalar.activation(out=onesr[:], in_=onesr_f[:], func=AF.Copy)

            def wsplit(dst, dram, width, nsplit):
                """Load [128, n, width] SBUF tile from partition-major DRAM."""
                ntiles = dst.shape[1]
                step = ntiles // nsplit
                for i in range(nsplit):
                    t0 = i * step
                    nc.sync.dma_start(
                        out=dst[:, t0 : t0 + step, :],
                        in_=dram[:, t0 * width : (t0 + step) * width],
                    )

            # K/V weights first: K-proj of chunk 0 can start early
            wsplit(wk_sb, wk, D, 4)
            wsplit(wv_sb, wv, D, 4)

            state = {}  # (qc, h) -> (o_ps, l_ps)

            def x_chunk(dram, sc, tag):
                """Stream one [128, NET, SC] x chunk, split along t over 8 queues."""
                xsb = xcp.tile([128, NET, SC], BF16, tag=tag, name=tag)
                rows = slice(sc * 128, (sc + 1) * 128)
                step = NET // 8
                for i in range(8):
                    t0 = i * step
                    nc.sync.dma_start(
                        out=xsb[:, t0 : t0 + step, :],
                        in_=dram[rows, t0 * SC : (t0 + step) * SC],
                    )
                return xsb

            def proj_block(sc):
                ssl = slice(sc * SC, (sc + 1) * SC)
                # K projection
                xsb = x_chunk(xk, sc, "xk")
                ps = acc.tile([128, SC], F32, tag="acc", name="kps")
                for t in range(NET):
                    nc.tensor.matmul(
                        ps[:], lhsT=wk_sb[:, t, :], rhs=xsb[:, t, :],
                        start=(t == 0), stop=(t == NET - 1),
                    )
                nc.vector.tensor_copy(out=kT[:, ssl], in_=ps[:])
                # V projection, then transpose to [s, d] tiles
                xsb = x_chunk(xv, sc, "xv")
                ps = acc.tile([128, SC], F32, tag="acc", name="vps")
                for t in range(NET):
                    nc.tensor.matmul(
                        ps[:], lhsT=wv_sb[:, t, :], rhs=xsb[:, t, :],
                        start=(t == 0), stop=(t == NET - 1),
                    )
                vt = small.tile([128, SC], BF16, tag="vt", name="vt")
                nc.scalar.activation(out=vt[:], in_=ps[:], func=AF.Copy)
                for i in range(SC // 128):
                    tp = misc.tile([128, 128], BF16, tag="mx", name="tp")
                    nc.tensor.transpose(tp[:], vt[:, i * 128 : (i + 1) * 128], ident[:])
                    nc.vector.tensor_copy(out=v_sb[:, sc * 4 + i, :], in_=tp[:])
                # Q projection (4 heads); wq streamed during K/V of chunk 0
                if sc == 0:
                    wsplit(wq_sb, wq, GD, 8)
                xsb = x_chunk(xq, sc, "xq")
                for h in range(G):
                    ps = acc.tile([128, SC], F32, tag="acc", name="qps")
                    for t in range(NET):
                        nc.tensor.matmul(
                            ps[:], lhsT=wq_sb[:, t, h * D : (h + 1) * D],
                            rhs=xsb[:, t, :], start=(t == 0), stop=(t == NET - 1),
                        )
                    # fold softmax scale into Q
                    nc.scalar.activation(
                        out=qT[:, h, ssl], in_=ps[:], func=AF.Copy, scale=SCALE
                    )

            def attn_head(qc, h, extras=None, start_iter=0):
                nkt = 4 * (qc + 1)
                # two non-diagonal tiles first (no mask-add latency in front
                # of the first exp), then the column-trimmed diagonal tiles,
                # then the rest. qc=0 has only diagonal tiles.
                nd = list(range(nkt - 4))
                order = [(kt, None) for kt in nd[:2]]
                order += [(nkt - 4 + j, j) for j in range(4)]
                order += [(kt, None) for kt in nd[2:]]
                n = len(order)
                o_ps = ops.tile([128, SC], F32, tag="o", name="o_ps")
                l_ps = lps.tile([1, SC], F32, tag="l", name="l_ps")
                state[(qc, h)] = (o_ps, l_ps)
                pend = []
                nfl = [0]

                def flush_one():
                    kt, cl, p = pend.pop(0)
                    first = nfl[0] == 0
                    last = nfl[0] == n - 1
                    nc.tensor.matmul(
                        o_ps[:, cl], lhsT=v_sb[:, kt, :], rhs=p[:, cl],
                        start=first, stop=last, skip_group_check=True,
                    )
                    nc.tensor.matmul(
                        l_ps[:, cl], lhsT=onesc[:, :], rhs=p[:, cl],
                        start=first, stop=last, skip_group_check=True,
                    )
                    nfl[0] += 1

                for i, (kt, j) in enumerate(order):
                    cl = slice(128 * j, SC) if j is not None else slice(0, SC)
                    qsl = slice(qc * SC + cl.start, (qc + 1) * SC)
                    s_ps = acc.tile([128, SC], F32, tag="acc", name="s_ps")
                    nc.tensor.matmul(
                        s_ps[:, cl], lhsT=kT[:, kt * 128 : (kt + 1) * 128],
                        rhs=qT[:, h, qsl], start=True, stop=True,
                    )
                    if j is not None:
                        nc.vector.tensor_add(s_ps[:, cl], s_ps[:, cl], mask_sb[:, j, cl])
                    p = pexp.tile([128, SC], BF16, tag="p", name="p")
                    nc.scalar.activation(out=p[:, cl], in_=s_ps[:, cl], func=AF.Exp)
                    pend.append((kt, cl, p))
                    if extras is not None and i >= start_iter:
                        for _ in range(2):
                            ex = next(extras, None)
                            if ex is not None:
                                ex()
                    if i >= 2:
                        flush_one()
                while pend:
                    flush_one()

            def norm(qc, h):
                o_ps, l_ps = state.pop((qc, h))
                qsl = slice(qc * SC, (qc + 1) * SC)
                l_bf = small.tile([1, SC], BF16, tag="lbf", name="l_bf")
                nc.scalar.activation(out=l_bf[:], in_=l_ps[:], func=AF.Copy)
                bc = misc.tile([128, SC], F32, tag="mx", name="bc")
                nc.tensor.matmul(bc[:], lhsT=onesr[:, :], rhs=l_bf[:], start=True, stop=True)
                o_unn = small.tile([128, SC], F32, tag="ou", name="o_unn")
                nc.scalar.activation(out=o_unn[:], in_=o_ps[:], func=AF.Copy)
                rinv = small.tile([128, SC], F32, tag="ri", name="rinv")
                nc.vector.reciprocal_approx_fast(out=rinv[:], in_=bc[:])
                nc.vector.tensor_mul(onrm[:, h, qsl], o_unn[:], rinv[:])

            def op_gen(qc):
                # out-projection of chunk qc: 16 chains of 4 matmuls; the 4
                # e-chunks of one s-tile stage into one SBUF tile so the out
                # DMA writes full 4KB partition lines.
                for sti in range(4):
                    st = qc * 4 + sti
                    stl = slice(st * 128, (st + 1) * 128)
                    holder = {}
                    for ec in range(NEC):
                        esl = slice(ec * SC, (ec + 1) * SC)

                        def chain(stl=stl, esl=esl, st=st, ec=ec, holder=holder):
                            if ec == 0:
                                holder["ob"] = obp.tile(
                                    [128, NEC, SC], BF16, tag="ob", name="ob"
                                )
                            ps = acc.tile([128, SC], F32, tag="acc", name="ops_ps")
                            for hh in range(G):
                                nc.tensor.matmul(
                                    ps[:], lhsT=onrm[:, hh, stl], rhs=wo_sb[:, hh, esl],
                                    start=(hh == 0), stop=(hh == G - 1),
                                )
                            nc.vector.tensor_copy(out=holder["ob"][:, ec, :], in_=ps[:])
                            if ec == NEC - 1:
                                nc.sync.dma_start(
                                    out=out[st * 128 : (st + 1) * 128, :],
                                    in_=holder["ob"][:],
                                )

                        yield chain

            def attn_block(qc, extras=None, start_iter=0):
                for h in range(G):
                    attn_head(qc, h, extras if h == 0 else None, start_iter)
                    if h == 0 and extras is not None:
                        for ex in extras:  # drain leftovers (shouldn't happen)
                            ex()
                    if h >= 1:
                        norm(qc, h - 1)

            # ---- emission schedule ----
            proj_block(0)
            wsplit(mask_sb, msk, SC, 4)
            wsplit(wo_sb, wo, E, 4)
            proj_block(1)
            attn_block(0)
            proj_block(2)
            norm(0, 3)
            attn_block(1, extras=op_gen(0), start_iter=0)
            proj_block(3)
            norm(1, 3)
            attn_block(2, extras=op_gen(1), start_iter=0)
            norm(2, 3)
            attn_block(3, extras=op_gen(2), start_iter=8)
            norm(3, 3)
            for ch in op_gen(3):
                ch()
    nc.compile()
    return nc


_NC_CACHE = None


def _get_nc():
    global _NC_CACHE
    if _NC_CACHE is None:
        _NC_CACHE = build_nc()
    return _NC_CACHE


def _block_x(xT_bf):
    """[E, S] bf16 -> [sc][p][t][f] partition-major blocks [NSC*128, NET*SC]."""
    return np.ascontiguousarray(
        xT_bf.reshape(NET, 128, NSC, SC).transpose(2, 1, 0, 3).reshape(NSC * 128, NET * SC)
    )


def _block_w(w, width):
    """[ntiles*128, width] -> partition-major [128, ntiles*width]."""
    nt = w.shape[0] // 128
    return np.ascontiguousarray(
        w.reshape(nt, 128, width).transpose(1, 0, 2).reshape(128, nt * width)
    )


def _prep_in_maps(query, key, value, attn_mask, Wq, Wk, Wv, Wo):
    query = np.asarray(query, dtype=np.float32)
    key = np.asarray(key, dtype=np.float32)
    value = np.asarray(value, dtype=np.float32)
    Wq = np.asarray(Wq, dtype=np.float32)
    Wk = np.asarray(Wk, dtype=np.float32)
    Wv = np.asarray(Wv, dtype=np.float32)
    Wo = np.asarray(Wo, dtype=np.float32)
    am = np.asarray(attn_mask)

    xqT = [_block_x(np.ascontiguousarray(query[b].T).astype(NPBF)) for b in range(B)]
    xkT = [_block_x(np.ascontiguousarray(key[b].T).astype(NPBF)) for b in range(B)]
    xvT = [_block_x(np.ascontiguousarray(value[b].T).astype(NPBF)) for b in range(B)]

    # 4 diagonal mask tiles [128, SC]: tile j covers k in [j*128,(j+1)*128)
    # relative to the q-chunk start; additive -1e9 on masked entries.
    m0 = np.asarray(am[0, 0, :SC, :SC], dtype=np.float32)  # [q, k] for chunk 0
    msk_tiles = np.zeros((4 * 128, SC), dtype=np.float32)
    for j in range(4):
        msk_tiles[j * 128 : (j + 1) * 128, :] = (
            m0[:, j * 128 : (j + 1) * 128].T - 1.0
        ) * 1e9
    msk_pm = _block_w(msk_tiles, SC)

    in_maps = []
    for b in range(B):
        for g in range(HKV):
            wq_g = np.ascontiguousarray(Wq[g * GD : (g + 1) * GD, :].T).astype(NPBF)
            wk_g = np.ascontiguousarray(Wk[g * D : (g + 1) * D, :].T).astype(NPBF)
            wv_g = np.ascontiguousarray(Wv[g * D : (g + 1) * D, :].T).astype(NPBF)
            wo_g = np.ascontiguousarray(Wo[:, g * GD : (g + 1) * GD].T).astype(NPBF)
            in_maps.append(
                {
                    "xq": xqT[b],
                    "xk": xkT[b],
                    "xv": xvT[b],
                    "wq": _block_w(wq_g, GD),
                    "wk": _block_w(wk_g, D),
                    "wv": _block_w(wv_g, D),
                    "wo": _block_w(wo_g, E),
                    "msk": msk_pm,
                }
            )
    return in_maps


def _run(inputs, trace=False, **kw):
    nc = _get_nc()
    in_maps = _prep_in_maps(**inputs)
    res = run_bass_kernel_spmd(nc, in_maps, list(range(NCORES)), trace=trace, **kw)
    outs = [np.asarray(r["out"]) for r in res.results]
    full = np.empty((B, S, E), dtype=np.float32)
    for b in range(B):
        acc = outs[b * HKV].astype(np.float32)
        for g in range(1, HKV):
            acc = acc + outs[b * HKV + g].astype(np.float32)
        full[b] = acc
    return full, res


def kernel(**inputs):
    full, _ = _run(inputs, trace=False)
    return full


# revision 11
# speedup vs baseline: 1.5966x; 1.0586x over previous
"""GQA kernel for Trainium2, 8 NeuronCores.

Sharding: core c = b*4 + g handles batch b, kv-head g (4 query heads).
Host sums the 4 partial out-projections per batch.

Design notes (v3):
- All matmuls bf16 (fp32 PSUM). PE cost is free-dim rows x clock, and the
  clock p-state ramps to 2.4GHz only after ~3us of *continuous* PE busy,
  so the emission order never lets the PE idle: projection, attention and
  out-projection chunks are interleaved, and the out-projection of chunk
  qc is folded into the first head's kt-loop of chunk qc+1.
- DMA descriptors are generated per SBUF partition line, so all DRAM
  layouts are partition-major: x is host-blocked [sc][p][t][f] (16KB
  contiguous per partition per chunk), weights [p][t][..], and the output
  is written per s-tile with 4KB lines. Chunk loads are split along t so
  8 queues stream one chunk in parallel and the first matmul of a chain
  only waits for its own t-range.
- Attention kt order: two non-diagonal tiles first (their exp has no
  DVE mask-add on the critical path, hiding the QK->exp->PV latency at
  each head-loop start), then the 4 diagonal tiles (column-trimmed: tile
  j only touches q columns >= 128j, the rest is fully masked), then the
  remaining tiles. The first flushed PV/l matmul covers all 512 columns
  so PSUM start=True initializes the full accumulator.
- Softmax normalization without a DRAM round trip: l row-sums accumulate
  via ones-matmuls per kt; then l -> bf16 copy (ACT), K=1 matmul
  broadcast across partitions (PE), reciprocal_approx_fast (DVE, ~0.7us
  vs 3.3us for reciprocal), multiply into onrm.
"""

import sys

import numpy as np

for _p in ("/opt/trn_rl_repo",):
    if _p not in sys.path:
        sys.path.insert(0, _p)

import ml_dtypes

import concourse.mybir as mybir
from concourse import bacc
from concourse.bass_utils import run_bass_kernel_spmd
from concourse.masks import make_identity
from concourse.tile import TileContext

B, S, E = 2, 2048, 2048
H, HKV = 16, 4
D = E // H  # 128
G = H // HKV  # 4 query heads per kv head
GD = G * D  # 512
NCORES = B * HKV  # 8
SC = 512  # s/q chunk width (free dim of matmuls)
NSC = S // SC  # 4
NET = E // 128  # 16 e-tiles (contraction)
NKT = S // 128  # 16 k-tiles
NEC = E // SC  # 4 e-chunks for output
SCALE = 1.0 / float(np.sqrt(D))

F32 = mybir.dt.float32
BF16 = mybir.dt.bfloat16
AF = mybir.ActivationFunctionType
NPBF = np.dtype(ml_dtypes.bfloat16)


def build_nc():
    nc = bacc.Bacc()
    # x inputs: [sc][p][t][f] partition-major blocks -> [NSC*128, NET*SC]
    xq = nc.declare_dram_parameter("xq", [NSC * 128, NET * SC], BF16, isOutput=False)
    xk = nc.declare_dram_parameter("xk", [NSC * 128, NET * SC], BF16, isOutput=False)
    xv = nc.declare_dram_parameter("xv", [NSC * 128, NET * SC], BF16, isOutput=False)
    # weights partition-major: [p][t][..]
    wq = nc.declare_dram_parameter("wq", [128, NET * GD], BF16, isOutput=False)
    wk = nc.declare_dram_parameter("wk", [128, NET * D], BF16, isOutput=False)
    wv = nc.declare_dram_parameter("wv", [128, NET * D], BF16, isOutput=False)
    wo = nc.declare_dram_parameter("wo", [128, G * E], BF16, isOutput=False)
    msk = nc.declare_dram_parameter("msk", [128, 4 * SC], F32, isOutput=False)
    # natural [S, E]: written per s-tile as [128, E] full-width rows
    out = nc.declare_dram_parameter("out", [S, E], BF16, isOutput=True)

    with TileContext(nc) as tc:
        with (
            tc.tile_pool(name="singles", bufs=1) as singles,
            tc.tile_pool(name="xc", bufs=1) as xcp,
            tc.tile_pool(name="pexp", bufs=6) as pexp,
            tc.tile_pool(name="small", bufs=2) as small,
            tc.tile_pool(name="ob", bufs=2) as obp,
            tc.tile_pool(name="acc", bufs=3, space="PSUM") as acc,
            tc.tile_pool(name="ops", bufs=2, space="PSUM") as ops,
            tc.tile_pool(name="lps", bufs=2, space="PSUM") as lps,
            tc.tile_pool(name="misc", bufs=1, space="PSUM") as misc,
        ):
            # ---- constants / weights resident in SBUF ----
            wq_sb = singles.tile([128, NET, GD], BF16)  # 16KB/p
            wk_sb = singles.tile([128, NET, D], BF16)  # 4KB/p
            wv_sb = singles.tile([128, NET, D], BF16)  # 4KB/p
            wo_sb = singles.tile([128, G, E], BF16)  # 16KB/p
            mask_sb = singles.tile([128, 4, SC], F32)  # 8KB/p
            ident_f = singles.tile([128, 128], F32)
            ident = singles.tile([128, 128], BF16)
            ones_f = singles.tile([128, 128], F32)
            # l-matmul lhsT: [128,128] ones -> row-sum REPLICATED on all 128
            # output partitions (same row cost as a 1-wide lhsT, but the
            # weight load pipelines and no separate broadcast is needed)
            ones_m = singles.tile([128, 128], BF16)
            qT = singles.tile([128, G, S], BF16)  # 16KB/p
            kT = singles.tile([128, S], BF16)  # 4KB/p
            v_sb = singles.tile([128, NKT, D], BF16)  # 4KB/p
            onrm = singles.tile([128, G, S], BF16)  # 16KB/p

            make_identity(nc, ident_f)
            nc.scalar.activation(out=ident[:], in_=ident_f[:], func=AF.Copy)
            nc.vector.memset(ones_f, 1.0)
            nc.scalar.activation(out=ones_m[:], in_=ones_f[:], func=AF.Copy)

            def wsplit(dst, dram, width, nsplit):
                """Load [128, n, width] SBUF tile from partition-major DRAM."""
                ntiles = dst.shape[1]
                step = ntiles // nsplit
                for i in range(nsplit):
                    t0 = i * step
                    nc.sync.dma_start(
                        out=dst[:, t0 : t0 + step, :],
                        in_=dram[:, t0 * width : (t0 + step) * width],
                    )

            # K/V weights first: K-proj of chunk 0 can start early
            wsplit(wk_sb, wk, D, 4)
            wsplit(wv_sb, wv, D, 4)

            state = {}  # (qc, h) -> (o_ps, l_ps)

            def x_chunk(dram, sc, tag):
                """Stream one [128, NET, SC] x chunk, split along t."""
                xsb = xcp.tile([128, NET, SC], BF16, tag=tag, name=tag)
                rows = slice(sc * 128, (sc + 1) * 128)
                nsplit = 16 if sc == 0 else 8  # finer first chunk: earlier start
                step = NET // nsplit
                for i in range(nsplit):
                    t0 = i * step
                    nc.sync.dma_start(
                        out=xsb[:, t0 : t0 + step, :],
                        in_=dram[rows, t0 * SC : (t0 + step) * SC],
                    )
                return xsb

            def proj_block(sc):
                ssl = slice(sc * SC, (sc + 1) * SC)
                # K projection
                xsb = x_chunk(xk, sc, "xk")
                ps = acc.tile([128, SC], F32, tag="acc", name="kps")
                for t in range(NET):
                    nc.tensor.matmul(
                        ps[:], lhsT=wk_sb[:, t, :], rhs=xsb[:, t, :],
                        start=(t == 0), stop=(t == NET - 1),
                    )
                nc.vector.tensor_copy(out=kT[:, ssl], in_=ps[:])
                # V projection, then transpose to [s, d] tiles
                xsb = x_chunk(xv, sc, "xv")
                ps = acc.tile([128, SC], F32, tag="acc", name="vps")
                for t in range(NET):
                    nc.tensor.matmul(
                        ps[:], lhsT=wv_sb[:, t, :], rhs=xsb[:, t, :],
                        start=(t == 0), stop=(t == NET - 1),
                    )
                vt = small.tile([128, SC], BF16, tag="vt", name="vt")
                nc.scalar.activation(out=vt[:], in_=ps[:], func=AF.Copy)
                for i in range(SC // 128):
                    tp = misc.tile([128, 128], BF16, tag="mx", name="tp")
                    nc.tensor.transpose(tp[:], vt[:, i * 128 : (i + 1) * 128], ident[:])
                    nc.vector.tensor_copy(out=v_sb[:, sc * 4 + i, :], in_=tp[:])
                # Q projection (4 heads); wq streamed during K/V of chunk 0
                if sc == 0:
                    wsplit(wq_sb, wq, GD, 8)
                xsb = x_chunk(xq, sc, "xq")
                for h in range(G):
                    ps = acc.tile([128, SC], F32, tag="acc", name="qps")
                    for t in range(NET):
                        nc.tensor.matmul(
                            ps[:], lhsT=wq_sb[:, t, h * D : (h + 1) * D],
                            rhs=xsb[:, t, :], start=(t == 0), stop=(t == NET - 1),
                        )
                    # fold softmax scale into Q
                    nc.scalar.activation(
                        out=qT[:, h, ssl], in_=ps[:], func=AF.Copy, scale=SCALE
                    )

            def attn_head(qc, h, extras=None, start_iter=0):
                nkt = 4 * (qc + 1)
                # two non-diagonal tiles first (no mask-add latency in front
                # of the first exp), then the column-trimmed diagonal tiles,
                # then the rest. qc=0 has only diagonal tiles.
                nd = list(range(nkt - 4))
                order = [(kt, None) for kt in nd[:2]]
                order += [(nkt - 4 + j, j) for j in range(4)]
                order += [(kt, None) for kt in nd[2:]]
                n = len(order)
                o_ps = ops.tile([128, SC], F32, tag="o", name="o_ps")
                l_ps = lps.tile([128, SC], F32, tag="l", name="l_ps")
                state[(qc, h)] = (o_ps, l_ps)
                pend = []
                nfl = [0]

                def flush_one():
                    kt, cl, p = pend.pop(0)
                    first = nfl[0] == 0
                    last = nfl[0] == n - 1
                    nc.tensor.matmul(
                        o_ps[:, cl], lhsT=v_sb[:, kt, :], rhs=p[:, cl],
                        start=first, stop=last, skip_group_check=True,
                    )
                    nc.tensor.matmul(
                        l_ps[:, cl], lhsT=ones_m[:, :], rhs=p[:, cl],
                        start=first, stop=last, skip_group_check=True,
                    )
                    nfl[0] += 1

                for i, (kt, j) in enumerate(order):
                    cl = slice(128 * j, SC) if j is not None else slice(0, SC)
                    qsl = slice(qc * SC + cl.start, (qc + 1) * SC)
                    s_ps = acc.tile([128, SC], F32, tag="acc", name="s_ps")
                    nc.tensor.matmul(
                        s_ps[:, cl], lhsT=kT[:, kt * 128 : (kt + 1) * 128],
                        rhs=qT[:, h, qsl], start=True, stop=True,
                    )
                    if j is not None:
                        nc.vector.tensor_add(s_ps[:, cl], s_ps[:, cl], mask_sb[:, j, cl])
                    p = pexp.tile([128, SC], BF16, tag="p", name="p")
                    nc.scalar.activation(out=p[:, cl], in_=s_ps[:, cl], func=AF.Exp)
                    pend.append((kt, cl, p))
                    if extras is not None and i >= start_iter:
                        for _ in range(2):
                            ex = next(extras, None)
                            if ex is not None:
                                ex()
                    if i >= 3:
                        flush_one()
                while pend:
                    flush_one()

            def norm(qc, h):
                # l_ps already holds l broadcast on all 128 partitions
                o_ps, l_ps = state.pop((qc, h))
                qsl = slice(qc * SC, (qc + 1) * SC)
                rinv = small.tile([128, SC], F32, tag="ri", name="rinv")
                nc.vector.reciprocal_approx_fast(out=rinv[:], in_=l_ps[:])
                nc.vector.tensor_mul(onrm[:, h, qsl], o_ps[:], rinv[:])

            def op_gen(qc):
                # out-projection of chunk qc: 16 chains of 4 matmuls; the 4
                # e-chunks of one s-tile stage into one SBUF tile so the out
                # DMA writes full 4KB partition lines.
                for sti in range(4):
                    st = qc * 4 + sti
                    stl = slice(st * 128, (st + 1) * 128)
                    holder = {}
                    for ec in range(NEC):
                        esl = slice(ec * SC, (ec + 1) * SC)

                        def chain(stl=stl, esl=esl, st=st, ec=ec, holder=holder):
                            if ec == 0:
                                holder["ob"] = obp.tile(
                                    [128, NEC, SC], BF16, tag="ob", name="ob"
                                )
                            ps = acc.tile([128, SC], F32, tag="acc", name="ops_ps")
                            for hh in range(G):
                                nc.tensor.matmul(
                                    ps[:], lhsT=onrm[:, hh, stl], rhs=wo_sb[:, hh, esl],
                                    start=(hh == 0), stop=(hh == G - 1),
                                )
                            nc.vector.tensor_copy(out=holder["ob"][:, ec, :], in_=ps[:])
                            if ec == NEC - 1:
                                nc.sync.dma_start(
                                    out=out[st * 128 : (st + 1) * 128, :],
                                    in_=holder["ob"][:],
                                )

                        yield chain

            def attn_block(qc, extras=None, start_iter=0):
                for h in range(G):
                    attn_head(qc, h, extras if h == 0 else None, start_iter)
                    if h == 0 and extras is not None:
                        for ex in extras:  # drain leftovers (shouldn't happen)
                            ex()
                    if h >= 1:
                        norm(qc, h - 1)

            # ---- emission schedule ----
            proj_block(0)
            wsplit(mask_sb, msk, SC, 4)
            wsplit(wo_sb, wo, E, 4)
            proj_block(1)
            attn_block(0)
            proj_block(2)
            norm(0, 3)
            attn_block(1, extras=op_gen(0), start_iter=0)
            proj_block(3)
            norm(1, 3)
            attn_block(2, extras=op_gen(1), start_iter=0)
            norm(2, 3)
            attn_block(3, extras=op_gen(2), start_iter=8)
            norm(3, 3)
            for ch in op_gen(3):
                ch()
    nc.compile()
    return nc


_NC_CACHE = None


def _get_nc():
    global _NC_CACHE
    if _NC_CACHE is None:
        _NC_CACHE = build_nc()
    return _NC_CACHE


def _block_x(xT_bf):
    """[E, S] bf16 -> [sc][p][t][f] partition-major blocks [NSC*128, NET*SC]."""
    return np.ascontiguousarray(
        xT_bf.reshape(NET, 128, NSC, SC).transpose(2, 1, 0, 3).reshape(NSC * 128, NET * SC)
    )


def _block_w(w, width):
    """[ntiles*128, width] -> partition-major [128, ntiles*width]."""
    nt = w.shape[0] // 128
    return np.ascontiguousarray(
        w.reshape(nt, 128, width).transpose(1, 0, 2).reshape(128, nt * width)
    )


def _prep_in_maps(query, key, value, attn_mask, Wq, Wk, Wv, Wo):
    query = np.asarray(query, dtype=np.float32)
    key = np.asarray(key, dtype=np.float32)
    value = np.asarray(value, dtype=np.float32)
    Wq = np.asarray(Wq, dtype=np.float32)
    Wk = np.asarray(Wk, dtype=np.float32)
    Wv = np.asarray(Wv, dtype=np.float32)
    Wo = np.asarray(Wo, dtype=np.float32)
    am = np.asarray(attn_mask)

    xqT = [_block_x(np.ascontiguousarray(query[b].T).astype(NPBF)) for b in range(B)]
    xkT = [_block_x(np.ascontiguousarray(key[b].T).astype(NPBF)) for b in range(B)]
    xvT = [_block_x(np.ascontiguousarray(value[b].T).astype(NPBF)) for b in range(B)]

    # 4 diagonal mask tiles [128, SC]: tile j covers k in [j*128,(j+1)*128)
    # relative to the q-chunk start; additive -1e9 on masked entries.
    m0 = np.asarray(am[0, 0, :SC, :SC], dtype=np.float32)  # [q, k] for chunk 0
    msk_tiles = np.zeros((4 * 128, SC), dtype=np.float32)
    for j in range(4):
        msk_tiles[j * 128 : (j + 1) * 128, :] = (
            m0[:, j * 128 : (j + 1) * 128].T - 1.0
        ) * 1e9
    msk_pm = _block_w(msk_tiles, SC)

    in_maps = []
    for b in range(B):
        for g in range(HKV):
            wq_g = np.ascontiguousarray(Wq[g * GD : (g + 1) * GD, :].T).astype(NPBF)
            wk_g = np.ascontiguousarray(Wk[g * D : (g + 1) * D, :].T).astype(NPBF)
            wv_g = np.ascontiguousarray(Wv[g * D : (g + 1) * D, :].T).astype(NPBF)
            wo_g = np.ascontiguousarray(Wo[:, g * GD : (g + 1) * GD].T).astype(NPBF)
            in_maps.append(
                {
                    "xq": xqT[b],
                    "xk": xkT[b],
                    "xv": xvT[b],
                    "wq": _block_w(wq_g, GD),
                    "wk": _block_w(wk_g, D),
                    "wv": _block_w(wv_g, D),
                    "wo": _block_w(wo_g, E),
                    "msk": msk_pm,
                }
            )
    return in_maps


def _run(inputs, trace=False, **kw):
    nc = _get_nc()
    in_maps = _prep_in_maps(**inputs)
    res = run_bass_kernel_spmd(nc, in_maps, list(range(NCORES)), trace=trace, **kw)
    outs = [np.asarray(r["out"]) for r in res.results]
    full = np.empty((B, S, E), dtype=np.float32)
    for b in range(B):
        acc = outs[b * HKV].astype(np.float32)
        for g in range(1, HKV):
            acc = acc + outs[b * HKV + g].astype(np.float32)
        full[b] = acc
    return full, res


def kernel(**inputs):
    full, _ = _run(inputs, trace=False)
    return full


# revision 15
# speedup vs baseline: 1.6283x; 1.0198x over previous
"""GQA kernel for Trainium2, 8 NeuronCores.

Sharding: core c = b*4 + g handles batch b, kv-head g (4 query heads).
Host sums the 4 partial out-projections per batch.

Design notes (v3):
- All matmuls bf16 (fp32 PSUM). PE cost is free-dim rows x clock, and the
  clock p-state ramps to 2.4GHz only after ~3us of *continuous* PE busy,
  so the emission order never lets the PE idle: projection, attention and
  out-projection chunks are interleaved, and the out-projection of chunk
  qc is folded into the first head's kt-loop of chunk qc+1.
- DMA descriptors are generated per SBUF partition line, so all DRAM
  layouts are partition-major: x is host-blocked [sc][p][t][f] (16KB
  contiguous per partition per chunk), weights [p][t][..], and the output
  is written per s-tile with 4KB lines. Chunk loads are split along t so
  8 queues stream one chunk in parallel and the first matmul of a chain
  only waits for its own t-range.
- Attention kt order: two non-diagonal tiles first (their exp has no
  DVE mask-add on the critical path, hiding the QK->exp->PV latency at
  each head-loop start), then the 4 diagonal tiles (column-trimmed: tile
  j only touches q columns >= 128j, the rest is fully masked), then the
  remaining tiles. The first flushed PV/l matmul covers all 512 columns
  so PSUM start=True initializes the full accumulator.
- Softmax normalization without a DRAM round trip: l row-sums accumulate
  via ones-matmuls per kt; then l -> bf16 copy (ACT), K=1 matmul
  broadcast across partitions (PE), reciprocal_approx_fast (DVE, ~0.7us
  vs 3.3us for reciprocal), multiply into onrm.
"""

import sys

import numpy as np

for _p in ("/opt/trn_rl_repo",):
    if _p not in sys.path:
        sys.path.insert(0, _p)

import ml_dtypes

import concourse.mybir as mybir
from concourse import bacc
from concourse.bass_utils import run_bass_kernel_spmd
from concourse.masks import make_identity
from concourse.tile import TileContext

B, S, E = 2, 2048, 2048
H, HKV = 16, 4
D = E // H  # 128
G = H // HKV  # 4 query heads per kv head
GD = G * D  # 512
NCORES = B * HKV  # 8
SC = 512  # s/q chunk width (free dim of matmuls)
NSC = S // SC  # 4
NET = E // 128  # 16 e-tiles (contraction)
NKT = S // 128  # 16 k-tiles
NEC = E // SC  # 4 e-chunks for output
SCALE = 1.0 / float(np.sqrt(D))

F32 = mybir.dt.float32
BF16 = mybir.dt.bfloat16
AF = mybir.ActivationFunctionType
NPBF = np.dtype(ml_dtypes.bfloat16)


def build_nc():
    nc = bacc.Bacc()
    # x inputs: [sc][p][t][f] partition-major blocks -> [NSC*128, NET*SC]
    xq = nc.declare_dram_parameter("xq", [NSC * 128, NET * SC], BF16, isOutput=False)
    xk = nc.declare_dram_parameter("xk", [NSC * 128, NET * SC], BF16, isOutput=False)
    xv = nc.declare_dram_parameter("xv", [NSC * 128, NET * SC], BF16, isOutput=False)
    # weights partition-major: [p][t][..]
    wq = nc.declare_dram_parameter("wq", [128, NET * GD], BF16, isOutput=False)
    wk = nc.declare_dram_parameter("wk", [128, NET * D], BF16, isOutput=False)
    wv = nc.declare_dram_parameter("wv", [128, NET * D], BF16, isOutput=False)
    wo = nc.declare_dram_parameter("wo", [128, G * E], BF16, isOutput=False)
    msk = nc.declare_dram_parameter("msk", [128, 4 * SC], F32, isOutput=False)
    # natural [S, E]: written per s-tile as [128, E] full-width rows
    out = nc.declare_dram_parameter("out", [S, E], BF16, isOutput=True)

    with TileContext(nc) as tc:
        with (
            tc.tile_pool(name="singles", bufs=1) as singles,
            tc.tile_pool(name="xc", bufs=1) as xcp,
            tc.tile_pool(name="pexp", bufs=6) as pexp,
            tc.tile_pool(name="small", bufs=2) as small,
            tc.tile_pool(name="ob", bufs=2) as obp,
            tc.tile_pool(name="acc", bufs=3, space="PSUM") as acc,
            tc.tile_pool(name="ops", bufs=2, space="PSUM") as ops,
            tc.tile_pool(name="lps", bufs=2, space="PSUM") as lps,
            tc.tile_pool(name="misc", bufs=1, space="PSUM") as misc,
        ):
            # ---- constants / weights resident in SBUF ----
            wq_sb = singles.tile([128, NET, GD], BF16)  # 16KB/p
            wk_sb = singles.tile([128, NET, D], BF16)  # 4KB/p
            wv_sb = singles.tile([128, NET, D], BF16)  # 4KB/p
            wo_sb = singles.tile([128, G, E], BF16)  # 16KB/p
            mask_sb = singles.tile([128, 4, SC], F32)  # 8KB/p
            ident_f = singles.tile([128, 128], F32)
            ident = singles.tile([128, 128], BF16)
            ones_f = singles.tile([128, 128], F32)
            # l-matmul lhsT: [128,128] ones -> row-sum REPLICATED on all 128
            # output partitions (same row cost as a 1-wide lhsT, but the
            # weight load pipelines and no separate broadcast is needed)
            ones_m = singles.tile([128, 128], BF16)
            qT = singles.tile([128, G, S], BF16)  # 16KB/p
            kT = singles.tile([128, S], BF16)  # 4KB/p
            v_sb = singles.tile([128, NKT, D], BF16)  # 4KB/p
            onrm = singles.tile([128, G, S], BF16)  # 16KB/p

            make_identity(nc, ident_f)
            nc.scalar.activation(out=ident[:], in_=ident_f[:], func=AF.Copy)
            nc.vector.memset(ones_f, 1.0)
            nc.scalar.activation(out=ones_m[:], in_=ones_f[:], func=AF.Copy)

            def wsplit(dst, dram, width, nsplit):
                """Load [128, n, width] SBUF tile from partition-major DRAM."""
                ntiles = dst.shape[1]
                step = ntiles // nsplit
                for i in range(nsplit):
                    t0 = i * step
                    nc.sync.dma_start(
                        out=dst[:, t0 : t0 + step, :],
                        in_=dram[:, t0 * width : (t0 + step) * width],
                    )

            # K/V weights first: K-proj of chunk 0 can start early
            wsplit(wk_sb, wk, D, 4)
            wsplit(wv_sb, wv, D, 4)

            state = {}  # (qc, h) -> (o_ps, l_ps)

            def x_chunk(dram, sc, tag):
                """Stream one [128, NET, SC] x chunk, split along t."""
                xsb = xcp.tile([128, NET, SC], BF16, tag=tag, name=tag)
                rows = slice(sc * 128, (sc + 1) * 128)
                nsplit = 16 if sc == 0 else 8  # finer first chunk: earlier start
                step = NET // nsplit
                for i in range(nsplit):
                    t0 = i * step
                    nc.sync.dma_start(
                        out=xsb[:, t0 : t0 + step, :],
                        in_=dram[rows, t0 * SC : (t0 + step) * SC],
                    )
                return xsb

            def prefetch_x(sc):
                return (
                    x_chunk(xk, sc, "xk"),
                    x_chunk(xv, sc, "xv"),
                    x_chunk(xq, sc, "xq"),
                )

            def proj_block(sc, pre=None):
                ssl = slice(sc * SC, (sc + 1) * SC)
                # K projection
                xsb = pre[0] if pre else x_chunk(xk, sc, "xk")
                ps = acc.tile([128, SC], F32, tag="acc", name="kps")
                for t in range(NET):
                    nc.tensor.matmul(
                        ps[:], lhsT=wk_sb[:, t, :], rhs=xsb[:, t, :],
                        start=(t == 0), stop=(t == NET - 1),
                    )
                nc.vector.tensor_copy(out=kT[:, ssl], in_=ps[:])
                # V projection, then transpose to [s, d] tiles
                xsb = pre[1] if pre else x_chunk(xv, sc, "xv")
                ps = acc.tile([128, SC], F32, tag="acc", name="vps")
                for t in range(NET):
                    nc.tensor.matmul(
                        ps[:], lhsT=wv_sb[:, t, :], rhs=xsb[:, t, :],
                        start=(t == 0), stop=(t == NET - 1),
                    )
                vt = small.tile([128, SC], BF16, tag="vt", name="vt")
                nc.scalar.activation(out=vt[:], in_=ps[:], func=AF.Copy)
                for i in range(SC // 128):
                    tp = misc.tile([128, 128], BF16, tag="mx", name="tp")
                    nc.tensor.transpose(tp[:], vt[:, i * 128 : (i + 1) * 128], ident[:])
                    nc.vector.tensor_copy(out=v_sb[:, sc * 4 + i, :], in_=tp[:])
                # Q projection (4 heads); wq streamed during K/V of chunk 0
                if sc == 0:
                    wsplit(wq_sb, wq, GD, 8)
                xsb = pre[2] if pre else x_chunk(xq, sc, "xq")
                for h in range(G):
                    ps = acc.tile([128, SC], F32, tag="acc", name="qps")
                    for t in range(NET):
                        nc.tensor.matmul(
                            ps[:], lhsT=wq_sb[:, t, h * D : (h + 1) * D],
                            rhs=xsb[:, t, :], start=(t == 0), stop=(t == NET - 1),
                        )
                    # fold softmax scale into Q
                    nc.scalar.activation(
                        out=qT[:, h, ssl], in_=ps[:], func=AF.Copy, scale=SCALE
                    )

            def attn_head(qc, h, extras=None, start_iter=0):
                nkt = 4 * (qc + 1)
                # two non-diagonal tiles first (no mask-add latency in front
                # of the first exp), then the column-trimmed diagonal tiles,
                # then the rest. qc=0 has only diagonal tiles.
                nd = list(range(nkt - 4))
                order = [(kt, None) for kt in nd[:2]]
                order += [(nkt - 4 + j, j) for j in range(4)]
                order += [(kt, None) for kt in nd[2:]]
                n = len(order)
                o_ps = ops.tile([128, SC], F32, tag="o", name="o_ps")
                l_ps = lps.tile([128, SC], F32, tag="l", name="l_ps")
                state[(qc, h)] = (o_ps, l_ps)
                pend = []
                nfl = [0]

                def flush_one():
                    kt, cl, p = pend.pop(0)
                    first = nfl[0] == 0
                    last = nfl[0] == n - 1
                    nc.tensor.matmul(
                        o_ps[:, cl], lhsT=v_sb[:, kt, :], rhs=p[:, cl],
                        start=first, stop=last, skip_group_check=True,
                    )
                    nc.tensor.matmul(
                        l_ps[:, cl], lhsT=ones_m[:, :], rhs=p[:, cl],
                        start=first, stop=last, skip_group_check=True,
                    )
                    nfl[0] += 1

                for i, (kt, j) in enumerate(order):
                    cl = slice(128 * j, SC) if j is not None else slice(0, SC)
                    qsl = slice(qc * SC + cl.start, (qc + 1) * SC)
                    s_ps = acc.tile([128, SC], F32, tag="acc", name="s_ps")
                    nc.tensor.matmul(
                        s_ps[:, cl], lhsT=kT[:, kt * 128 : (kt + 1) * 128],
                        rhs=qT[:, h, qsl], start=True, stop=True,
                    )
                    if j is not None:
                        nc.vector.tensor_add(s_ps[:, cl], s_ps[:, cl], mask_sb[:, j, cl])
                    p = pexp.tile([128, SC], BF16, tag="p", name="p")
                    nc.scalar.activation(out=p[:, cl], in_=s_ps[:, cl], func=AF.Exp)
                    pend.append((kt, cl, p))
                    if extras is not None and i >= start_iter:
                        for _ in range(2):
                            ex = next(extras, None)
                            if ex is not None:
                                ex()
                    if i >= 3:
                        flush_one()
                while pend:
                    flush_one()

            def norm(qc, h):
                # l_ps already holds l broadcast on all 128 partitions
                o_ps, l_ps = state.pop((qc, h))
                qsl = slice(qc * SC, (qc + 1) * SC)
                rinv = small.tile([128, SC], F32, tag="ri", name="rinv")
                nc.vector.reciprocal_approx_fast(out=rinv[:], in_=l_ps[:])
                nc.vector.tensor_mul(onrm[:, h, qsl], o_ps[:], rinv[:])

            def op_gen(qc):
                # out-projection of chunk qc: 16 chains of 4 matmuls; the 4
                # e-chunks of one s-tile stage into one SBUF tile so the out
                # DMA writes full 4KB partition lines.
                for sti in range(4):
                    st = qc * 4 + sti
                    stl = slice(st * 128, (st + 1) * 128)
                    holder = {}
                    for ec in range(NEC):
                        esl = slice(ec * SC, (ec + 1) * SC)

                        def chain(stl=stl, esl=esl, st=st, ec=ec, holder=holder):
                            if ec == 0:
                                holder["ob"] = obp.tile(
                                    [128, NEC, SC], BF16, tag="ob", name="ob"
                                )
                            ps = acc.tile([128, SC], F32, tag="acc", name="ops_ps")
                            for hh in range(G):
                                nc.tensor.matmul(
                                    ps[:], lhsT=onrm[:, hh, stl], rhs=wo_sb[:, hh, esl],
                                    start=(hh == 0), stop=(hh == G - 1),
                                )
                            nc.vector.tensor_copy(out=holder["ob"][:, ec, :], in_=ps[:])
                            if ec == NEC - 1:
                                nc.sync.dma_start(
                                    out=out[st * 128 : (st + 1) * 128, :],
                                    in_=holder["ob"][:],
                                )

                        yield chain

            def attn_block(qc, extras=None, start_iter=0):
                for h in range(G):
                    attn_head(qc, h, extras if h == 0 else None, start_iter)
                    if h == 0 and extras is not None:
                        for ex in extras:  # drain leftovers (shouldn't happen)
                            ex()
                    if h >= 1:
                        norm(qc, h - 1)

            # ---- emission schedule ----
            proj_block(0)
            proj_block(1)
            # mask/wo queue behind chunk-1 x so they don't delay it; they are
            # only needed from A0 / A1 onwards.
            wsplit(mask_sb, msk, SC, 4)
            wsplit(wo_sb, wo, E, 4)
            attn_block(0)
            proj_block(2)
            norm(0, 3)
            # chunk-3 x queues ahead of A1's out-DMAs
            pre3 = prefetch_x(3)
            attn_block(1, extras=op_gen(0), start_iter=0)
            proj_block(3, pre=pre3)
            norm(1, 3)
            attn_block(2, extras=op_gen(1), start_iter=0)
            norm(2, 3)
            attn_block(3, extras=op_gen(2), start_iter=8)
            norm(3, 3)
            for ch in op_gen(3):
                ch()
    nc.compile()
    return nc


_NC_CACHE = None


def _get_nc():
    global _NC_CACHE
    if _NC_CACHE is None:
        _NC_CACHE = build_nc()
    return _NC_CACHE


def _block_x(xT_bf):
    """[E, S] bf16 -> [sc][p][t][f] partition-major blocks [NSC*128, NET*SC]."""
    return np.ascontiguousarray(
        xT_bf.reshape(NET, 128, NSC, SC).transpose(2, 1, 0, 3).reshape(NSC * 128, NET * SC)
    )


def _block_w(w, width):
    """[ntiles*128, width] -> partition-major [128, ntiles*width]."""
    nt = w.shape[0] // 128
    return np.ascontiguousarray(
        w.reshape(nt, 128, width).transpose(1, 0, 2).reshape(128, nt * width)
    )


def _prep_in_maps(query, key, value, attn_mask, Wq, Wk, Wv, Wo):
    query = np.asarray(query, dtype=np.float32)
    key = np.asarray(key, dtype=np.float32)
    value = np.asarray(value, dtype=np.float32)
    Wq = np.asarray(Wq, dtype=np.float32)
    Wk = np.asarray(Wk, dtype=np.float32)
    Wv = np.asarray(Wv, dtype=np.float32)
    Wo = np.asarray(Wo, dtype=np.float32)
    am = np.asarray(attn_mask)

    xqT = [_block_x(np.ascontiguousarray(query[b].T).astype(NPBF)) for b in range(B)]
    xkT = [_block_x(np.ascontiguousarray(key[b].T).astype(NPBF)) for b in range(B)]
    xvT = [_block_x(np.ascontiguousarray(value[b].T).astype(NPBF)) for b in range(B)]

    # 4 diagonal mask tiles [128, SC]: tile j covers k in [j*128,(j+1)*128)
    # relative to the q-chunk start; additive -1e9 on masked entries.
    m0 = np.asarray(am[0, 0, :SC, :SC], dtype=np.float32)  # [q, k] for chunk 0
    msk_tiles = np.zeros((4 * 128, SC), dtype=np.float32)
    for j in range(4):
        msk_tiles[j * 128 : (j + 1) * 128, :] = (
            m0[:, j * 128 : (j + 1) * 128].T - 1.0
        ) * 1e9
    msk_pm = _block_w(msk_tiles, SC)

    in_maps = []
    for b in range(B):
        for g in range(HKV):
            wq_g = np.ascontiguousarray(Wq[g * GD : (g + 1) * GD, :].T).astype(NPBF)
            wk_g = np.ascontiguousarray(Wk[g * D : (g + 1) * D, :].T).astype(NPBF)
            wv_g = np.ascontiguousarray(Wv[g * D : (g + 1) * D, :].T).astype(NPBF)
            wo_g = np.ascontiguousarray(Wo[:, g * GD : (g + 1) * GD].T).astype(NPBF)
            in_maps.append(
                {
                    "xq": xqT[b],
                    "xk": xkT[b],
                    "xv": xvT[b],
                    "wq": _block_w(wq_g, GD),
                    "wk": _block_w(wk_g, D),
                    "wv": _block_w(wv_g, D),
                    "wo": _block_w(wo_g, E),
                    "msk": msk_pm,
                }
            )
    return in_maps


def _run(inputs, trace=False, **kw):
    nc = _get_nc()
    in_maps = _prep_in_maps(**inputs)
    res = run_bass_kernel_spmd(nc, in_maps, list(range(NCORES)), trace=trace, **kw)
    outs = [np.asarray(r["out"]) for r in res.results]
    full = np.empty((B, S, E), dtype=np.float32)
    for b in range(B):
        acc = outs[b * HKV].astype(np.float32)
        for g in range(1, HKV):
            acc = acc + outs[b * HKV + g].astype(np.float32)
        full[b] = acc
    return full, res


def kernel(**inputs):
    full, _ = _run(inputs, trace=False)
    return full


# revision 24
# speedup vs baseline: 1.6338x; 1.0034x over previous
"""GQA kernel for Trainium2, 8 NeuronCores.

Sharding: core c = b*4 + g handles batch b, kv-head g (4 query heads).
Host sums the 4 partial out-projections per batch.

Design notes (v3):
- All matmuls bf16 (fp32 PSUM). PE cost is free-dim rows x clock, and the
  clock p-state ramps to 2.4GHz only after ~3us of *continuous* PE busy,
  so the emission order never lets the PE idle: projection, attention and
  out-projection chunks are interleaved, and the out-projection of chunk
  qc is folded into the first head's kt-loop of chunk qc+1.
- DMA descriptors are generated per SBUF partition line, so all DRAM
  layouts are partition-major: x is host-blocked [sc][p][t][f] (16KB
  contiguous per partition per chunk), weights [p][t][..], and the output
  is written per s-tile with 4KB lines. Chunk loads are split along t so
  8 queues stream one chunk in parallel and the first matmul of a chain
  only waits for its own t-range.
- Attention kt order: two non-diagonal tiles first (their exp has no
  DVE mask-add on the critical path, hiding the QK->exp->PV latency at
  each head-loop start), then the 4 diagonal tiles (column-trimmed: tile
  j only touches q columns >= 128j, the rest is fully masked), then the
  remaining tiles. The first flushed PV/l matmul covers all 512 columns
  so PSUM start=True initializes the full accumulator.
- Softmax normalization without a DRAM round trip: l row-sums accumulate
  via ones-matmuls per kt; then l -> bf16 copy (ACT), K=1 matmul
  broadcast across partitions (PE), reciprocal_approx_fast (DVE, ~0.7us
  vs 3.3us for reciprocal), multiply into onrm.
"""

import sys

import numpy as np

for _p in ("/opt/trn_rl_repo",):
    if _p not in sys.path:
        sys.path.insert(0, _p)

import ml_dtypes

import concourse.mybir as mybir
from concourse import bacc
from concourse.bass_utils import run_bass_kernel_spmd
from concourse.masks import make_identity
from concourse.tile import TileContext

B, S, E = 2, 2048, 2048
H, HKV = 16, 4
D = E // H  # 128
G = H // HKV  # 4 query heads per kv head
GD = G * D  # 512
NCORES = B * HKV  # 8
SC = 512  # s/q chunk width (free dim of matmuls)
NSC = S // SC  # 4
NET = E // 128  # 16 e-tiles (contraction)
NKT = S // 128  # 16 k-tiles
NEC = E // SC  # 4 e-chunks for output
SCALE = 1.0 / float(np.sqrt(D))

F32 = mybir.dt.float32
BF16 = mybir.dt.bfloat16
AF = mybir.ActivationFunctionType
NPBF = np.dtype(ml_dtypes.bfloat16)


def build_nc():
    nc = bacc.Bacc()
    # x inputs: [sc][p][t][f] partition-major blocks -> [NSC*128, NET*SC]
    xq = nc.declare_dram_parameter("xq", [NSC * 128, NET * SC], BF16, isOutput=False)
    xk = nc.declare_dram_parameter("xk", [NSC * 128, NET * SC], BF16, isOutput=False)
    xv = nc.declare_dram_parameter("xv", [NSC * 128, NET * SC], BF16, isOutput=False)
    # weights partition-major: [p][t][..]
    wq = nc.declare_dram_parameter("wq", [128, NET * GD], BF16, isOutput=False)
    wk = nc.declare_dram_parameter("wk", [128, NET * D], BF16, isOutput=False)
    wv = nc.declare_dram_parameter("wv", [128, NET * D], BF16, isOutput=False)
    wo = nc.declare_dram_parameter("wo", [128, G * E], BF16, isOutput=False)
    # natural [S, E]: written per s-tile as [128, E] full-width rows
    out = nc.declare_dram_parameter("out", [S, E], BF16, isOutput=True)

    with TileContext(nc) as tc:
        with (
            tc.tile_pool(name="singles", bufs=1) as singles,
            tc.tile_pool(name="xc", bufs=1) as xcp,
            tc.tile_pool(name="pexp", bufs=6) as pexp,
            tc.tile_pool(name="small", bufs=2) as small,
            tc.tile_pool(name="ob", bufs=2) as obp,
            tc.tile_pool(name="acc", bufs=3, space="PSUM") as acc,
            tc.tile_pool(name="ops", bufs=2, space="PSUM") as ops,
            tc.tile_pool(name="lps", bufs=2, space="PSUM") as lps,
            tc.tile_pool(name="misc", bufs=1, space="PSUM") as misc,
        ):
            # ---- constants / weights resident in SBUF ----
            wq_sb = singles.tile([128, NET, GD], BF16)  # 16KB/p
            wk_sb = singles.tile([128, NET, D], BF16)  # 4KB/p
            wv_sb = singles.tile([128, NET, D], BF16)  # 4KB/p
            wo_sb = singles.tile([128, G, E], BF16)  # 16KB/p
            mask_sb = singles.tile([128, 4, SC], F32)  # 8KB/p
            ident_f = singles.tile([128, 128], F32)
            ident = singles.tile([128, 128], BF16)
            ones_f = singles.tile([128, 128], F32)
            # l-matmul lhsT: [128,128] ones -> row-sum REPLICATED on all 128
            # output partitions (same row cost as a 1-wide lhsT, but the
            # weight load pipelines and no separate broadcast is needed)
            ones_m = singles.tile([128, 128], BF16)
            qT = singles.tile([128, G, S], BF16)  # 16KB/p
            kT = singles.tile([128, S], BF16)  # 4KB/p
            v_sb = singles.tile([128, NKT, D], BF16)  # 4KB/p
            onrm = singles.tile([128, G, S], BF16)  # 16KB/p

            make_identity(nc, ident_f)
            nc.scalar.activation(out=ident[:], in_=ident_f[:], func=AF.Copy)
            nc.vector.memset(ones_f, 1.0)
            nc.scalar.activation(out=ones_m[:], in_=ones_f[:], func=AF.Copy)
            # causal mask built on-device (gpsimd, idle at t=0): tile j is the
            # additive mask for k-rows 128j..128j+127 vs q-columns of a chunk:
            # keep 0 where q - p - 128j >= 0, else -1e9.
            nc.gpsimd.memset(mask_sb, 0.0)
            for j in range(4):
                nc.gpsimd.affine_select(
                    out=mask_sb[:, j, :], in_=mask_sb[:, j, :],
                    compare_op=mybir.AluOpType.is_ge, fill=-1e9,
                    base=-128 * j, channel_multiplier=-1, pattern=[[1, SC]],
                )

            def wsplit(dst, dram, width, nsplit):
                """Load [128, n, width] SBUF tile from partition-major DRAM."""
                ntiles = dst.shape[1]
                step = ntiles // nsplit
                for i in range(nsplit):
                    t0 = i * step
                    nc.sync.dma_start(
                        out=dst[:, t0 : t0 + step, :],
                        in_=dram[:, t0 * width : (t0 + step) * width],
                    )

            # K/V weights first: K-proj of chunk 0 can start early
            wsplit(wk_sb, wk, D, 4)
            wsplit(wv_sb, wv, D, 4)

            state = {}  # (qc, h) -> (o_ps, l_ps)

            def x_chunk(dram, sc, tag):
                """Stream one [128, NET, SC] x chunk, split along t."""
                # xq double-buffers: its ring-1 WAR (next chunk's DMA vs the
                # 4 head chains of the previous Q-proj) otherwise serializes
                # the stream late in the run.
                xsb = xcp.tile(
                    [128, NET, SC], BF16, tag=tag, name=tag,
                    bufs=2 if tag == "xq" else 1,
                )
                rows = slice(sc * 128, (sc + 1) * 128)
                nsplit = 16 if sc == 0 else 8  # finer first chunk: earlier start
                step = NET // nsplit
                for i in range(nsplit):
                    t0 = i * step
                    nc.sync.dma_start(
                        out=xsb[:, t0 : t0 + step, :],
                        in_=dram[rows, t0 * SC : (t0 + step) * SC],
                    )
                return xsb

            def prefetch_x(sc):
                return (
                    x_chunk(xk, sc, "xk"),
                    x_chunk(xv, sc, "xv"),
                    x_chunk(xq, sc, "xq"),
                )

            def proj_block(sc, pre=None):
                ssl = slice(sc * SC, (sc + 1) * SC)
                # K projection
                xsb = pre[0] if pre else x_chunk(xk, sc, "xk")
                ps = acc.tile([128, SC], F32, tag="acc", name="kps")
                for t in range(NET):
                    nc.tensor.matmul(
                        ps[:], lhsT=wk_sb[:, t, :], rhs=xsb[:, t, :],
                        start=(t == 0), stop=(t == NET - 1),
                    )
                nc.vector.tensor_copy(out=kT[:, ssl], in_=ps[:])
                # V projection, then transpose to [s, d] tiles
                xsb = pre[1] if pre else x_chunk(xv, sc, "xv")
                ps = acc.tile([128, SC], F32, tag="acc", name="vps")
                for t in range(NET):
                    nc.tensor.matmul(
                        ps[:], lhsT=wv_sb[:, t, :], rhs=xsb[:, t, :],
                        start=(t == 0), stop=(t == NET - 1),
                    )
                vt = small.tile([128, SC], BF16, tag="vt", name="vt")
                nc.scalar.activation(out=vt[:], in_=ps[:], func=AF.Copy)
                for i in range(SC // 128):
                    tp = misc.tile([128, 128], BF16, tag="mx", name="tp")
                    nc.tensor.transpose(tp[:], vt[:, i * 128 : (i + 1) * 128], ident[:])
                    nc.vector.tensor_copy(out=v_sb[:, sc * 4 + i, :], in_=tp[:])
                # Q projection (4 heads); wq streamed during K/V of chunk 0
                if sc == 0:
                    wsplit(wq_sb, wq, GD, 8)
                xsb = pre[2] if pre else x_chunk(xq, sc, "xq")
                for h in range(G):
                    ps = acc.tile([128, SC], F32, tag="acc", name="qps")
                    for t in range(NET):
                        nc.tensor.matmul(
                            ps[:], lhsT=wq_sb[:, t, h * D : (h + 1) * D],
                            rhs=xsb[:, t, :], start=(t == 0), stop=(t == NET - 1),
                        )
                    # fold softmax scale into Q
                    nc.scalar.activation(
                        out=qT[:, h, ssl], in_=ps[:], func=AF.Copy, scale=SCALE
                    )

            def attn_head(qc, h, extras=None, hook=None):
                nkt = 4 * (qc + 1)
                # two non-diagonal tiles first (no mask-add latency in front
                # of the first exp), then the column-trimmed diagonal tiles,
                # then the rest. qc=0 has only diagonal tiles.
                nd = list(range(nkt - 4))
                order = [(kt, None) for kt in nd[:2]]
                order += [(nkt - 4 + j, j) for j in range(4)]
                order += [(kt, None) for kt in nd[2:]]
                n = len(order)
                o_ps = ops.tile([128, SC], F32, tag="o", name="o_ps")
                l_ps = lps.tile([128, SC], F32, tag="l", name="l_ps")
                state[(qc, h)] = (o_ps, l_ps)
                pend = []
                nfl = [0]

                def flush_one():
                    kt, cl, p = pend.pop(0)
                    first = nfl[0] == 0
                    last = nfl[0] == n - 1
                    nc.tensor.matmul(
                        o_ps[:, cl], lhsT=v_sb[:, kt, :], rhs=p[:, cl],
                        start=first, stop=last, skip_group_check=True,
                    )
                    nc.tensor.matmul(
                        l_ps[:, cl], lhsT=ones_m[:, :], rhs=p[:, cl],
                        start=first, stop=last, skip_group_check=True,
                    )
                    nfl[0] += 1

                for i, (kt, j) in enumerate(order):
                    cl = slice(128 * j, SC) if j is not None else slice(0, SC)
                    qsl = slice(qc * SC + cl.start, (qc + 1) * SC)
                    s_ps = acc.tile([128, SC], F32, tag="acc", name="s_ps")
                    nc.tensor.matmul(
                        s_ps[:, cl], lhsT=kT[:, kt * 128 : (kt + 1) * 128],
                        rhs=qT[:, h, qsl], start=True, stop=True,
                    )
                    if j is not None:
                        nc.vector.tensor_add(s_ps[:, cl], s_ps[:, cl], mask_sb[:, j, cl])
                    p = pexp.tile([128, SC], BF16, tag="p", name="p")
                    nc.scalar.activation(out=p[:, cl], in_=s_ps[:, cl], func=AF.Exp)
                    pend.append((kt, cl, p))
                    # norm of the previous head + outproj filler go after the
                    # diagonal mask-adds (i=2..5) so the DVE queue serves the
                    # adds (which gate exp -> PV) first.
                    if i == 6 and hook is not None:
                        hook()
                        hook = None
                    if extras is not None and i >= 6:
                        for _ in range(2):
                            ex = next(extras, None)
                            if ex is not None:
                                ex()
                    if i >= 3:
                        flush_one()
                while pend:
                    flush_one()
                return hook  # non-None if the loop was too short to fire it

            def norm(qc, h):
                # l_ps already holds l broadcast on all 128 partitions
                o_ps, l_ps = state.pop((qc, h))
                qsl = slice(qc * SC, (qc + 1) * SC)
                rinv = small.tile([128, SC], F32, tag="ri", name="rinv")
                nc.vector.reciprocal_approx_fast(out=rinv[:], in_=l_ps[:])
                nc.vector.tensor_mul(onrm[:, h, qsl], o_ps[:], rinv[:])

            def op_gen(qc):
                # out-projection of chunk qc: 16 chains of 4 matmuls; the 4
                # e-chunks of one s-tile stage into one SBUF tile so the out
                # DMA writes full 4KB partition lines.
                for sti in range(4):
                    st = qc * 4 + sti
                    stl = slice(st * 128, (st + 1) * 128)
                    holder = {}
                    for ec in range(NEC):
                        esl = slice(ec * SC, (ec + 1) * SC)

                        def chain(stl=stl, esl=esl, st=st, ec=ec, holder=holder):
                            if ec == 0:
                                holder["ob"] = obp.tile(
                                    [128, NEC, SC], BF16, tag="ob", name="ob"
                                )
                            ps = acc.tile([128, SC], F32, tag="acc", name="ops_ps")
                            for hh in range(G):
                                nc.tensor.matmul(
                                    ps[:], lhsT=onrm[:, hh, stl], rhs=wo_sb[:, hh, esl],
                                    start=(hh == 0), stop=(hh == G - 1),
                                )
                            nc.vector.tensor_copy(out=holder["ob"][:, ec, :], in_=ps[:])
                            if ec == NEC - 1:
                                nc.sync.dma_start(
                                    out=out[st * 128 : (st + 1) * 128, :],
                                    in_=holder["ob"][:],
                                )

                        yield chain

            def attn_block(qc, extras=None):
                for h in range(G):
                    if h >= 1:
                        prev = (qc, h - 1)
                    elif qc >= 1:
                        prev = (qc - 1, 3)
                    else:
                        prev = None
                    hook = (lambda p=prev: norm(*p)) if prev is not None else None
                    left = attn_head(qc, h, extras, hook)
                    if left is not None:
                        left()  # loop too short (qc=0): norm after the loop
                if extras is not None:
                    for ex in extras:  # drain leftovers (shouldn't happen)
                        ex()

            # ---- emission schedule ----
            proj_block(0)
            proj_block(1)
            # wo queues behind chunk-1 x so it doesn't delay it; it is only
            # needed from the A1-interleaved out-projection onwards.
            wsplit(wo_sb, wo, E, 4)
            attn_block(0)
            proj_block(2)
            # chunk-3 x queues ahead of A1's out-DMAs
            pre3 = prefetch_x(3)
            attn_block(1, extras=op_gen(0))
            proj_block(3, pre=pre3)
            attn_block(2, extras=op_gen(1))
            attn_block(3, extras=op_gen(2))
            norm(3, 3)
            for ch in op_gen(3):
                ch()
    nc.compile()
    return nc


_NC_CACHE = None


def _get_nc():
    global _NC_CACHE
    if _NC_CACHE is None:
        _NC_CACHE = build_nc()
    return _NC_CACHE


def _block_x(xT_bf):
    """[E, S] bf16 -> [sc][p][t][f] partition-major blocks [NSC*128, NET*SC]."""
    return np.ascontiguousarray(
        xT_bf.reshape(NET, 128, NSC, SC).transpose(2, 1, 0, 3).reshape(NSC * 128, NET * SC)
    )


def _block_w(w, width):
    """[ntiles*128, width] -> partition-major [128, ntiles*width]."""
    nt = w.shape[0] // 128
    return np.ascontiguousarray(
        w.reshape(nt, 128, width).transpose(1, 0, 2).reshape(128, nt * width)
    )


def _prep_in_maps(query, key, value, attn_mask, Wq, Wk, Wv, Wo):
    query = np.asarray(query, dtype=np.float32)
    key = np.asarray(key, dtype=np.float32)
    value = np.asarray(value, dtype=np.float32)
    Wq = np.asarray(Wq, dtype=np.float32)
    Wk = np.asarray(Wk, dtype=np.float32)
    Wv = np.asarray(Wv, dtype=np.float32)
    Wo = np.asarray(Wo, dtype=np.float32)
    am = np.asarray(attn_mask)

    xqT = [_block_x(np.ascontiguousarray(query[b].T).astype(NPBF)) for b in range(B)]
    xkT = [_block_x(np.ascontiguousarray(key[b].T).astype(NPBF)) for b in range(B)]
    xvT = [_block_x(np.ascontiguousarray(value[b].T).astype(NPBF)) for b in range(B)]

    # the kernel generates the causal mask on-device; sanity-check the input
    # mask really is causal (it is for this problem by construction)
    assert np.array_equal(
        np.asarray(am[0, 0, :4, :4]), np.tril(np.ones((4, 4), am.dtype))
    )

    in_maps = []
    for b in range(B):
        for g in range(HKV):
            wq_g = np.ascontiguousarray(Wq[g * GD : (g + 1) * GD, :].T).astype(NPBF)
            wk_g = np.ascontiguousarray(Wk[g * D : (g + 1) * D, :].T).astype(NPBF)
            wv_g = np.ascontiguousarray(Wv[g * D : (g + 1) * D, :].T).astype(NPBF)
            wo_g = np.ascontiguousarray(Wo[:, g * GD : (g + 1) * GD].T).astype(NPBF)
            in_maps.append(
                {
                    "xq": xqT[b],
                    "xk": xkT[b],
                    "xv": xvT[b],
                    "wq": _block_w(wq_g, GD),
                    "wk": _block_w(wk_g, D),
                    "wv": _block_w(wv_g, D),
                    "wo": _block_w(wo_g, E),
                }
            )
    return in_maps


def _run(inputs, trace=False, **kw):
    nc = _get_nc()
    in_maps = _prep_in_maps(**inputs)
    res = run_bass_kernel_spmd(nc, in_maps, list(range(NCORES)), trace=trace, **kw)
    outs = [np.asarray(r["out"]) for r in res.results]
    full = np.empty((B, S, E), dtype=np.float32)
    for b in range(B):
        acc = outs[b * HKV].astype(np.float32)
        for g in range(1, HKV):
            acc = acc + outs[b * HKV + g].astype(np.float32)
        full[b] = acc
    return full, res


def kernel(**inputs):
    full, _ = _run(inputs, trace=False)
    return full


# revision 28
# speedup vs baseline: 1.6667x; 1.0201x over previous
"""GQA kernel for Trainium2, 8 NeuronCores.

Sharding: core c = b*4 + g handles batch b, kv-head g (4 query heads).
Host sums the 4 partial out-projections per batch.

Design notes (v3):
- All matmuls bf16 (fp32 PSUM). PE cost is free-dim rows x clock, and the
  clock p-state ramps to 2.4GHz only after ~3us of *continuous* PE busy,
  so the emission order never lets the PE idle: projection, attention and
  out-projection chunks are interleaved, and the out-projection of chunk
  qc is folded into the first head's kt-loop of chunk qc+1.
- DMA descriptors are generated per SBUF partition line, so all DRAM
  layouts are partition-major: x is host-blocked [sc][p][t][f] (16KB
  contiguous per partition per chunk), weights [p][t][..], and the output
  is written per s-tile with 4KB lines. Chunk loads are split along t so
  8 queues stream one chunk in parallel and the first matmul of a chain
  only waits for its own t-range.
- Attention kt order: two non-diagonal tiles first (their exp has no
  DVE mask-add on the critical path, hiding the QK->exp->PV latency at
  each head-loop start), then the 4 diagonal tiles (column-trimmed: tile
  j only touches q columns >= 128j, the rest is fully masked), then the
  remaining tiles. The first flushed PV/l matmul covers all 512 columns
  so PSUM start=True initializes the full accumulator.
- Softmax normalization without a DRAM round trip: l row-sums accumulate
  via ones-matmuls per kt; then l -> bf16 copy (ACT), K=1 matmul
  broadcast across partitions (PE), reciprocal_approx_fast (DVE, ~0.7us
  vs 3.3us for reciprocal), multiply into onrm.
"""

import sys

import numpy as np

for _p in ("/opt/trn_rl_repo",):
    if _p not in sys.path:
        sys.path.insert(0, _p)

import ml_dtypes

import concourse.mybir as mybir
from concourse import bacc
from concourse.bass_utils import run_bass_kernel_spmd
from concourse.masks import make_identity
from concourse.tile import TileContext

B, S, E = 2, 2048, 2048
H, HKV = 16, 4
D = E // H  # 128
G = H // HKV  # 4 query heads per kv head
GD = G * D  # 512
NCORES = B * HKV  # 8
SC = 512  # s/q chunk width (free dim of matmuls)
NSC = S // SC  # 4
NET = E // 128  # 16 e-tiles (contraction)
NKT = S // 128  # 16 k-tiles
NEC = E // SC  # 4 e-chunks for output
SCALE = 1.0 / float(np.sqrt(D))

F32 = mybir.dt.float32
BF16 = mybir.dt.bfloat16
AF = mybir.ActivationFunctionType
NPBF = np.dtype(ml_dtypes.bfloat16)


def build_nc():
    nc = bacc.Bacc()
    # x inputs: [sc][p][t][f] partition-major blocks -> [NSC*128, NET*SC]
    xq = nc.declare_dram_parameter("xq", [NSC * 128, NET * SC], BF16, isOutput=False)
    xk = nc.declare_dram_parameter("xk", [NSC * 128, NET * SC], BF16, isOutput=False)
    xv = nc.declare_dram_parameter("xv", [NSC * 128, NET * SC], BF16, isOutput=False)
    # weights partition-major: [p][t][..]
    wq = nc.declare_dram_parameter("wq", [128, NET * GD], BF16, isOutput=False)
    wk = nc.declare_dram_parameter("wk", [128, NET * D], BF16, isOutput=False)
    wv = nc.declare_dram_parameter("wv", [128, NET * D], BF16, isOutput=False)
    wo = nc.declare_dram_parameter("wo", [128, G * E], BF16, isOutput=False)
    # natural [S, E]: written per s-tile as [128, E] full-width rows
    out = nc.declare_dram_parameter("out", [S, E], BF16, isOutput=True)

    with TileContext(nc) as tc:
        with (
            tc.tile_pool(name="singles", bufs=1) as singles,
            tc.tile_pool(name="xc", bufs=1) as xcp,
            tc.tile_pool(name="pexp", bufs=6) as pexp,
            tc.tile_pool(name="small", bufs=2) as small,
            tc.tile_pool(name="ob", bufs=2) as obp,
            tc.tile_pool(name="acc", bufs=3, space="PSUM") as acc,
            tc.tile_pool(name="ops", bufs=2, space="PSUM") as ops,
            tc.tile_pool(name="lps", bufs=2, space="PSUM") as lps,
            tc.tile_pool(name="misc", bufs=1, space="PSUM") as misc,
        ):
            # ---- constants / weights resident in SBUF ----
            wq_sb = singles.tile([128, NET, GD], BF16)  # 16KB/p
            wk_sb = singles.tile([128, NET, D], BF16)  # 4KB/p
            wv_sb = singles.tile([128, NET, D], BF16)  # 4KB/p
            wo_sb = singles.tile([128, G, E], BF16)  # 16KB/p
            mask_sb = singles.tile([128, 4, SC], F32)  # 8KB/p
            ident_f = singles.tile([128, 128], F32)
            ident = singles.tile([128, 128], BF16)
            ones_f = singles.tile([128, 128], F32)
            # l-matmul lhsT: [128,128] ones -> row-sum REPLICATED on all 128
            # output partitions (same row cost as a 1-wide lhsT, but the
            # weight load pipelines and no separate broadcast is needed)
            ones_m = singles.tile([128, 128], BF16)
            qT = singles.tile([128, G, S], BF16)  # 16KB/p
            kT = singles.tile([128, S], BF16)  # 4KB/p
            v_sb = singles.tile([128, NKT, D], BF16)  # 4KB/p
            onrm = singles.tile([128, G, S], BF16)  # 16KB/p

            make_identity(nc, ident_f)
            nc.scalar.activation(out=ident[:], in_=ident_f[:], func=AF.Copy)
            nc.vector.memset(ones_f, 1.0)
            nc.scalar.activation(out=ones_m[:], in_=ones_f[:], func=AF.Copy)
            # causal mask built on-device (gpsimd, idle at t=0): tile j is the
            # additive mask for k-rows 128j..128j+127 vs q-columns of a chunk:
            # keep 0 where q - p - 128j >= 0, else -1e9.
            nc.gpsimd.memset(mask_sb, 0.0)
            for j in range(4):
                nc.gpsimd.affine_select(
                    out=mask_sb[:, j, :], in_=mask_sb[:, j, :],
                    compare_op=mybir.AluOpType.is_ge, fill=-1e9,
                    base=-128 * j, channel_multiplier=-1, pattern=[[1, SC]],
                )

            def wsplit(dst, dram, width, nsplit):
                """Load [128, n, width] SBUF tile from partition-major DRAM."""
                ntiles = dst.shape[1]
                step = ntiles // nsplit
                for i in range(nsplit):
                    t0 = i * step
                    nc.sync.dma_start(
                        out=dst[:, t0 : t0 + step, :],
                        in_=dram[:, t0 * width : (t0 + step) * width],
                    )

            # K/V weights first: K-proj of chunk 0 can start early
            wsplit(wk_sb, wk, D, 4)
            wsplit(wv_sb, wv, D, 4)

            state = {}  # (qc, h) -> (o_ps, l_ps)

            def x_chunk(dram, sc, tag):
                """Stream one [128, NET, SC] x chunk, split along t."""
                # xq double-buffers: its ring-1 WAR (next chunk's DMA vs the
                # 4 head chains of the previous Q-proj) otherwise serializes
                # the stream late in the run.
                xsb = xcp.tile(
                    [128, NET, SC], BF16, tag=tag, name=tag,
                    bufs=2 if tag == "xq" else 1,
                )
                rows = slice(sc * 128, (sc + 1) * 128)
                # 4KB descriptor lines in steady state; finer first chunk so
                # the first projection chain starts earlier
                nsplit = 8 if sc == 0 else 4
                step = NET // nsplit
                for i in range(nsplit):
                    t0 = i * step
                    nc.sync.dma_start(
                        out=xsb[:, t0 : t0 + step, :],
                        in_=dram[rows, t0 * SC : (t0 + step) * SC],
                    )
                return xsb

            def prefetch_x(sc):
                return (
                    x_chunk(xk, sc, "xk"),
                    x_chunk(xv, sc, "xv"),
                    x_chunk(xq, sc, "xq"),
                )

            def proj_block(sc, pre=None):
                ssl = slice(sc * SC, (sc + 1) * SC)
                # K projection
                xsb = pre[0] if pre else x_chunk(xk, sc, "xk")
                ps = acc.tile([128, SC], F32, tag="acc", name="kps")
                for t in range(NET):
                    nc.tensor.matmul(
                        ps[:], lhsT=wk_sb[:, t, :], rhs=xsb[:, t, :],
                        start=(t == 0), stop=(t == NET - 1),
                    )
                nc.vector.tensor_copy(out=kT[:, ssl], in_=ps[:])
                # V projection, then transpose to [s, d] tiles
                xsb = pre[1] if pre else x_chunk(xv, sc, "xv")
                ps = acc.tile([128, SC], F32, tag="acc", name="vps")
                for t in range(NET):
                    nc.tensor.matmul(
                        ps[:], lhsT=wv_sb[:, t, :], rhs=xsb[:, t, :],
                        start=(t == 0), stop=(t == NET - 1),
                    )
                vt = small.tile([128, SC], BF16, tag="vt", name="vt")
                nc.scalar.activation(out=vt[:], in_=ps[:], func=AF.Copy)
                for i in range(SC // 128):
                    tp = misc.tile([128, 128], BF16, tag="mx", name="tp")
                    nc.tensor.transpose(tp[:], vt[:, i * 128 : (i + 1) * 128], ident[:])
                    nc.vector.tensor_copy(out=v_sb[:, sc * 4 + i, :], in_=tp[:])
                # Q projection (4 heads); wq streamed during K/V of chunk 0
                if sc == 0:
                    wsplit(wq_sb, wq, GD, 8)
                xsb = pre[2] if pre else x_chunk(xq, sc, "xq")
                for h in range(G):
                    ps = acc.tile([128, SC], F32, tag="acc", name="qps")
                    for t in range(NET):
                        nc.tensor.matmul(
                            ps[:], lhsT=wq_sb[:, t, h * D : (h + 1) * D],
                            rhs=xsb[:, t, :], start=(t == 0), stop=(t == NET - 1),
                        )
                    # fold softmax scale into Q
                    nc.scalar.activation(
                        out=qT[:, h, ssl], in_=ps[:], func=AF.Copy, scale=SCALE
                    )

            def attn_head(qc, h, extras=None, hook=None):
                nkt = 4 * (qc + 1)
                # two non-diagonal tiles first (no mask-add latency in front
                # of the first exp), then the column-trimmed diagonal tiles,
                # then the rest. qc=0 has only diagonal tiles.
                nd = list(range(nkt - 4))
                order = [(kt, None) for kt in nd[:2]]
                order += [(nkt - 4 + j, j) for j in range(4)]
                order += [(kt, None) for kt in nd[2:]]
                n = len(order)
                o_ps = ops.tile([128, SC], F32, tag="o", name="o_ps")
                l_ps = lps.tile([128, SC], F32, tag="l", name="l_ps")
                state[(qc, h)] = (o_ps, l_ps)
                pend = []
                nfl = [0]

                def flush_one():
                    kt, cl, p = pend.pop(0)
                    first = nfl[0] == 0
                    last = nfl[0] == n - 1
                    nc.tensor.matmul(
                        o_ps[:, cl], lhsT=v_sb[:, kt, :], rhs=p[:, cl],
                        start=first, stop=last, skip_group_check=True,
                    )
                    nc.tensor.matmul(
                        l_ps[:, cl], lhsT=ones_m[:, :], rhs=p[:, cl],
                        start=first, stop=last, skip_group_check=True,
                    )
                    nfl[0] += 1

                for i, (kt, j) in enumerate(order):
                    cl = slice(128 * j, SC) if j is not None else slice(0, SC)
                    qsl = slice(qc * SC + cl.start, (qc + 1) * SC)
                    s_ps = acc.tile([128, SC], F32, tag="acc", name="s_ps")
                    nc.tensor.matmul(
                        s_ps[:, cl], lhsT=kT[:, kt * 128 : (kt + 1) * 128],
                        rhs=qT[:, h, qsl], start=True, stop=True,
                    )
                    if j is not None:
                        nc.vector.tensor_add(s_ps[:, cl], s_ps[:, cl], mask_sb[:, j, cl])
                    p = pexp.tile([128, SC], BF16, tag="p", name="p")
                    nc.scalar.activation(out=p[:, cl], in_=s_ps[:, cl], func=AF.Exp)
                    pend.append((kt, cl, p))
                    # norm of the previous head + outproj filler go after the
                    # diagonal mask-adds (i=2..5) so the DVE queue serves the
                    # adds (which gate exp -> PV) first; the norm fires one
                    # iteration before the outproj filler that reads it.
                    if i == 5 and hook is not None:
                        hook()
                        hook = None
                    if extras is not None and i >= 6:
                        for _ in range(2):
                            ex = next(extras, None)
                            if ex is not None:
                                ex()
                    if i >= 3:
                        flush_one()
                while pend:
                    flush_one()
                return hook  # non-None if the loop was too short to fire it

            def norm(qc, h):
                # l_ps already holds l broadcast on all 128 partitions
                o_ps, l_ps = state.pop((qc, h))
                qsl = slice(qc * SC, (qc + 1) * SC)
                rinv = small.tile([128, SC], F32, tag="ri", name="rinv")
                nc.vector.reciprocal_approx_fast(out=rinv[:], in_=l_ps[:])
                nc.vector.tensor_mul(onrm[:, h, qsl], o_ps[:], rinv[:])

            def op_gen(qc):
                # out-projection of chunk qc: 16 chains of 4 matmuls; the 4
                # e-chunks of one s-tile stage into one SBUF tile so the out
                # DMA writes full 4KB partition lines.
                for sti in range(4):
                    st = qc * 4 + sti
                    stl = slice(st * 128, (st + 1) * 128)
                    holder = {}
                    for ec in range(NEC):
                        esl = slice(ec * SC, (ec + 1) * SC)

                        def chain(stl=stl, esl=esl, st=st, ec=ec, holder=holder):
                            if ec == 0:
                                holder["ob"] = obp.tile(
                                    [128, NEC, SC], BF16, tag="ob", name="ob"
                                )
                            ps = acc.tile([128, SC], F32, tag="acc", name="ops_ps")
                            for hh in range(G):
                                nc.tensor.matmul(
                                    ps[:], lhsT=onrm[:, hh, stl], rhs=wo_sb[:, hh, esl],
                                    start=(hh == 0), stop=(hh == G - 1),
                                )
                            nc.vector.tensor_copy(out=holder["ob"][:, ec, :], in_=ps[:])
                            if ec == NEC - 1:
                                # split over 4 queues so the last tile doesn't
                                # leave a single-queue drain tail
                                for pr in range(0, 128, 32):
                                    nc.sync.dma_start(
                                        out=out[st * 128 + pr : st * 128 + pr + 32, :],
                                        in_=holder["ob"][pr : pr + 32, :, :],
                                    )

                        yield chain

            def attn_block(qc, extras=None):
                for h in range(G):
                    if h >= 1:
                        prev = (qc, h - 1)
                    elif qc >= 1:
                        prev = (qc - 1, 3)
                    else:
                        prev = None
                    hook = (lambda p=prev: norm(*p)) if prev is not None else None
                    left = attn_head(qc, h, extras, hook)
                    if left is not None:
                        left()  # loop too short (qc=0): norm after the loop
                if extras is not None:
                    for ex in extras:  # drain leftovers (shouldn't happen)
                        ex()

            # ---- emission schedule ----
            # A0 sits right after P0: it only needs chunk-0 projections and
            # its 10us of DMA-free PE work covers the chunk-1 x stream.
            proj_block(0)
            attn_block(0)
            proj_block(1)
            proj_block(2)
            # wo queues behind the x chunks it must not delay; it is only
            # needed from the A1-interleaved out-projection onwards.
            wsplit(wo_sb, wo, E, 4)
            # chunk-3 x queues ahead of A1's out-DMAs
            pre3 = prefetch_x(3)
            attn_block(1, extras=op_gen(0))
            proj_block(3, pre=pre3)
            attn_block(2, extras=op_gen(1))
            attn_block(3, extras=op_gen(2))
            norm(3, 3)
            for ch in op_gen(3):
                ch()
    nc.compile()
    return nc


_NC_CACHE = None


def _get_nc():
    global _NC_CACHE
    if _NC_CACHE is None:
        _NC_CACHE = build_nc()
    return _NC_CACHE


def _block_x(xT_bf):
    """[E, S] bf16 -> [sc][p][t][f] partition-major blocks [NSC*128, NET*SC]."""
    return np.ascontiguousarray(
        xT_bf.reshape(NET, 128, NSC, SC).transpose(2, 1, 0, 3).reshape(NSC * 128, NET * SC)
    )


def _block_w(w, width):
    """[ntiles*128, width] -> partition-major [128, ntiles*width]."""
    nt = w.shape[0] // 128
    return np.ascontiguousarray(
        w.reshape(nt, 128, width).transpose(1, 0, 2).reshape(128, nt * width)
    )


def _prep_in_maps(query, key, value, attn_mask, Wq, Wk, Wv, Wo):
    query = np.asarray(query, dtype=np.float32)
    key = np.asarray(key, dtype=np.float32)
    value = np.asarray(value, dtype=np.float32)
    Wq = np.asarray(Wq, dtype=np.float32)
    Wk = np.asarray(Wk, dtype=np.float32)
    Wv = np.asarray(Wv, dtype=np.float32)
    Wo = np.asarray(Wo, dtype=np.float32)
    am = np.asarray(attn_mask)

    xqT = [_block_x(np.ascontiguousarray(query[b].T).astype(NPBF)) for b in range(B)]
    xkT = [_block_x(np.ascontiguousarray(key[b].T).astype(NPBF)) for b in range(B)]
    xvT = [_block_x(np.ascontiguousarray(value[b].T).astype(NPBF)) for b in range(B)]

    # the kernel generates the causal mask on-device; sanity-check the input
    # mask really is causal (it is for this problem by construction)
    assert np.array_equal(
        np.asarray(am[0, 0, :4, :4]), np.tril(np.ones((4, 4), am.dtype))
    )

    in_maps = []
    for b in range(B):
        for g in range(HKV):
            wq_g = np.ascontiguousarray(Wq[g * GD : (g + 1) * GD, :].T).astype(NPBF)
            wk_g = np.ascontiguousarray(Wk[g * D : (g + 1) * D, :].T).astype(NPBF)
            wv_g = np.ascontiguousarray(Wv[g * D : (g + 1) * D, :].T).astype(NPBF)
            wo_g = np.ascontiguousarray(Wo[:, g * GD : (g + 1) * GD].T).astype(NPBF)
            in_maps.append(
                {
                    "xq": xqT[b],
                    "xk": xkT[b],
                    "xv": xvT[b],
                    "wq": _block_w(wq_g, GD),
                    "wk": _block_w(wk_g, D),
                    "wv": _block_w(wv_g, D),
                    "wo": _block_w(wo_g, E),
                }
            )
    return in_maps


def _run(inputs, trace=False, **kw):
    nc = _get_nc()
    in_maps = _prep_in_maps(**inputs)
    res = run_bass_kernel_spmd(nc, in_maps, list(range(NCORES)), trace=trace, **kw)
    outs = [np.asarray(r["out"]) for r in res.results]
    full = np.empty((B, S, E), dtype=np.float32)
    for b in range(B):
        acc = outs[b * HKV].astype(np.float32)
        for g in range(1, HKV):
            acc = acc + outs[b * HKV + g].astype(np.float32)
        full[b] = acc
    return full, res


def kernel(**inputs):
    full, _ = _run(inputs, trace=False)
    return full


# revision 32
# speedup vs baseline: 1.7247x; 1.0348x over previous
"""GQA kernel for Trainium2, 8 NeuronCores.

Sharding: core c = b*4 + g handles batch b, kv-head g (4 query heads).
Host sums the 4 partial out-projections per batch.

Design notes (v3):
- All matmuls bf16 (fp32 PSUM). PE cost is free-dim rows x clock, and the
  clock p-state ramps to 2.4GHz only after ~3us of *continuous* PE busy,
  so the emission order never lets the PE idle: projection, attention and
  out-projection chunks are interleaved, and the out-projection of chunk
  qc is folded into the first head's kt-loop of chunk qc+1.
- DMA descriptors are generated per SBUF partition line, so all DRAM
  layouts are partition-major: x is host-blocked [sc][p][t][f] (16KB
  contiguous per partition per chunk), weights [p][t][..], and the output
  is written per s-tile with 4KB lines. Chunk loads are split along t so
  8 queues stream one chunk in parallel and the first matmul of a chain
  only waits for its own t-range.
- Attention kt order: two non-diagonal tiles first (their exp has no
  DVE mask-add on the critical path, hiding the QK->exp->PV latency at
  each head-loop start), then the 4 diagonal tiles (column-trimmed: tile
  j only touches q columns >= 128j, the rest is fully masked), then the
  remaining tiles. The first flushed PV/l matmul covers all 512 columns
  so PSUM start=True initializes the full accumulator.
- Softmax normalization without a DRAM round trip: l row-sums accumulate
  via ones-matmuls per kt; then l -> bf16 copy (ACT), K=1 matmul
  broadcast across partitions (PE), reciprocal_approx_fast (DVE, ~0.7us
  vs 3.3us for reciprocal), multiply into onrm.
"""

import sys

import numpy as np

for _p in ("/opt/trn_rl_repo",):
    if _p not in sys.path:
        sys.path.insert(0, _p)

import ml_dtypes

import concourse.mybir as mybir
from concourse import bacc
from concourse.bass_utils import run_bass_kernel_spmd
from concourse.masks import make_identity
from concourse.tile import TileContext

B, S, E = 2, 2048, 2048
H, HKV = 16, 4
D = E // H  # 128
G = H // HKV  # 4 query heads per kv head
GD = G * D  # 512
NCORES = B * HKV  # 8
SC = 512  # s/q chunk width (free dim of matmuls)
NSC = S // SC  # 4
NET = E // 128  # 16 e-tiles (contraction)
NKT = S // 128  # 16 k-tiles
NEC = E // SC  # 4 e-chunks for output
SCALE = 1.0 / float(np.sqrt(D))

F32 = mybir.dt.float32
BF16 = mybir.dt.bfloat16
AF = mybir.ActivationFunctionType
NPBF = np.dtype(ml_dtypes.bfloat16)


def build_nc():
    nc = bacc.Bacc()
    # x inputs: [sc][p][t][f] partition-major blocks -> [NSC*128, NET*SC]
    xq = nc.declare_dram_parameter("xq", [NSC * 128, NET * SC], BF16, isOutput=False)
    xk = nc.declare_dram_parameter("xk", [NSC * 128, NET * SC], BF16, isOutput=False)
    xv = nc.declare_dram_parameter("xv", [NSC * 128, NET * SC], BF16, isOutput=False)
    # weights partition-major: [p][t][..]
    wq = nc.declare_dram_parameter("wq", [128, NET * GD], BF16, isOutput=False)
    wk = nc.declare_dram_parameter("wk", [128, NET * D], BF16, isOutput=False)
    wv = nc.declare_dram_parameter("wv", [128, NET * D], BF16, isOutput=False)
    wo = nc.declare_dram_parameter("wo", [128, G * E], BF16, isOutput=False)
    # natural [S, E]: written per s-tile as [128, E] full-width rows
    out = nc.declare_dram_parameter("out", [S, E], BF16, isOutput=True)

    with TileContext(nc) as tc:
        with (
            tc.tile_pool(name="singles", bufs=1) as singles,
            tc.tile_pool(name="xc", bufs=1) as xcp,
            tc.tile_pool(name="pexp", bufs=7) as pexp,
            tc.tile_pool(name="small", bufs=2) as small,
            tc.tile_pool(name="ob", bufs=2) as obp,
            tc.tile_pool(name="acc", bufs=4, space="PSUM") as acc,
            tc.tile_pool(name="ops", bufs=2, space="PSUM") as ops,
            tc.tile_pool(name="lps", bufs=2, space="PSUM") as lps,
        ):
            # ---- constants / weights resident in SBUF ----
            wq_sb = singles.tile([128, NET, GD], BF16)  # 16KB/p
            wk_sb = singles.tile([128, NET, D], BF16)  # 4KB/p
            wv_sb = singles.tile([128, NET, D], BF16)  # 4KB/p
            wo_sb = singles.tile([128, G, E], BF16)  # 16KB/p
            mask_sb = singles.tile([128, 4, SC], F32)  # 8KB/p
            ident_f = singles.tile([128, 128], F32)
            ident = singles.tile([128, 128], BF16)
            ones_f = singles.tile([128, 128], F32)
            # l-matmul lhsT: [128,128] ones -> row-sum REPLICATED on all 128
            # output partitions (same row cost as a 1-wide lhsT, but the
            # weight load pipelines and no separate broadcast is needed)
            ones_m = singles.tile([128, 128], BF16)
            qT = singles.tile([128, G, S], BF16)  # 16KB/p
            kT = singles.tile([128, S], BF16)  # 4KB/p
            v_sb = singles.tile([128, NKT, D], BF16)  # 4KB/p
            onrm = singles.tile([128, G, S], BF16)  # 16KB/p

            make_identity(nc, ident_f)
            nc.scalar.activation(out=ident[:], in_=ident_f[:], func=AF.Copy)
            nc.vector.memset(ones_f, 1.0)
            nc.scalar.activation(out=ones_m[:], in_=ones_f[:], func=AF.Copy)
            # causal mask built on-device (gpsimd, idle at t=0): tile j is the
            # additive mask for k-rows 128j..128j+127 vs q-columns of a chunk:
            # keep 0 where q - p - 128j >= 0, else -1e9.
            nc.gpsimd.memset(mask_sb, 0.0)
            for j in range(4):
                nc.gpsimd.affine_select(
                    out=mask_sb[:, j, :], in_=mask_sb[:, j, :],
                    compare_op=mybir.AluOpType.is_ge, fill=-1e9,
                    base=-128 * j, channel_multiplier=-1, pattern=[[1, SC]],
                )

            def wsplit(dst, dram, width, nsplit):
                """Load [128, n, width] SBUF tile from partition-major DRAM."""
                ntiles = dst.shape[1]
                step = ntiles // nsplit
                for i in range(nsplit):
                    t0 = i * step
                    nc.sync.dma_start(
                        out=dst[:, t0 : t0 + step, :],
                        in_=dram[:, t0 * width : (t0 + step) * width],
                    )

            # Q weights first: Q-proj of chunk 0 starts as early as possible
            wsplit(wq_sb, wq, GD, 8)

            state = {}  # (qc, h) -> (o_ps, l_ps)

            def x_chunk(dram, sc, tag):
                """Stream one [128, NET, SC] x chunk, split along t."""
                # xq double-buffers: its ring-1 WAR (next chunk's DMA vs the
                # 4 head chains of the previous Q-proj) otherwise serializes
                # the stream late in the run.
                xsb = xcp.tile(
                    [128, NET, SC], BF16, tag=tag, name=tag,
                    bufs=2 if tag == "xq" else 1,
                )
                rows = slice(sc * 128, (sc + 1) * 128)
                # 4KB descriptor lines in steady state; finer first chunk so
                # the first projection chain starts earlier
                nsplit = 8 if sc == 0 else 4
                step = NET // nsplit
                for i in range(nsplit):
                    t0 = i * step
                    nc.sync.dma_start(
                        out=xsb[:, t0 : t0 + step, :],
                        in_=dram[rows, t0 * SC : (t0 + step) * SC],
                    )
                return xsb

            def prefetch_x(sc):
                return (
                    x_chunk(xq, sc, "xq"),
                    x_chunk(xk, sc, "xk"),
                    x_chunk(xv, sc, "xv"),
                )

            def proj_block(sc, pre=None):
                # Q first: its 13.6us of chains pace the chunk's K/V x stream
                ssl = slice(sc * SC, (sc + 1) * SC)
                xsb = pre[0] if pre else x_chunk(xq, sc, "xq")
                for h in range(G):
                    ps = acc.tile([128, SC], F32, tag="acc", name="qps")
                    for t in range(NET):
                        nc.tensor.matmul(
                            ps[:], lhsT=wq_sb[:, t, h * D : (h + 1) * D],
                            rhs=xsb[:, t, :], start=(t == 0), stop=(t == NET - 1),
                        )
                    # fold softmax scale into Q
                    nc.scalar.activation(
                        out=qT[:, h, ssl], in_=ps[:], func=AF.Copy, scale=SCALE
                    )
                    if sc == 0 and h == 0:
                        wsplit(wk_sb, wk, D, 4)
                        wsplit(wv_sb, wv, D, 4)
                # K projection
                xsb = pre[1] if pre else x_chunk(xk, sc, "xk")
                ps = acc.tile([128, SC], F32, tag="acc", name="kps")
                for t in range(NET):
                    nc.tensor.matmul(
                        ps[:], lhsT=wk_sb[:, t, :], rhs=xsb[:, t, :],
                        start=(t == 0), stop=(t == NET - 1),
                    )
                nc.vector.tensor_copy(out=kT[:, ssl], in_=ps[:])
                # V projection, then transpose to [s, d] tiles
                xsb = pre[2] if pre else x_chunk(xv, sc, "xv")
                ps = acc.tile([128, SC], F32, tag="acc", name="vps")
                for t in range(NET):
                    nc.tensor.matmul(
                        ps[:], lhsT=wv_sb[:, t, :], rhs=xsb[:, t, :],
                        start=(t == 0), stop=(t == NET - 1),
                    )
                vt = small.tile([128, SC], BF16, tag="vt", name="vt")
                nc.scalar.activation(out=vt[:], in_=ps[:], func=AF.Copy)
                for i in range(SC // 128):
                    tp = acc.tile([128, 128], BF16, tag="acc", name="tp")
                    nc.tensor.transpose(tp[:], vt[:, i * 128 : (i + 1) * 128], ident[:])
                    nc.vector.tensor_copy(out=v_sb[:, sc * 4 + i, :], in_=tp[:])

            def attn_head(qc, h, extras=None, hook=None):
                nkt = 4 * (qc + 1)
                # two non-diagonal tiles first (no mask-add latency in front
                # of the first exp), then the column-trimmed diagonal tiles,
                # then the rest. qc=0 has only diagonal tiles.
                nd = list(range(nkt - 4))
                order = [(kt, None) for kt in nd[:2]]
                order += [(nkt - 4 + j, j) for j in range(4)]
                order += [(kt, None) for kt in nd[2:]]
                n = len(order)
                o_ps = ops.tile([128, SC], F32, tag="o", name="o_ps")
                l_ps = lps.tile([128, SC], F32, tag="l", name="l_ps")
                state[(qc, h)] = (o_ps, l_ps)
                pend = []
                nfl = [0]

                def flush_one():
                    kt, cl, p = pend.pop(0)
                    first = nfl[0] == 0
                    last = nfl[0] == n - 1
                    nc.tensor.matmul(
                        o_ps[:, cl], lhsT=v_sb[:, kt, :], rhs=p[:, cl],
                        start=first, stop=last, skip_group_check=True,
                    )
                    nc.tensor.matmul(
                        l_ps[:, cl], lhsT=ones_m[:, :], rhs=p[:, cl],
                        start=first, stop=last, skip_group_check=True,
                    )
                    nfl[0] += 1

                for i, (kt, j) in enumerate(order):
                    cl = slice(128 * j, SC) if j is not None else slice(0, SC)
                    qsl = slice(qc * SC + cl.start, (qc + 1) * SC)
                    s_ps = acc.tile([128, SC], F32, tag="acc", name="s_ps")
                    nc.tensor.matmul(
                        s_ps[:, cl], lhsT=kT[:, kt * 128 : (kt + 1) * 128],
                        rhs=qT[:, h, qsl], start=True, stop=True,
                    )
                    if j is not None:
                        nc.vector.tensor_add(s_ps[:, cl], s_ps[:, cl], mask_sb[:, j, cl])
                    p = pexp.tile([128, SC], BF16, tag="p", name="p")
                    nc.scalar.activation(out=p[:, cl], in_=s_ps[:, cl], func=AF.Exp)
                    pend.append((kt, cl, p))
                    # norm of the previous head + outproj filler go after the
                    # diagonal mask-adds (i=2..5) so the DVE queue serves the
                    # adds (which gate exp -> PV) first; the norm fires one
                    # iteration before the outproj filler that reads it.
                    if i == 5 and hook is not None:
                        hook()
                        hook = None
                    if extras is not None and i >= 6:
                        for _ in range(2):
                            ex = next(extras, None)
                            if ex is not None:
                                ex()
                    if i >= 4:
                        flush_one()
                while pend:
                    flush_one()
                return hook  # non-None if the loop was too short to fire it

            def norm(qc, h):
                # l_ps already holds l broadcast on all 128 partitions
                o_ps, l_ps = state.pop((qc, h))
                qsl = slice(qc * SC, (qc + 1) * SC)
                rinv = small.tile([128, SC], F32, tag="ri", name="rinv")
                nc.vector.reciprocal_approx_fast(out=rinv[:], in_=l_ps[:])
                nc.vector.tensor_mul(onrm[:, h, qsl], o_ps[:], rinv[:])

            def op_gen(qc):
                # out-projection of chunk qc: 16 chains of 4 matmuls; the 4
                # e-chunks of one s-tile stage into one SBUF tile so the out
                # DMA writes full 4KB partition lines.
                for sti in range(4):
                    st = qc * 4 + sti
                    stl = slice(st * 128, (st + 1) * 128)
                    holder = {}
                    for ec in range(NEC):
                        esl = slice(ec * SC, (ec + 1) * SC)

                        def chain(stl=stl, esl=esl, st=st, ec=ec, holder=holder):
                            if ec == 0:
                                holder["ob"] = obp.tile(
                                    [128, NEC, SC], BF16, tag="ob", name="ob"
                                )
                            ps = acc.tile([128, SC], F32, tag="acc", name="ops_ps")
                            for hh in range(G):
                                nc.tensor.matmul(
                                    ps[:], lhsT=onrm[:, hh, stl], rhs=wo_sb[:, hh, esl],
                                    start=(hh == 0), stop=(hh == G - 1),
                                )
                            nc.vector.tensor_copy(out=holder["ob"][:, ec, :], in_=ps[:])
                            if ec == NEC - 1:
                                # split over 4 queues so the last tile doesn't
                                # leave a single-queue drain tail
                                for pr in range(0, 128, 32):
                                    nc.sync.dma_start(
                                        out=out[st * 128 + pr : st * 128 + pr + 32, :],
                                        in_=holder["ob"][pr : pr + 32, :, :],
                                    )

                        yield chain

            def attn_block(qc, extras=None):
                for h in range(G):
                    if h >= 1:
                        prev = (qc, h - 1)
                    elif qc >= 1:
                        prev = (qc - 1, 3)
                    else:
                        prev = None
                    hook = (lambda p=prev: norm(*p)) if prev is not None else None
                    left = attn_head(qc, h, extras, hook)
                    if left is not None:
                        left()  # loop too short (qc=0): norm after the loop
                if extras is not None:
                    for ex in extras:  # drain leftovers (shouldn't happen)
                        ex()

            # ---- emission schedule ----
            # A0 sits right after P0: it only needs chunk-0 projections and
            # its 10us of DMA-free PE work covers the chunk-1 x stream.
            proj_block(0)
            attn_block(0)
            proj_block(1)
            proj_block(2)
            # wo queues behind the x chunks it must not delay; it is only
            # needed from the A1-interleaved out-projection onwards.
            wsplit(wo_sb, wo, E, 4)
            # chunk-3 x queues ahead of A1's out-DMAs
            pre3 = prefetch_x(3)
            attn_block(1, extras=op_gen(0))
            proj_block(3, pre=pre3)
            attn_block(2, extras=op_gen(1))
            attn_block(3, extras=op_gen(2))
            norm(3, 3)
            for ch in op_gen(3):
                ch()
    nc.compile()
    return nc


_NC_CACHE = None


def _get_nc():
    global _NC_CACHE
    if _NC_CACHE is None:
        _NC_CACHE = build_nc()
    return _NC_CACHE


def _block_x(xT_bf):
    """[E, S] bf16 -> [sc][p][t][f] partition-major blocks [NSC*128, NET*SC]."""
    return np.ascontiguousarray(
        xT_bf.reshape(NET, 128, NSC, SC).transpose(2, 1, 0, 3).reshape(NSC * 128, NET * SC)
    )


def _block_w(w, width):
    """[ntiles*128, width] -> partition-major [128, ntiles*width]."""
    nt = w.shape[0] // 128
    return np.ascontiguousarray(
        w.reshape(nt, 128, width).transpose(1, 0, 2).reshape(128, nt * width)
    )


def _prep_in_maps(query, key, value, attn_mask, Wq, Wk, Wv, Wo):
    query = np.asarray(query, dtype=np.float32)
    key = np.asarray(key, dtype=np.float32)
    value = np.asarray(value, dtype=np.float32)
    Wq = np.asarray(Wq, dtype=np.float32)
    Wk = np.asarray(Wk, dtype=np.float32)
    Wv = np.asarray(Wv, dtype=np.float32)
    Wo = np.asarray(Wo, dtype=np.float32)
    am = np.asarray(attn_mask)

    xqT = [_block_x(np.ascontiguousarray(query[b].T).astype(NPBF)) for b in range(B)]
    xkT = [_block_x(np.ascontiguousarray(key[b].T).astype(NPBF)) for b in range(B)]
    xvT = [_block_x(np.ascontiguousarray(value[b].T).astype(NPBF)) for b in range(B)]

    # the kernel generates the causal mask on-device; sanity-check the input
    # mask really is causal (it is for this problem by construction)
    assert np.array_equal(
        np.asarray(am[0, 0, :4, :4]), np.tril(np.ones((4, 4), am.dtype))
    )

    in_maps = []
    for b in range(B):
        for g in range(HKV):
            wq_g = np.ascontiguousarray(Wq[g * GD : (g + 1) * GD, :].T).astype(NPBF)
            wk_g = np.ascontiguousarray(Wk[g * D : (g + 1) * D, :].T).astype(NPBF)
            wv_g = np.ascontiguousarray(Wv[g * D : (g + 1) * D, :].T).astype(NPBF)
            wo_g = np.ascontiguousarray(Wo[:, g * GD : (g + 1) * GD].T).astype(NPBF)
            in_maps.append(
                {
                    "xq": xqT[b],
                    "xk": xkT[b],
                    "xv": xvT[b],
                    "wq": _block_w(wq_g, GD),
                    "wk": _block_w(wk_g, D),
                    "wv": _block_w(wv_g, D),
                    "wo": _block_w(wo_g, E),
                }
            )
    return in_maps


def _run(inputs, trace=False, **kw):
    nc = _get_nc()
    in_maps = _prep_in_maps(**inputs)
    res = run_bass_kernel_spmd(nc, in_maps, list(range(NCORES)), trace=trace, **kw)
    outs = [np.asarray(r["out"]) for r in res.results]
    full = np.empty((B, S, E), dtype=np.float32)
    for b in range(B):
        acc = outs[b * HKV].astype(np.float32)
        for g in range(1, HKV):
            acc = acc + outs[b * HKV + g].astype(np.float32)
        full[b] = acc
    return full, res


def kernel(**inputs):
    full, _ = _run(inputs, trace=False)
    return full


# revision 36
# speedup vs baseline: 1.7727x; 1.0279x over previous
"""GQA kernel for Trainium2, 8 NeuronCores.

Sharding: core c = b*4 + g handles batch b, kv-head g (4 query heads).
Host sums the 4 partial out-projections per batch.

Design notes (v3):
- All matmuls bf16 (fp32 PSUM). PE cost is free-dim rows x clock, and the
  clock p-state ramps to 2.4GHz only after ~3us of *continuous* PE busy,
  so the emission order never lets the PE idle: projection, attention and
  out-projection chunks are interleaved, and the out-projection of chunk
  qc is folded into the first head's kt-loop of chunk qc+1.
- DMA descriptors are generated per SBUF partition line, so all DRAM
  layouts are partition-major: x is host-blocked [sc][p][t][f] (16KB
  contiguous per partition per chunk), weights [p][t][..], and the output
  is written per s-tile with 4KB lines. Chunk loads are split along t so
  8 queues stream one chunk in parallel and the first matmul of a chain
  only waits for its own t-range.
- Attention kt order: two non-diagonal tiles first (their exp has no
  DVE mask-add on the critical path, hiding the QK->exp->PV latency at
  each head-loop start), then the 4 diagonal tiles (column-trimmed: tile
  j only touches q columns >= 128j, the rest is fully masked), then the
  remaining tiles. The first flushed PV/l matmul covers all 512 columns
  so PSUM start=True initializes the full accumulator.
- Softmax normalization without a DRAM round trip: l row-sums accumulate
  via ones-matmuls per kt; then l -> bf16 copy (ACT), K=1 matmul
  broadcast across partitions (PE), reciprocal_approx_fast (DVE, ~0.7us
  vs 3.3us for reciprocal), multiply into onrm.
"""

import sys

import numpy as np

for _p in ("/opt/trn_rl_repo",):
    if _p not in sys.path:
        sys.path.insert(0, _p)

import ml_dtypes

import concourse.mybir as mybir
from concourse import bacc
from concourse.bass_utils import run_bass_kernel_spmd
from concourse.masks import make_identity
from concourse.tile import TileContext

B, S, E = 2, 2048, 2048
H, HKV = 16, 4
D = E // H  # 128
G = H // HKV  # 4 query heads per kv head
GD = G * D  # 512
NCORES = B * HKV  # 8
SC = 512  # s/q chunk width (free dim of matmuls)
NSC = S // SC  # 4
NET = E // 128  # 16 e-tiles (contraction)
NKT = S // 128  # 16 k-tiles
NEC = E // SC  # 4 e-chunks for output
SCALE = 1.0 / float(np.sqrt(D))

F32 = mybir.dt.float32
BF16 = mybir.dt.bfloat16
AF = mybir.ActivationFunctionType
NPBF = np.dtype(ml_dtypes.bfloat16)


def build_nc():
    nc = bacc.Bacc()
    # x inputs: [sc][p][t][f] partition-major blocks -> [NSC*128, NET*SC]
    xq = nc.declare_dram_parameter("xq", [NSC * 128, NET * SC], BF16, isOutput=False)
    xk = nc.declare_dram_parameter("xk", [NSC * 128, NET * SC], BF16, isOutput=False)
    xv = nc.declare_dram_parameter("xv", [NSC * 128, NET * SC], BF16, isOutput=False)
    # weights partition-major: [p][t][..]
    wq = nc.declare_dram_parameter("wq", [128, NET * GD], BF16, isOutput=False)
    wk = nc.declare_dram_parameter("wk", [128, NET * D], BF16, isOutput=False)
    wv = nc.declare_dram_parameter("wv", [128, NET * D], BF16, isOutput=False)
    wo = nc.declare_dram_parameter("wo", [128, G * E], BF16, isOutput=False)
    # natural [S, E]: written per s-tile as [128, E] full-width rows
    out = nc.declare_dram_parameter("out", [S, E], BF16, isOutput=True)

    with TileContext(nc) as tc:
        with (
            tc.tile_pool(name="singles", bufs=1) as singles,
            tc.tile_pool(name="xc", bufs=1) as xcp,
            tc.tile_pool(name="pexp", bufs=7) as pexp,
            tc.tile_pool(name="small", bufs=2) as small,
            tc.tile_pool(name="ob", bufs=2) as obp,
            tc.tile_pool(name="acc", bufs=4, space="PSUM") as acc,
            tc.tile_pool(name="ops", bufs=2, space="PSUM") as ops,
            tc.tile_pool(name="lps", bufs=2, space="PSUM") as lps,
        ):
            # ---- constants / weights resident in SBUF ----
            wq_sb = singles.tile([128, NET, GD], BF16)  # 16KB/p
            wk_sb = singles.tile([128, NET, D], BF16)  # 4KB/p
            wv_sb = singles.tile([128, NET, D], BF16)  # 4KB/p
            wo_sb = singles.tile([128, G, E], BF16)  # 16KB/p
            mask_sb = singles.tile([128, 4, SC], F32)  # 8KB/p
            ident_f = singles.tile([128, 128], F32)
            ident = singles.tile([128, 128], BF16)
            ones_f = singles.tile([128, 128], F32)
            # l-matmul lhsT: [128,128] ones -> row-sum REPLICATED on all 128
            # output partitions (same row cost as a 1-wide lhsT, but the
            # weight load pipelines and no separate broadcast is needed)
            ones_m = singles.tile([128, 128], BF16)
            qT = singles.tile([128, G, S], BF16)  # 16KB/p
            kT = singles.tile([128, S], BF16)  # 4KB/p
            v_sb = singles.tile([128, NKT, D], BF16)  # 4KB/p
            onrm = singles.tile([128, G, S], BF16)  # 16KB/p

            make_identity(nc, ident_f)
            nc.scalar.activation(out=ident[:], in_=ident_f[:], func=AF.Copy)
            nc.vector.memset(ones_f, 1.0)
            nc.scalar.activation(out=ones_m[:], in_=ones_f[:], func=AF.Copy)
            # causal mask built on-device (gpsimd, idle at t=0): tile j is the
            # additive mask for k-rows 128j..128j+127 vs q-columns of a chunk:
            # keep 0 where q - p - 128j >= 0, else -1e9.
            nc.gpsimd.memset(mask_sb, 0.0)
            for j in range(4):
                nc.gpsimd.affine_select(
                    out=mask_sb[:, j, :], in_=mask_sb[:, j, :],
                    compare_op=mybir.AluOpType.is_ge, fill=-1e9,
                    base=-128 * j, channel_multiplier=-1, pattern=[[1, SC]],
                )

            def wsplit(dst, dram, width, nsplit):
                """Load [128, n, width] SBUF tile from partition-major DRAM."""
                ntiles = dst.shape[1]
                step = ntiles // nsplit
                for i in range(nsplit):
                    t0 = i * step
                    nc.sync.dma_start(
                        out=dst[:, t0 : t0 + step, :],
                        in_=dram[:, t0 * width : (t0 + step) * width],
                    )

            # Q weights first: Q-proj of chunk 0 starts as early as possible
            wsplit(wq_sb, wq, GD, 8)

            state = {}  # (qc, h) -> (o_ps, l_ps)

            def x_chunk(dram, sc, tag):
                """Stream one [128, NET, SC] x chunk, split along t."""
                # xq double-buffers: its ring-1 WAR (next chunk's DMA vs the
                # 4 head chains of the previous Q-proj) otherwise serializes
                # the stream late in the run.
                xsb = xcp.tile(
                    [128, NET, SC], BF16, tag=tag, name=tag,
                    bufs=2 if tag == "xq" else 1,
                )
                rows = slice(sc * 128, (sc + 1) * 128)
                # 4KB descriptor lines in steady state; finer first chunk so
                # the first projection chain starts earlier
                nsplit = 8 if sc == 0 else 4
                step = NET // nsplit
                for i in range(nsplit):
                    t0 = i * step
                    nc.sync.dma_start(
                        out=xsb[:, t0 : t0 + step, :],
                        in_=dram[rows, t0 * SC : (t0 + step) * SC],
                    )
                return xsb

            def prefetch_x(sc):
                return (
                    x_chunk(xq, sc, "xq"),
                    x_chunk(xk, sc, "xk"),
                    x_chunk(xv, sc, "xv"),
                )

            def proj_block(sc, pre=None):
                # Q first: its 13.6us of chains pace the chunk's K/V x stream
                ssl = slice(sc * SC, (sc + 1) * SC)
                xsb = pre[0] if pre else x_chunk(xq, sc, "xq")
                for h in range(G):
                    ps = acc.tile([128, SC], F32, tag="acc", name="qps")
                    for t in range(NET):
                        nc.tensor.matmul(
                            ps[:], lhsT=wq_sb[:, t, h * D : (h + 1) * D],
                            rhs=xsb[:, t, :], start=(t == 0), stop=(t == NET - 1),
                        )
                    # fold softmax scale into Q
                    nc.scalar.activation(
                        out=qT[:, h, ssl], in_=ps[:], func=AF.Copy, scale=SCALE
                    )
                    if sc == 0 and h == 0:
                        wsplit(wk_sb, wk, D, 4)
                        wsplit(wv_sb, wv, D, 4)
                # K projection
                xsb = pre[1] if pre else x_chunk(xk, sc, "xk")
                ps = acc.tile([128, SC], F32, tag="acc", name="kps")
                for t in range(NET):
                    nc.tensor.matmul(
                        ps[:], lhsT=wk_sb[:, t, :], rhs=xsb[:, t, :],
                        start=(t == 0), stop=(t == NET - 1),
                    )
                nc.vector.tensor_copy(out=kT[:, ssl], in_=ps[:])
                # V projection, then transpose to [s, d] tiles
                xsb = pre[2] if pre else x_chunk(xv, sc, "xv")
                ps = acc.tile([128, SC], F32, tag="acc", name="vps")
                for t in range(NET):
                    nc.tensor.matmul(
                        ps[:], lhsT=wv_sb[:, t, :], rhs=xsb[:, t, :],
                        start=(t == 0), stop=(t == NET - 1),
                    )
                vt = small.tile([128, SC], BF16, tag="vt", name="vt")
                nc.scalar.activation(out=vt[:], in_=ps[:], func=AF.Copy)
                for i in range(SC // 128):
                    tp = acc.tile([128, 128], BF16, tag="acc", name="tp")
                    nc.tensor.transpose(tp[:], vt[:, i * 128 : (i + 1) * 128], ident[:])
                    nc.vector.tensor_copy(out=v_sb[:, sc * 4 + i, :], in_=tp[:])

            def norm(qc, h):
                # l_ps already holds l broadcast on all 128 partitions
                o_ps, l_ps = state.pop((qc, h))
                qsl = slice(qc * SC, (qc + 1) * SC)
                rinv = small.tile([128, SC], F32, tag="ri", name="rinv")
                nc.vector.reciprocal_approx_fast(out=rinv[:], in_=l_ps[:])
                nc.vector.tensor_mul(onrm[:, h, qsl], o_ps[:], rinv[:])

            def attn_block(qc, extras=None):
                nkt = 4 * (qc + 1)
                nd = list(range(nkt - 4))
                # two non-diagonal tiles first (no mask-add latency in front
                # of the first exp), then the column-trimmed diagonal tiles,
                # then the rest. qc=0 has only diagonal tiles.
                order = [(kt, None) for kt in nd[:2]]
                order += [(nkt - 4 + j, j) for j in range(4)]
                order += [(kt, None) for kt in nd[2:]]
                n = len(order)
                # the PV/l flush queue is shared across the 4 heads: a head's
                # tail flushes become PE filler for the next head's early
                # iterations, keeping a 4-deep exp->PV cushion everywhere
                pend = []

                def flush_one():
                    o_ps, l_ps, kt, cl, p, first, last = pend.pop(0)
                    nc.tensor.matmul(
                        o_ps[:, cl], lhsT=v_sb[:, kt, :], rhs=p[:, cl],
                        start=first, stop=last, skip_group_check=True,
                    )
                    nc.tensor.matmul(
                        l_ps[:, cl], lhsT=ones_m[:, :], rhs=p[:, cl],
                        start=first, stop=last, skip_group_check=True,
                    )

                for h in range(G):
                    if h >= 1:
                        prev = (qc, h - 1)
                    elif qc >= 1:
                        prev = (qc - 1, 3)
                    else:
                        prev = None
                    hook = (lambda p=prev: norm(*p)) if prev is not None else None
                    o_ps = ops.tile([128, SC], F32, tag="o", name="o_ps")
                    l_ps = lps.tile([128, SC], F32, tag="l", name="l_ps")
                    state[(qc, h)] = (o_ps, l_ps)
                    for i, (kt, j) in enumerate(order):
                        cl = slice(128 * j, SC) if j is not None else slice(0, SC)
                        qsl = slice(qc * SC + cl.start, (qc + 1) * SC)
                        s_ps = acc.tile([128, SC], F32, tag="acc", name="s_ps")
                        nc.tensor.matmul(
                            s_ps[:, cl], lhsT=kT[:, kt * 128 : (kt + 1) * 128],
                            rhs=qT[:, h, qsl], start=True, stop=True,
                        )
                        if j is not None:
                            nc.vector.tensor_add(
                                s_ps[:, cl], s_ps[:, cl], mask_sb[:, j, cl]
                            )
                        p = pexp.tile([128, SC], BF16, tag="p", name="p")
                        nc.scalar.activation(out=p[:, cl], in_=s_ps[:, cl], func=AF.Exp)
                        pend.append((o_ps, l_ps, kt, cl, p, i == 0, i == n - 1))
                        # norm of the previous head + outproj filler go after
                        # the diagonal mask-adds so the DVE queue serves the
                        # adds (which gate exp -> PV) first
                        if i == 5 and hook is not None:
                            hook()
                            hook = None
                        if extras is not None and i >= 6:
                            for _ in range(2):
                                ex = next(extras, None)
                                if ex is not None:
                                    ex()
                        while len(pend) > 4:
                            flush_one()
                    if hook is not None:
                        hook()  # loop too short (qc=0): norm after the loop
                while pend:
                    flush_one()
                if extras is not None:
                    for ex in extras:  # drain leftovers (shouldn't happen)
                        ex()

            def op_gen(qc):
                # out-projection of chunk qc: 16 chains of 4 matmuls; the 4
                # e-chunks of one s-tile stage into one SBUF tile so the out
                # DMA writes full 4KB partition lines.
                for sti in range(4):
                    st = qc * 4 + sti
                    stl = slice(st * 128, (st + 1) * 128)
                    holder = {}
                    for ec in range(NEC):
                        esl = slice(ec * SC, (ec + 1) * SC)

                        def chain(stl=stl, esl=esl, st=st, ec=ec, holder=holder):
                            if ec == 0:
                                holder["ob"] = obp.tile(
                                    [128, NEC, SC], BF16, tag="ob", name="ob"
                                )
                            ps = acc.tile([128, SC], F32, tag="acc", name="ops_ps")
                            for hh in range(G):
                                nc.tensor.matmul(
                                    ps[:], lhsT=onrm[:, hh, stl], rhs=wo_sb[:, hh, esl],
                                    start=(hh == 0), stop=(hh == G - 1),
                                )
                            nc.vector.tensor_copy(out=holder["ob"][:, ec, :], in_=ps[:])
                            if ec == NEC - 1:
                                # split over 4 queues so the last tile doesn't
                                # leave a single-queue drain tail
                                for pr in range(0, 128, 32):
                                    nc.sync.dma_start(
                                        out=out[st * 128 + pr : st * 128 + pr + 32, :],
                                        in_=holder["ob"][pr : pr + 32, :, :],
                                    )

                        yield chain

            # ---- emission schedule ----
            # A0 sits right after P0: it only needs chunk-0 projections and
            # its 10us of DMA-free PE work covers the chunk-1 x stream.
            proj_block(0)
            attn_block(0)
            proj_block(1)
            proj_block(2)
            # wo queues behind the x chunks it must not delay; it is only
            # needed from the A1-interleaved out-projection onwards.
            wsplit(wo_sb, wo, E, 4)
            # chunk-3 x queues ahead of A1's out-DMAs
            pre3 = prefetch_x(3)
            attn_block(1, extras=op_gen(0))
            proj_block(3, pre=pre3)
            attn_block(2, extras=op_gen(1))
            attn_block(3, extras=op_gen(2))
            norm(3, 3)
            for ch in op_gen(3):
                ch()
    nc.compile()
    return nc


_NC_CACHE = None


def _get_nc():
    global _NC_CACHE
    if _NC_CACHE is None:
        _NC_CACHE = build_nc()
    return _NC_CACHE


def _block_x(xT_bf):
    """[E, S] bf16 -> [sc][p][t][f] partition-major blocks [NSC*128, NET*SC]."""
    return np.ascontiguousarray(
        xT_bf.reshape(NET, 128, NSC, SC).transpose(2, 1, 0, 3).reshape(NSC * 128, NET * SC)
    )


def _block_w(w, width):
    """[ntiles*128, width] -> partition-major [128, ntiles*width]."""
    nt = w.shape[0] // 128
    return np.ascontiguousarray(
        w.reshape(nt, 128, width).transpose(1, 0, 2).reshape(128, nt * width)
    )


def _prep_in_maps(query, key, value, attn_mask, Wq, Wk, Wv, Wo):
    query = np.asarray(query, dtype=np.float32)
    key = np.asarray(key, dtype=np.float32)
    value = np.asarray(value, dtype=np.float32)
    Wq = np.asarray(Wq, dtype=np.float32)
    Wk = np.asarray(Wk, dtype=np.float32)
    Wv = np.asarray(Wv, dtype=np.float32)
    Wo = np.asarray(Wo, dtype=np.float32)
    am = np.asarray(attn_mask)

    xqT = [_block_x(np.ascontiguousarray(query[b].T).astype(NPBF)) for b in range(B)]
    xkT = [_block_x(np.ascontiguousarray(key[b].T).astype(NPBF)) for b in range(B)]
    xvT = [_block_x(np.ascontiguousarray(value[b].T).astype(NPBF)) for b in range(B)]

    # the kernel generates the causal mask on-device; sanity-check the input
    # mask really is causal (it is for this problem by construction)
    assert np.array_equal(
        np.asarray(am[0, 0, :4, :4]), np.tril(np.ones((4, 4), am.dtype))
    )

    in_maps = []
    for b in range(B):
        for g in range(HKV):
            wq_g = np.ascontiguousarray(Wq[g * GD : (g + 1) * GD, :].T).astype(NPBF)
            wk_g = np.ascontiguousarray(Wk[g * D : (g + 1) * D, :].T).astype(NPBF)
            wv_g = np.ascontiguousarray(Wv[g * D : (g + 1) * D, :].T).astype(NPBF)
            wo_g = np.ascontiguousarray(Wo[:, g * GD : (g + 1) * GD].T).astype(NPBF)
            in_maps.append(
                {
                    "xq": xqT[b],
                    "xk": xkT[b],
                    "xv": xvT[b],
                    "wq": _block_w(wq_g, GD),
                    "wk": _block_w(wk_g, D),
                    "wv": _block_w(wv_g, D),
                    "wo": _block_w(wo_g, E),
                }
            )
    return in_maps


def _run(inputs, trace=False, **kw):
    nc = _get_nc()
    in_maps = _prep_in_maps(**inputs)
    res = run_bass_kernel_spmd(nc, in_maps, list(range(NCORES)), trace=trace, **kw)
    outs = [np.asarray(r["out"]) for r in res.results]
    full = np.empty((B, S, E), dtype=np.float32)
    for b in range(B):
        acc = outs[b * HKV].astype(np.float32)
        for g in range(1, HKV):
            acc = acc + outs[b * HKV + g].astype(np.float32)
        full[b] = acc
    return full, res


def kernel(**inputs):
    full, _ = _run(inputs, trace=False)
    return full


# revision 43
# speedup vs baseline: 1.7774x; 1.0027x over previous
"""GQA kernel for Trainium2, 8 NeuronCores.

Sharding: core c = b*4 + g handles batch b, kv-head g (4 query heads).
Host sums the 4 partial out-projections per batch.

Design notes (v3):
- All matmuls bf16 (fp32 PSUM). PE cost is free-dim rows x clock, and the
  clock p-state ramps to 2.4GHz only after ~3us of *continuous* PE busy,
  so the emission order never lets the PE idle: projection, attention and
  out-projection chunks are interleaved, and the out-projection of chunk
  qc is folded into the first head's kt-loop of chunk qc+1.
- DMA descriptors are generated per SBUF partition line, so all DRAM
  layouts are partition-major: x is host-blocked [sc][p][t][f] (16KB
  contiguous per partition per chunk), weights [p][t][..], and the output
  is written per s-tile with 4KB lines. Chunk loads are split along t so
  8 queues stream one chunk in parallel and the first matmul of a chain
  only waits for its own t-range.
- Attention kt order: two non-diagonal tiles first (their exp has no
  DVE mask-add on the critical path, hiding the QK->exp->PV latency at
  each head-loop start), then the 4 diagonal tiles (column-trimmed: tile
  j only touches q columns >= 128j, the rest is fully masked), then the
  remaining tiles. The first flushed PV/l matmul covers all 512 columns
  so PSUM start=True initializes the full accumulator.
- Softmax normalization without a DRAM round trip: l row-sums accumulate
  via ones-matmuls per kt; then l -> bf16 copy (ACT), K=1 matmul
  broadcast across partitions (PE), reciprocal_approx_fast (DVE, ~0.7us
  vs 3.3us for reciprocal), multiply into onrm.
"""

import sys

import numpy as np

for _p in ("/opt/trn_rl_repo",):
    if _p not in sys.path:
        sys.path.insert(0, _p)

import ml_dtypes

import concourse.mybir as mybir
from concourse import bacc
from concourse.bass_utils import run_bass_kernel_spmd
from concourse.masks import make_identity
from concourse.tile import TileContext

B, S, E = 2, 2048, 2048
H, HKV = 16, 4
D = E // H  # 128
G = H // HKV  # 4 query heads per kv head
GD = G * D  # 512
NCORES = B * HKV  # 8
SC = 512  # s/q chunk width (free dim of matmuls)
NSC = S // SC  # 4
NET = E // 128  # 16 e-tiles (contraction)
NKT = S // 128  # 16 k-tiles
NEC = E // SC  # 4 e-chunks for output
SCALE = 1.0 / float(np.sqrt(D))

F32 = mybir.dt.float32
BF16 = mybir.dt.bfloat16
AF = mybir.ActivationFunctionType
NPBF = np.dtype(ml_dtypes.bfloat16)


def build_nc():
    nc = bacc.Bacc()
    # x inputs: [sc][p][t][f] partition-major blocks -> [NSC*128, NET*SC]
    xq = nc.declare_dram_parameter("xq", [NSC * 128, NET * SC], BF16, isOutput=False)
    xk = nc.declare_dram_parameter("xk", [NSC * 128, NET * SC], BF16, isOutput=False)
    xv = nc.declare_dram_parameter("xv", [NSC * 128, NET * SC], BF16, isOutput=False)
    # weights partition-major: [p][t][..]
    wq = nc.declare_dram_parameter("wq", [128, NET * GD], BF16, isOutput=False)
    wk = nc.declare_dram_parameter("wk", [128, NET * D], BF16, isOutput=False)
    wv = nc.declare_dram_parameter("wv", [128, NET * D], BF16, isOutput=False)
    wo = nc.declare_dram_parameter("wo", [128, G * E], BF16, isOutput=False)
    # natural [S, E]: written per s-tile as [128, E] full-width rows
    out = nc.declare_dram_parameter("out", [S, E], BF16, isOutput=True)

    with TileContext(nc) as tc:
        with (
            tc.tile_pool(name="singles", bufs=1) as singles,
            tc.tile_pool(name="xc", bufs=1) as xcp,
            tc.tile_pool(name="pexp", bufs=7) as pexp,
            tc.tile_pool(name="small", bufs=2) as small,
            tc.tile_pool(name="ob", bufs=2) as obp,
            tc.tile_pool(name="acc", bufs=4, space="PSUM") as acc,
            tc.tile_pool(name="ops", bufs=2, space="PSUM") as ops,
            tc.tile_pool(name="lps", bufs=2, space="PSUM") as lps,
        ):
            # ---- constants / weights resident in SBUF ----
            wq_sb = singles.tile([128, NET, GD], BF16)  # 16KB/p
            wk_sb = singles.tile([128, NET, D], BF16)  # 4KB/p
            wv_sb = singles.tile([128, NET, D], BF16)  # 4KB/p
            wo_sb = singles.tile([128, G, E], BF16)  # 16KB/p
            mask_sb = singles.tile([128, 4, SC], F32)  # 8KB/p
            ident_f = singles.tile([128, 128], F32)
            ident = singles.tile([128, 128], BF16)
            ones_f = singles.tile([128, 128], F32)
            # l-matmul lhsT: [128,128] ones -> row-sum REPLICATED on all 128
            # output partitions (same row cost as a 1-wide lhsT, but the
            # weight load pipelines and no separate broadcast is needed)
            ones_m = singles.tile([128, 128], BF16)
            qT = singles.tile([128, G, S], BF16)  # 16KB/p
            kT = singles.tile([128, S], BF16)  # 4KB/p
            v_sb = singles.tile([128, NKT, D], BF16)  # 4KB/p
            onrm = singles.tile([128, G, S], BF16)  # 16KB/p

            make_identity(nc, ident_f)
            nc.scalar.activation(out=ident[:], in_=ident_f[:], func=AF.Copy)
            nc.vector.memset(ones_f, 1.0)
            nc.scalar.activation(out=ones_m[:], in_=ones_f[:], func=AF.Copy)
            # causal mask built on-device (gpsimd, idle at t=0): tile j is the
            # additive mask for k-rows 128j..128j+127 vs q-columns of a chunk:
            # keep 0 where q - p - 128j >= 0, else -1e9.
            nc.gpsimd.memset(mask_sb, 0.0)
            for j in range(4):
                nc.gpsimd.affine_select(
                    out=mask_sb[:, j, :], in_=mask_sb[:, j, :],
                    compare_op=mybir.AluOpType.is_ge, fill=-1e9,
                    base=-128 * j, channel_multiplier=-1, pattern=[[1, SC]],
                )

            def wsplit(dst, dram, width, nsplit):
                """Load [128, n, width] SBUF tile from partition-major DRAM."""
                ntiles = dst.shape[1]
                step = ntiles // nsplit
                for i in range(nsplit):
                    t0 = i * step
                    nc.sync.dma_start(
                        out=dst[:, t0 : t0 + step, :],
                        in_=dram[:, t0 * width : (t0 + step) * width],
                    )

            # PE p-state warmup: the clock ramps to 2.4GHz only after ~3us of
            # continuous busy, and the first x split lands at ~14us. These
            # dummy matmuls bridge the idle window so real work starts at
            # full clock.
            warm_ps = acc.tile([128, SC], F32, tag="acc", name="warm_ps")
            for _ in range(80):
                nc.tensor.matmul(
                    warm_ps[:, 0:128], lhsT=ident[:], rhs=ident[:],
                    start=True, stop=True, skip_group_check=True,
                )

            state = {}  # (qc, h) -> (o_ps, l_ps)

            def x_chunk(dram, sc, tag):
                """Stream one [128, NET, SC] x chunk, split along t."""
                # xq double-buffers: its ring-1 WAR (next chunk's DMA vs the
                # 4 head chains of the previous Q-proj) otherwise serializes
                # the stream late in the run.
                xsb = xcp.tile(
                    [128, NET, SC], BF16, tag=tag, name=tag,
                    bufs=2 if tag == "xq" else 1,
                )
                rows = slice(sc * 128, (sc + 1) * 128)
                # 4KB descriptor lines in steady state; finer first chunk so
                # the first projection chain starts earlier
                nsplit = 8 if sc == 0 else 4
                step = NET // nsplit
                for i in range(nsplit):
                    t0 = i * step
                    nc.sync.dma_start(
                        out=xsb[:, t0 : t0 + step, :],
                        in_=dram[rows, t0 * SC : (t0 + step) * SC],
                    )
                return xsb

            def prefetch_x(sc):
                return (
                    x_chunk(xq, sc, "xq"),
                    x_chunk(xk, sc, "xk"),
                    x_chunk(xv, sc, "xv"),
                )

            def proj_block(sc, pre=None):
                # Q first: its 13.6us of chains pace the chunk's K/V x stream
                ssl = slice(sc * SC, (sc + 1) * SC)
                xsb = pre[0] if pre and pre[0] is not None else x_chunk(xq, sc, "xq")
                for h in range(G):
                    ps = acc.tile([128, SC], F32, tag="acc", name="qps")
                    for t in range(NET):
                        nc.tensor.matmul(
                            ps[:], lhsT=wq_sb[:, t, h * D : (h + 1) * D],
                            rhs=xsb[:, t, :], start=(t == 0), stop=(t == NET - 1),
                        )
                    # fold softmax scale into Q
                    nc.scalar.activation(
                        out=qT[:, h, ssl], in_=ps[:], func=AF.Copy, scale=SCALE
                    )
                    if sc == 0 and h == 0:
                        wsplit(wk_sb, wk, D, 4)
                        wsplit(wv_sb, wv, D, 4)
                # K projection
                xsb = pre[1] if pre and pre[1] is not None else x_chunk(xk, sc, "xk")
                ps = acc.tile([128, SC], F32, tag="acc", name="kps")
                for t in range(NET):
                    nc.tensor.matmul(
                        ps[:], lhsT=wk_sb[:, t, :], rhs=xsb[:, t, :],
                        start=(t == 0), stop=(t == NET - 1),
                    )
                nc.vector.tensor_copy(out=kT[:, ssl], in_=ps[:])
                # V projection, then transpose to [s, d] tiles
                xsb = pre[2] if pre and pre[2] is not None else x_chunk(xv, sc, "xv")
                ps = acc.tile([128, SC], F32, tag="acc", name="vps")
                for t in range(NET):
                    nc.tensor.matmul(
                        ps[:], lhsT=wv_sb[:, t, :], rhs=xsb[:, t, :],
                        start=(t == 0), stop=(t == NET - 1),
                    )
                vt = small.tile([128, SC], BF16, tag="vt", name="vt")
                nc.scalar.activation(out=vt[:], in_=ps[:], func=AF.Copy)
                for i in range(SC // 128):
                    tp = acc.tile([128, 128], BF16, tag="acc", name="tp")
                    nc.tensor.transpose(tp[:], vt[:, i * 128 : (i + 1) * 128], ident[:])
                    nc.vector.tensor_copy(out=v_sb[:, sc * 4 + i, :], in_=tp[:])

            def norm(qc, h):
                # l_ps already holds l broadcast on all 128 partitions
                o_ps, l_ps = state.pop((qc, h))
                qsl = slice(qc * SC, (qc + 1) * SC)
                rinv = small.tile([128, SC], F32, tag="ri", name="rinv")
                nc.vector.reciprocal_approx_fast(out=rinv[:], in_=l_ps[:])
                nc.vector.tensor_mul(onrm[:, h, qsl], o_ps[:], rinv[:])

            def attn_block(qc, extras=None):
                nkt = 4 * (qc + 1)
                nd = list(range(nkt - 4))
                # two non-diagonal tiles first (no mask-add latency in front
                # of the first exp), then the column-trimmed diagonal tiles,
                # then the rest. qc=0 has only diagonal tiles.
                order = [(kt, None) for kt in nd[:2]]
                order += [(nkt - 4 + j, j) for j in range(4)]
                order += [(kt, None) for kt in nd[2:]]
                n = len(order)
                # the PV/l flush queue is shared across the 4 heads: a head's
                # tail flushes become PE filler for the next head's early
                # iterations, keeping a 4-deep exp->PV cushion everywhere
                pend = []

                def flush_one():
                    o_ps, l_ps, kt, cl, p, first, last = pend.pop(0)
                    nc.tensor.matmul(
                        o_ps[:, cl], lhsT=v_sb[:, kt, :], rhs=p[:, cl],
                        start=first, stop=last, skip_group_check=True,
                    )
                    nc.tensor.matmul(
                        l_ps[:, cl], lhsT=ones_m[:, :], rhs=p[:, cl],
                        start=first, stop=last, skip_group_check=True,
                    )

                for h in range(G):
                    if h >= 1:
                        prev = (qc, h - 1)
                    elif qc >= 1:
                        prev = (qc - 1, 3)
                    else:
                        prev = None
                    hook = (lambda p=prev: norm(*p)) if prev is not None else None
                    quota = 4  # outproj filler rationed so every head's
                    # boundary gets PE cover, not just the first one's
                    o_ps = ops.tile([128, SC], F32, tag="o", name="o_ps")
                    l_ps = lps.tile([128, SC], F32, tag="l", name="l_ps")
                    state[(qc, h)] = (o_ps, l_ps)
                    for i, (kt, j) in enumerate(order):
                        cl = slice(128 * j, SC) if j is not None else slice(0, SC)
                        qsl = slice(qc * SC + cl.start, (qc + 1) * SC)
                        s_ps = acc.tile([128, SC], F32, tag="acc", name="s_ps")
                        nc.tensor.matmul(
                            s_ps[:, cl], lhsT=kT[:, kt * 128 : (kt + 1) * 128],
                            rhs=qT[:, h, qsl], start=True, stop=True,
                        )
                        if j is not None:
                            nc.vector.tensor_add(
                                s_ps[:, cl], s_ps[:, cl], mask_sb[:, j, cl]
                            )
                        p = pexp.tile([128, SC], BF16, tag="p", name="p")
                        nc.scalar.activation(out=p[:, cl], in_=s_ps[:, cl], func=AF.Exp)
                        pend.append((o_ps, l_ps, kt, cl, p, i == 0, i == n - 1))
                        # norm of the previous head + outproj filler go after
                        # the diagonal mask-adds so the DVE queue serves the
                        # adds (which gate exp -> PV) first
                        if i == 5 and hook is not None:
                            hook()
                            hook = None
                        if extras is not None and i >= 6 and quota > 0:
                            for _ in range(2):
                                ex = next(extras, None)
                                if ex is not None:
                                    ex()
                                quota -= 1
                        while len(pend) > 4:
                            flush_one()
                    if hook is not None:
                        hook()  # loop too short (qc=0): norm after the loop
                while pend:
                    flush_one()
                if extras is not None:
                    for ex in extras:  # drain leftovers (shouldn't happen)
                        ex()

            def op_gen(qc):
                # out-projection of chunk qc: 16 chains of 4 matmuls; the 4
                # e-chunks of one s-tile stage into one SBUF tile so the out
                # DMA writes full 4KB partition lines.
                for sti in range(4):
                    st = qc * 4 + sti
                    stl = slice(st * 128, (st + 1) * 128)
                    holder = {}
                    for ec in range(NEC):
                        esl = slice(ec * SC, (ec + 1) * SC)

                        def chain(stl=stl, esl=esl, st=st, ec=ec, holder=holder):
                            if ec == 0:
                                holder["ob"] = obp.tile(
                                    [128, NEC, SC], BF16, tag="ob", name="ob"
                                )
                            ps = acc.tile([128, SC], F32, tag="acc", name="ops_ps")
                            for hh in range(G):
                                nc.tensor.matmul(
                                    ps[:], lhsT=onrm[:, hh, stl], rhs=wo_sb[:, hh, esl],
                                    start=(hh == 0), stop=(hh == G - 1),
                                )
                            nc.vector.tensor_copy(out=holder["ob"][:, ec, :], in_=ps[:])
                            if ec == NEC - 1:
                                # split over 4 queues so the last tile doesn't
                                # leave a single-queue drain tail
                                for pr in range(0, 128, 32):
                                    nc.sync.dma_start(
                                        out=out[st * 128 + pr : st * 128 + pr + 32, :],
                                        in_=holder["ob"][pr : pr + 32, :, :],
                                    )

                        yield chain

            # ---- emission schedule ----
            # interleave the first xq chunk's splits with wq's: DMA triggers
            # serialize on the sync engine (~0.6us each), so the first Q-proj
            # matmul's two dependencies must be the first two triggered.
            xq0 = xcp.tile([128, NET, SC], BF16, tag="xq", name="xq0", bufs=2)
            for i in range(8):
                t0 = i * 2
                nc.sync.dma_start(
                    out=xq0[:, t0 : t0 + 2, :], in_=xq[0:128, t0 * SC : (t0 + 2) * SC]
                )
                nc.sync.dma_start(
                    out=wq_sb[:, t0 : t0 + 2, :], in_=wq[:, t0 * GD : (t0 + 2) * GD]
                )
            # A0 sits right after P0: it only needs chunk-0 projections and
            # its 10us of DMA-free PE work covers the chunk-1 x stream.
            proj_block(0, pre=(xq0, None, None))
            attn_block(0)
            proj_block(1)
            proj_block(2)
            # wo queues behind the x chunks it must not delay; it is only
            # needed from the A1-interleaved out-projection onwards.
            wsplit(wo_sb, wo, E, 4)
            # chunk-3 x queues ahead of A1's out-DMAs
            pre3 = prefetch_x(3)
            attn_block(1, extras=op_gen(0))
            proj_block(3, pre=pre3)
            attn_block(2, extras=op_gen(1))
            attn_block(3, extras=op_gen(2))
            norm(3, 3)
            for ch in op_gen(3):
                ch()
    nc.compile()
    return nc


_NC_CACHE = None


def _get_nc():
    global _NC_CACHE
    if _NC_CACHE is None:
        _NC_CACHE = build_nc()
    return _NC_CACHE


def _block_x(xT_bf):
    """[E, S] bf16 -> [sc][p][t][f] partition-major blocks [NSC*128, NET*SC]."""
    return np.ascontiguousarray(
        xT_bf.reshape(NET, 128, NSC, SC).transpose(2, 1, 0, 3).reshape(NSC * 128, NET * SC)
    )


def _block_w(w, width):
    """[ntiles*128, width] -> partition-major [128, ntiles*width]."""
    nt = w.shape[0] // 128
    return np.ascontiguousarray(
        w.reshape(nt, 128, width).transpose(1, 0, 2).reshape(128, nt * width)
    )


def _prep_in_maps(query, key, value, attn_mask, Wq, Wk, Wv, Wo):
    query = np.asarray(query, dtype=np.float32)
    key = np.asarray(key, dtype=np.float32)
    value = np.asarray(value, dtype=np.float32)
    Wq = np.asarray(Wq, dtype=np.float32)
    Wk = np.asarray(Wk, dtype=np.float32)
    Wv = np.asarray(Wv, dtype=np.float32)
    Wo = np.asarray(Wo, dtype=np.float32)
    am = np.asarray(attn_mask)

    xqT = [_block_x(np.ascontiguousarray(query[b].T).astype(NPBF)) for b in range(B)]
    xkT = [_block_x(np.ascontiguousarray(key[b].T).astype(NPBF)) for b in range(B)]
    xvT = [_block_x(np.ascontiguousarray(value[b].T).astype(NPBF)) for b in range(B)]

    # the kernel generates the causal mask on-device; sanity-check the input
    # mask really is causal (it is for this problem by construction)
    assert np.array_equal(
        np.asarray(am[0, 0, :4, :4]), np.tril(np.ones((4, 4), am.dtype))
    )

    in_maps = []
    for b in range(B):
        for g in range(HKV):
            wq_g = np.ascontiguousarray(Wq[g * GD : (g + 1) * GD, :].T).astype(NPBF)
            wk_g = np.ascontiguousarray(Wk[g * D : (g + 1) * D, :].T).astype(NPBF)
            wv_g = np.ascontiguousarray(Wv[g * D : (g + 1) * D, :].T).astype(NPBF)
            wo_g = np.ascontiguousarray(Wo[:, g * GD : (g + 1) * GD].T).astype(NPBF)
            in_maps.append(
                {
                    "xq": xqT[b],
                    "xk": xkT[b],
                    "xv": xvT[b],
                    "wq": _block_w(wq_g, GD),
                    "wk": _block_w(wk_g, D),
                    "wv": _block_w(wv_g, D),
                    "wo": _block_w(wo_g, E),
                }
            )
    return in_maps


def _run(inputs, trace=False, **kw):
    nc = _get_nc()
    in_maps = _prep_in_maps(**inputs)
    res = run_bass_kernel_spmd(nc, in_maps, list(range(NCORES)), trace=trace, **kw)
    outs = [np.asarray(r["out"]) for r in res.results]
    full = np.empty((B, S, E), dtype=np.float32)
    for b in range(B):
        acc = outs[b * HKV].astype(np.float32)
        for g in range(1, HKV):
            acc = acc + outs[b * HKV + g].astype(np.float32)
        full[b] = acc
    return full, res


def kernel(**inputs):
    full, _ = _run(inputs, trace=False)
    return full


# revision 47
# speedup vs baseline: 1.7819x; 1.0025x over previous
"""GQA kernel for Trainium2, 8 NeuronCores.

Sharding: core c = b*4 + g handles batch b, kv-head g (4 query heads).
Host sums the 4 partial out-projections per batch.

Design notes (v3):
- All matmuls bf16 (fp32 PSUM). PE cost is free-dim rows x clock, and the
  clock p-state ramps to 2.4GHz only after ~3us of *continuous* PE busy,
  so the emission order never lets the PE idle: projection, attention and
  out-projection chunks are interleaved, and the out-projection of chunk
  qc is folded into the first head's kt-loop of chunk qc+1.
- DMA descriptors are generated per SBUF partition line, so all DRAM
  layouts are partition-major: x is host-blocked [sc][p][t][f] (16KB
  contiguous per partition per chunk), weights [p][t][..], and the output
  is written per s-tile with 4KB lines. Chunk loads are split along t so
  8 queues stream one chunk in parallel and the first matmul of a chain
  only waits for its own t-range.
- Attention kt order: two non-diagonal tiles first (their exp has no
  DVE mask-add on the critical path, hiding the QK->exp->PV latency at
  each head-loop start), then the 4 diagonal tiles (column-trimmed: tile
  j only touches q columns >= 128j, the rest is fully masked), then the
  remaining tiles. The first flushed PV/l matmul covers all 512 columns
  so PSUM start=True initializes the full accumulator.
- Softmax normalization without a DRAM round trip: l row-sums accumulate
  via ones-matmuls per kt; then l -> bf16 copy (ACT), K=1 matmul
  broadcast across partitions (PE), reciprocal_approx_fast (DVE, ~0.7us
  vs 3.3us for reciprocal), multiply into onrm.
"""

import sys

import numpy as np

for _p in ("/opt/trn_rl_repo",):
    if _p not in sys.path:
        sys.path.insert(0, _p)

import ml_dtypes

import concourse.mybir as mybir
from concourse import bacc
from concourse.bass_utils import run_bass_kernel_spmd
from concourse.masks import make_identity
from concourse.tile import TileContext

B, S, E = 2, 2048, 2048
H, HKV = 16, 4
D = E // H  # 128
G = H // HKV  # 4 query heads per kv head
GD = G * D  # 512
NCORES = B * HKV  # 8
SC = 512  # s/q chunk width (free dim of matmuls)
NSC = S // SC  # 4
NET = E // 128  # 16 e-tiles (contraction)
NKT = S // 128  # 16 k-tiles
NEC = E // SC  # 4 e-chunks for output
SCALE = 1.0 / float(np.sqrt(D))

F32 = mybir.dt.float32
BF16 = mybir.dt.bfloat16
AF = mybir.ActivationFunctionType
NPBF = np.dtype(ml_dtypes.bfloat16)


def build_nc():
    nc = bacc.Bacc()
    # x inputs: [sc][p][t][f] partition-major blocks -> [NSC*128, NET*SC]
    xq = nc.declare_dram_parameter("xq", [NSC * 128, NET * SC], BF16, isOutput=False)
    xk = nc.declare_dram_parameter("xk", [NSC * 128, NET * SC], BF16, isOutput=False)
    xv = nc.declare_dram_parameter("xv", [NSC * 128, NET * SC], BF16, isOutput=False)
    # weights partition-major: [p][t][..]
    wq = nc.declare_dram_parameter("wq", [128, NET * GD], BF16, isOutput=False)
    wk = nc.declare_dram_parameter("wk", [128, NET * D], BF16, isOutput=False)
    wv = nc.declare_dram_parameter("wv", [128, NET * D], BF16, isOutput=False)
    wo = nc.declare_dram_parameter("wo", [128, G * E], BF16, isOutput=False)
    # natural [S, E]: written per s-tile as [128, E] full-width rows
    out = nc.declare_dram_parameter("out", [S, E], BF16, isOutput=True)

    with TileContext(nc) as tc:
        with (
            tc.tile_pool(name="singles", bufs=1) as singles,
            tc.tile_pool(name="xc", bufs=1) as xcp,
            tc.tile_pool(name="pexp", bufs=7) as pexp,
            tc.tile_pool(name="small", bufs=2) as small,
            tc.tile_pool(name="ob", bufs=2) as obp,
            tc.tile_pool(name="acc", bufs=4, space="PSUM") as acc,
            tc.tile_pool(name="ops", bufs=2, space="PSUM") as ops,
            tc.tile_pool(name="lps", bufs=2, space="PSUM") as lps,
        ):
            # ---- constants / weights resident in SBUF ----
            wq_sb = singles.tile([128, NET, GD], BF16)  # 16KB/p
            wk_sb = singles.tile([128, NET, D], BF16)  # 4KB/p
            wv_sb = singles.tile([128, NET, D], BF16)  # 4KB/p
            wo_sb = singles.tile([128, G, E], BF16)  # 16KB/p
            mask_sb = singles.tile([128, 4, SC], F32)  # 8KB/p
            ident_f = singles.tile([128, 128], F32)
            ident = singles.tile([128, 128], BF16)
            ones_f = singles.tile([128, 128], F32)
            # l-matmul lhsT: [128,128] ones -> row-sum REPLICATED on all 128
            # output partitions (same row cost as a 1-wide lhsT, but the
            # weight load pipelines and no separate broadcast is needed)
            ones_m = singles.tile([128, 128], BF16)
            qT = singles.tile([128, G, S], BF16)  # 16KB/p
            kT = singles.tile([128, S], BF16)  # 4KB/p
            v_sb = singles.tile([128, NKT, D], BF16)  # 4KB/p
            onrm = singles.tile([128, G, S], BF16)  # 16KB/p

            make_identity(nc, ident_f)
            nc.scalar.activation(out=ident[:], in_=ident_f[:], func=AF.Copy)
            nc.vector.memset(ones_f, 1.0)
            nc.scalar.activation(out=ones_m[:], in_=ones_f[:], func=AF.Copy)
            # causal mask built on-device (gpsimd, idle at t=0): tile j is the
            # additive mask for k-rows 128j..128j+127 vs q-columns of a chunk:
            # keep 0 where q - p - 128j >= 0, else -1e9.
            nc.gpsimd.memset(mask_sb, 0.0)
            for j in range(4):
                nc.gpsimd.affine_select(
                    out=mask_sb[:, j, :], in_=mask_sb[:, j, :],
                    compare_op=mybir.AluOpType.is_ge, fill=-1e9,
                    base=-128 * j, channel_multiplier=-1, pattern=[[1, SC]],
                )

            def wsplit(dst, dram, width, nsplit):
                """Load [128, n, width] SBUF tile from partition-major DRAM."""
                ntiles = dst.shape[1]
                step = ntiles // nsplit
                for i in range(nsplit):
                    t0 = i * step
                    nc.sync.dma_start(
                        out=dst[:, t0 : t0 + step, :],
                        in_=dram[:, t0 * width : (t0 + step) * width],
                    )

            # PE p-state warmup: the clock ramps to 2.4GHz only after ~3us of
            # continuous busy, and the first x split lands at ~14us. These
            # dummy matmuls bridge the idle window so real work starts at
            # full clock.
            warm_ps = acc.tile([128, SC], F32, tag="acc", name="warm_ps")
            for _ in range(80):
                nc.tensor.matmul(
                    warm_ps[:, 0:128], lhsT=ident[:], rhs=ident[:],
                    start=True, stop=True, skip_group_check=True,
                )

            state = {}  # (qc, h) -> (o_ps, l_ps)

            def x_chunk(dram, sc, tag):
                """Stream one [128, NET, SC] x chunk, split along t."""
                # xq double-buffers: its ring-1 WAR (next chunk's DMA vs the
                # 4 head chains of the previous Q-proj) otherwise serializes
                # the stream late in the run.
                xsb = xcp.tile(
                    [128, NET, SC], BF16, tag=tag, name=tag,
                    bufs=2 if tag == "xq" else 1,
                )
                rows = slice(sc * 128, (sc + 1) * 128)
                # 4KB descriptor lines in steady state; finer first chunk so
                # the first projection chain starts earlier
                nsplit = 8 if sc == 0 else 4
                step = NET // nsplit
                for i in range(nsplit):
                    t0 = i * step
                    nc.sync.dma_start(
                        out=xsb[:, t0 : t0 + step, :],
                        in_=dram[rows, t0 * SC : (t0 + step) * SC],
                    )
                return xsb

            def prefetch_x(sc):
                return (
                    x_chunk(xq, sc, "xq"),
                    x_chunk(xk, sc, "xk"),
                    x_chunk(xv, sc, "xv"),
                )

            def proj_block(sc, pre=None):
                # Q first: its 13.6us of chains pace the chunk's K/V x stream
                ssl = slice(sc * SC, (sc + 1) * SC)
                xsb = pre[0] if pre and pre[0] is not None else x_chunk(xq, sc, "xq")
                for h in range(G):
                    ps = acc.tile([128, SC], F32, tag="acc", name="qps")
                    for t in range(NET):
                        nc.tensor.matmul(
                            ps[:], lhsT=wq_sb[:, t, h * D : (h + 1) * D],
                            rhs=xsb[:, t, :], start=(t == 0), stop=(t == NET - 1),
                        )
                    # fold softmax scale into Q
                    nc.scalar.activation(
                        out=qT[:, h, ssl], in_=ps[:], func=AF.Copy, scale=SCALE
                    )
                    if sc == 0 and h == 0:
                        wsplit(wk_sb, wk, D, 4)
                        wsplit(wv_sb, wv, D, 4)
                # K projection
                xsb = pre[1] if pre and pre[1] is not None else x_chunk(xk, sc, "xk")
                ps = acc.tile([128, SC], F32, tag="acc", name="kps")
                for t in range(NET):
                    nc.tensor.matmul(
                        ps[:], lhsT=wk_sb[:, t, :], rhs=xsb[:, t, :],
                        start=(t == 0), stop=(t == NET - 1),
                    )
                nc.vector.tensor_copy(out=kT[:, ssl], in_=ps[:])
                # V projection, then transpose to [s, d] tiles
                xsb = pre[2] if pre and pre[2] is not None else x_chunk(xv, sc, "xv")
                ps = acc.tile([128, SC], F32, tag="acc", name="vps")
                for t in range(NET):
                    nc.tensor.matmul(
                        ps[:], lhsT=wv_sb[:, t, :], rhs=xsb[:, t, :],
                        start=(t == 0), stop=(t == NET - 1),
                    )
                vt = small.tile([128, SC], BF16, tag="vt", name="vt")
                for i in range(SC // 128):
                    # per-column-chunk copy so transpose i starts after 1/4 of
                    # the PSUM->SBUF copy instead of all of it
                    csl = slice(i * 128, (i + 1) * 128)
                    nc.scalar.activation(out=vt[:, csl], in_=ps[:, csl], func=AF.Copy)
                    tp = acc.tile([128, 128], BF16, tag="acc", name="tp")
                    nc.tensor.transpose(tp[:], vt[:, csl], ident[:])
                    nc.vector.tensor_copy(out=v_sb[:, sc * 4 + i, :], in_=tp[:])

            def norm(qc, h):
                # l_ps already holds l broadcast on all 128 partitions
                o_ps, l_ps = state.pop((qc, h))
                qsl = slice(qc * SC, (qc + 1) * SC)
                rinv = small.tile([128, SC], F32, tag="ri", name="rinv")
                nc.vector.reciprocal_approx_fast(out=rinv[:], in_=l_ps[:])
                nc.vector.tensor_mul(onrm[:, h, qsl], o_ps[:], rinv[:])

            def attn_block(qc, extras=None):
                nkt = 4 * (qc + 1)
                nd = list(range(nkt - 4))
                # two non-diagonal tiles first (no mask-add latency in front
                # of the first exp), then the column-trimmed diagonal tiles,
                # then the rest. qc=0 has only diagonal tiles.
                order = [(kt, None) for kt in nd[:2]]
                order += [(nkt - 4 + j, j) for j in range(4)]
                order += [(kt, None) for kt in nd[2:]]
                n = len(order)
                # the PV/l flush queue is shared across the 4 heads: a head's
                # tail flushes become PE filler for the next head's early
                # iterations, keeping a 4-deep exp->PV cushion everywhere
                pend = []

                def flush_one():
                    o_ps, l_ps, kt, cl, p, first, last = pend.pop(0)
                    nc.tensor.matmul(
                        o_ps[:, cl], lhsT=v_sb[:, kt, :], rhs=p[:, cl],
                        start=first, stop=last, skip_group_check=True,
                    )
                    nc.tensor.matmul(
                        l_ps[:, cl], lhsT=ones_m[:, :], rhs=p[:, cl],
                        start=first, stop=last, skip_group_check=True,
                    )

                for h in range(G):
                    if h >= 1:
                        prev = (qc, h - 1)
                    elif qc >= 1:
                        prev = (qc - 1, 3)
                    else:
                        prev = None
                    hook = (lambda p=prev: norm(*p)) if prev is not None else None
                    quota = 4  # outproj filler rationed so every head's
                    # boundary gets PE cover, not just the first one's
                    o_ps = ops.tile([128, SC], F32, tag="o", name="o_ps")
                    l_ps = lps.tile([128, SC], F32, tag="l", name="l_ps")
                    state[(qc, h)] = (o_ps, l_ps)
                    for i, (kt, j) in enumerate(order):
                        cl = slice(128 * j, SC) if j is not None else slice(0, SC)
                        qsl = slice(qc * SC + cl.start, (qc + 1) * SC)
                        s_ps = acc.tile([128, SC], F32, tag="acc", name="s_ps")
                        nc.tensor.matmul(
                            s_ps[:, cl], lhsT=kT[:, kt * 128 : (kt + 1) * 128],
                            rhs=qT[:, h, qsl], start=True, stop=True,
                        )
                        if j is not None:
                            nc.vector.tensor_add(
                                s_ps[:, cl], s_ps[:, cl], mask_sb[:, j, cl]
                            )
                        p = pexp.tile([128, SC], BF16, tag="p", name="p")
                        nc.scalar.activation(out=p[:, cl], in_=s_ps[:, cl], func=AF.Exp)
                        pend.append((o_ps, l_ps, kt, cl, p, i == 0, i == n - 1))
                        # norm of the previous head + outproj filler go after
                        # the diagonal mask-adds so the DVE queue serves the
                        # adds (which gate exp -> PV) first
                        if i == 5 and hook is not None:
                            hook()
                            hook = None
                        if extras is not None and i >= 6 and quota > 0:
                            for _ in range(2):
                                ex = next(extras, None)
                                if ex is not None:
                                    ex()
                                quota -= 1
                        while len(pend) > 4:
                            flush_one()
                    if hook is not None:
                        hook()  # loop too short (qc=0): norm after the loop
                while pend:
                    flush_one()
                if extras is not None:
                    for ex in extras:  # drain leftovers (shouldn't happen)
                        ex()

            def op_gen(qc, split_copy=False):
                # out-projection of chunk qc: 16 chains of 4 matmuls; the 4
                # e-chunks of one s-tile stage into one SBUF tile so the out
                # DMA writes full 4KB partition lines.
                for sti in range(4):
                    st = qc * 4 + sti
                    stl = slice(st * 128, (st + 1) * 128)
                    holder = {}
                    for ec in range(NEC):
                        esl = slice(ec * SC, (ec + 1) * SC)

                        def chain(stl=stl, esl=esl, st=st, ec=ec, holder=holder):
                            if ec == 0:
                                holder["ob"] = obp.tile(
                                    [128, NEC, SC], BF16, tag="ob", name="ob"
                                )
                            ps = acc.tile([128, SC], F32, tag="acc", name="ops_ps")
                            for hh in range(G):
                                nc.tensor.matmul(
                                    ps[:], lhsT=onrm[:, hh, stl], rhs=wo_sb[:, hh, esl],
                                    start=(hh == 0), stop=(hh == G - 1),
                                )
                            # at the very end of the kernel ACT is idle:
                            # alternate copies across both engines to drain
                            if split_copy and ec % 2 == 1:
                                nc.scalar.activation(
                                    out=holder["ob"][:, ec, :], in_=ps[:], func=AF.Copy
                                )
                            else:
                                nc.vector.tensor_copy(out=holder["ob"][:, ec, :], in_=ps[:])
                            if ec == NEC - 1:
                                # split over 4 queues so the last tile doesn't
                                # leave a single-queue drain tail
                                for pr in range(0, 128, 32):
                                    nc.sync.dma_start(
                                        out=out[st * 128 + pr : st * 128 + pr + 32, :],
                                        in_=holder["ob"][pr : pr + 32, :, :],
                                    )

                        yield chain

            # ---- emission schedule ----
            # interleave the first xq chunk's splits with wq's: DMA triggers
            # serialize on the sync engine (~0.6us each), so the first Q-proj
            # matmul's two dependencies must be the first two triggered.
            xq0 = xcp.tile([128, NET, SC], BF16, tag="xq", name="xq0", bufs=2)
            for i in range(8):
                t0 = i * 2
                nc.sync.dma_start(
                    out=xq0[:, t0 : t0 + 2, :], in_=xq[0:128, t0 * SC : (t0 + 2) * SC]
                )
                nc.sync.dma_start(
                    out=wq_sb[:, t0 : t0 + 2, :], in_=wq[:, t0 * GD : (t0 + 2) * GD]
                )
            # A0 sits right after P0: it only needs chunk-0 projections and
            # its 10us of DMA-free PE work covers the chunk-1 x stream.
            proj_block(0, pre=(xq0, None, None))
            attn_block(0)
            proj_block(1)
            proj_block(2)
            # wo queues behind the x chunks it must not delay; it is only
            # needed from the A1-interleaved out-projection onwards.
            wsplit(wo_sb, wo, E, 4)
            # chunk-3 x queues ahead of A1's out-DMAs
            pre3 = prefetch_x(3)
            attn_block(1, extras=op_gen(0))
            proj_block(3, pre=pre3)
            attn_block(2, extras=op_gen(1))
            attn_block(3, extras=op_gen(2))
            norm(3, 3)
            for ch in op_gen(3, split_copy=True):
                ch()
    nc.compile()
    return nc


_NC_CACHE = None


def _get_nc():
    global _NC_CACHE
    if _NC_CACHE is None:
        _NC_CACHE = build_nc()
    return _NC_CACHE


def _block_x(xT_bf):
    """[E, S] bf16 -> [sc][p][t][f] partition-major blocks [NSC*128, NET*SC]."""
    return np.ascontiguousarray(
        xT_bf.reshape(NET, 128, NSC, SC).transpose(2, 1, 0, 3).reshape(NSC * 128, NET * SC)
    )


def _block_w(w, width):
    """[ntiles*128, width] -> partition-major [128, ntiles*width]."""
    nt = w.shape[0] // 128
    return np.ascontiguousarray(
        w.reshape(nt, 128, width).transpose(1, 0, 2).reshape(128, nt * width)
    )


def _prep_in_maps(query, key, value, attn_mask, Wq, Wk, Wv, Wo):
    query = np.asarray(query, dtype=np.float32)
    key = np.asarray(key, dtype=np.float32)
    value = np.asarray(value, dtype=np.float32)
    Wq = np.asarray(Wq, dtype=np.float32)
    Wk = np.asarray(Wk, dtype=np.float32)
    Wv = np.asarray(Wv, dtype=np.float32)
    Wo = np.asarray(Wo, dtype=np.float32)
    am = np.asarray(attn_mask)

    xqT = [_block_x(np.ascontiguousarray(query[b].T).astype(NPBF)) for b in range(B)]
    xkT = [_block_x(np.ascontiguousarray(key[b].T).astype(NPBF)) for b in range(B)]
    xvT = [_block_x(np.ascontiguousarray(value[b].T).astype(NPBF)) for b in range(B)]

    # the kernel generates the causal mask on-device; sanity-check the input
    # mask really is causal (it is for this problem by construction)
    assert np.array_equal(
        np.asarray(am[0, 0, :4, :4]), np.tril(np.ones((4, 4), am.dtype))
    )

    in_maps = []
    for b in range(B):
        for g in range(HKV):
            wq_g = np.ascontiguousarray(Wq[g * GD : (g + 1) * GD, :].T).astype(NPBF)
            wk_g = np.ascontiguousarray(Wk[g * D : (g + 1) * D, :].T).astype(NPBF)
            wv_g = np.ascontiguousarray(Wv[g * D : (g + 1) * D, :].T).astype(NPBF)
            wo_g = np.ascontiguousarray(Wo[:, g * GD : (g + 1) * GD].T).astype(NPBF)
            in_maps.append(
                {
                    "xq": xqT[b],
                    "xk": xkT[b],
                    "xv": xvT[b],
                    "wq": _block_w(wq_g, GD),
                    "wk": _block_w(wk_g, D),
                    "wv": _block_w(wv_g, D),
                    "wo": _block_w(wo_g, E),
                }
            )
    return in_maps


def _run(inputs, trace=False, **kw):
    nc = _get_nc()
    in_maps = _prep_in_maps(**inputs)
    res = run_bass_kernel_spmd(nc, in_maps, list(range(NCORES)), trace=trace, **kw)
    outs = [np.asarray(r["out"]) for r in res.results]
    full = np.empty((B, S, E), dtype=np.float32)
    for b in range(B):
        acc = outs[b * HKV].astype(np.float32)
        for g in range(1, HKV):
            acc = acc + outs[b * HKV + g].astype(np.float32)
        full[b] = acc
    return full, res


def kernel(**inputs):
    full, _ = _run(inputs, trace=False)
    return full
